# revision 1
# baseline (speedup 1.0000x reference)
"""GNN message-passing aggregator on 8 Trainium2 NeuronCores.

Computes, for the full graph:
    node = entity_embed * out_sqrt_degree
    msg  = node[src] * edge_weight
    N_h  = segment_sum(msg, dst, N) * in_sqrt_degree
    out  = leaky_relu((entity_embed + N_h) @ W.T + b, 0.01)

Strategy (dst-partitioned edge shard, no collectives):
  * Host: fold degree factors into the gather table / edge weights, sort
    edges by dst, cut the node space into 392 aligned 128-node tiles,
    and deal tiles onto 8 cores x 49 slots so every core runs the same
    instruction stream (SPMD: one program, per-core data).
  * Device, per 128-node tile: dma_gather the source rows (256B each)
    from the replicated node table, build a one-hot selection matrix
    S[e, n] = (dst_local[e] == n) against a constant iota, weight the
    gathered rows by edge weight, and accumulate N_h.T += gm.T @ S on
    the PE in PSUM.  The dst-sorted edge order makes each tile's edges
    contiguous, so no scatter and no cross-core reduction is needed.
    The small linear runs per tile: out = leaky(X @ W.T + b) via two
    more matmuls (ones-row trick for the bias), leaky = max(x, 0.01x).
  * src indices must fit int16 for dma_gather, so the 50176-row table is
    gathered as two 25088-row halves; each tile's edge list is split by
    src half (A: src < 25088, B: src >= 25088), each half padded to a
    multiple of 128 edges with (idx=0, w=0) no-op edges.
  * dma_gather descriptor generation runs on one Q7 core pair selected
    by queue_num (~9.5ns/idx on the pair) — the whole kernel is bound by
    it.  The per-core A/B edge streams are chunked into 8-block
    (1024-edge) gathers independent of slot boundaries and cycled over
    4 SWDGE queues so up to four generators run concurrently
    (single_packet=False lifts the 64-descriptor packet cap).
  * DVE per-op fixed costs dominate per-block elementwise work, so the
    one-hot build and the edge-weight multiply are batched per chunk as
    single tensor_tensor ops over [128, nblk*128] / [128, nblk*64] using
    0-stride broadcast APs of the per-block scalars.
"""

import json
import sys
import types

import numpy as np

P = 128
D = 64
N_NODES = 50000
N_CORES = 8
HALF = 25088            # int16-safe gather-table half (196 * 128)
NPAD = 2 * HALF         # 50176 = 392 tiles
NT = NPAD // P          # 392
SLOTS = NT // N_CORES   # 49
CHUNK = 8               # blocks per dma_gather
NQ = 4                  # SWDGE queues (Q7 core pairs)


# ----------------------------------------------------------------------------
# Environment fixups (self-contained; kernel.py must run alone).
# ----------------------------------------------------------------------------

_SPLIT_COUNT = 0


def _split_multi_waits_json(bir: bytes) -> bytes:
    """This container's walrus accepts only ONE sync wait per instruction
    ('Too many sync wait commands'), while Tile's scheduler attaches
    several.  Rewrite each instruction with N>1 waits into N-1 same-engine
    NoOps (one wait each) followed by the instruction with the last wait;
    same-engine sequencer order makes this equivalent."""
    global _SPLIT_COUNT
    d = json.loads(bir)
    changed = False
    for fn in d.get("functions", []):
        for bb in fn.get("blocks", []):
            out = []
            for ins in bb.get("instructions", []):
                si = ins.get("sync_info") or {}
                ow = si.get("on_wait") or []
                if len(ow) > 1:
                    changed = True
                    for w in ow[:-1]:
                        _SPLIT_COUNT += 1
                        out.append({
                            "opcode": "NoOp",
                            "engine": ins.get("engine", "Unassigned"),
                            "name": f"I-waitsplit-{_SPLIT_COUNT}",
                            "ins": [],
                            "outs": [],
                            "sync_info": {"on_update": [], "on_wait": [w]},
                        })
                    si["on_wait"] = [ow[-1]]
                out.append(ins)
            bb["instructions"] = out
    return json.dumps(d).encode() if changed else bir


def _install_fixups():
    import concourse.bass_utils as bass_utils
    import concourse.bass2jax as bass2jax

    if not getattr(bass_utils, "_waitsplit_installed", False):
        bass_utils._waitsplit_installed = True
        orig_compile = bass_utils.compile_bir_kernel

        def patched_compile(bir_json, tmpdir, neff_name="file.neff"):
            if isinstance(bir_json, str):
                bir_json = bir_json.encode()
            return orig_compile(_split_multi_waits_json(bir_json), tmpdir,
                                neff_name=neff_name)

        bass_utils.compile_bir_kernel = patched_compile
        bass2jax.compile_bir_kernel = patched_compile
        # No artifact bucket in this container; keep profiles local.
        bass_utils.upload_artifacts = lambda tmpdir: tmpdir

    # run_bass_kernel_spmd(trace=True) under axon needs antenv.axon_hooks,
    # which this image doesn't ship.  Synthesize it and install the ctypes
    # NTFF hook from trn_agent_boot so neuron-profile works.
    if "antenv.axon_hooks" not in sys.modules:
        m = types.ModuleType("antenv.axon_hooks")
        m._hook = None
        m.set_axon_ntff_profile_hook = lambda h: setattr(m, "_hook", h)
        m.get_axon_ntff_profile_hook = lambda: m._hook
        sys.modules["antenv.axon_hooks"] = m
        try:
            import antenv
            antenv.axon_hooks = m
        except ImportError:
            pass
        try:
            from trn_agent_boot.trn_boot import _ntff_profile_via_ctypes
            hook = _ntff_profile_via_ctypes("/opt/axon/libaxon_pjrt.so")
            if hook is not None:
                m._hook = hook
        except Exception:
            pass


# ----------------------------------------------------------------------------
# Host-side graph partitioning
# ----------------------------------------------------------------------------

def _wrap16(seg: np.ndarray) -> np.ndarray:
    """dma_gather index layout: index i lives at [i % 16, i // 16]."""
    assert seg.size % 16 == 0
    return seg.reshape(-1, 16).T


def _prepare(entity_embed, src, dst, edge_weight, out_sqrt_degree,
             in_sqrt_degree):
    f32 = np.float32
    node = (entity_embed * out_sqrt_degree).astype(f32)
    node_pad = np.zeros((NPAD, D), f32)
    node_pad[:N_NODES] = node
    embT_pad = np.zeros((D, NPAD), f32)
    embT_pad[:, :N_NODES] = entity_embed.astype(f32).T

    ew2 = (edge_weight[:, 0] * in_sqrt_degree[dst, 0]).astype(f32)

    order = np.argsort(dst, kind="stable")
    sdst = dst[order].astype(np.int64)
    ssrc = src[order].astype(np.int64)
    sew = ew2[order]

    counts = np.bincount(sdst // P, minlength=NT)
    starts = np.concatenate([[0], np.cumsum(counts)])

    # Per tile: split by gather-table half, count padded 128-edge blocks.
    tiles = []
    for t in range(NT):
        lo, hi = starts[t], starts[t + 1]
        t_src = ssrc[lo:hi]
        t_dstl = (sdst[lo:hi] - t * P).astype(f32)
        t_ew = sew[lo:hi]
        a = t_src < HALF
        tiles.append((t, t_src[a], t_dstl[a], t_ew[a],
                      t_src[~a] - HALF, t_dstl[~a], t_ew[~a]))
    ba = np.array([-(-len(x[1]) // P) for x in tiles])
    bb = np.array([-(-len(x[4]) // P) for x in tiles])

    # Deal tiles into 49 slots x 8 cores; similar (BA, BB) tiles share a
    # slot so the per-slot max padding stays small.  Slot block counts are
    # shared by all cores (one SPMD program).
    rank = sorted(range(NT), key=lambda t: (-ba[t], -bb[t]))
    slot_ba = np.zeros(SLOTS, np.int64)
    slot_bb = np.zeros(SLOTS, np.int64)
    tile_of = np.zeros((N_CORES, SLOTS), np.int64)
    for s in range(SLOTS):
        octet = rank[s * N_CORES:(s + 1) * N_CORES]
        slot_ba[s] = max(ba[t] for t in octet)
        slot_bb[s] = max(bb[t] for t in octet)
        for c, t in enumerate(octet):
            tile_of[c, s] = t

    ta = int(slot_ba.sum())           # A-stream blocks per core
    tbb = int(slot_bb.sum())          # B-stream blocks per core
    tb = ta + tbb
    # idx columns padded so each stream's chunks are CHUNK-aligned
    CH = CHUNK
    acols = 8 * CH * (-(-ta // CH))
    bcols = 8 * CH * (-(-tbb // CH))
    icols = acols + bcols

    idx_all = np.zeros((N_CORES, 16, icols), np.int16)
    dstl_all = np.zeros((N_CORES, P, tb), f32)
    ew_all = np.zeros((N_CORES, P, tb), f32)
    embT_all = np.zeros((N_CORES, D, SLOTS * P), f32)

    a0 = np.concatenate([[0], np.cumsum(slot_ba)])   # A-stream block offsets
    b0 = np.concatenate([[0], np.cumsum(slot_bb)])   # B-stream block offsets

    for c in range(N_CORES):
        for s in range(SLOTS):
            t, srcA, dstlA, ewA, srcB, dstlB, ewB = tiles[tile_of[c, s]]
            nA, nB = P * slot_ba[s], P * slot_bb[s]
            ia = np.zeros(nA, np.int16)
            ia[:len(srcA)] = srcA
            ib = np.zeros(nB, np.int16)
            ib[:len(srcB)] = srcB
            # idx columns: A-stream first [0, acols), then B-stream.
            ca = 8 * a0[s]
            cb = acols + 8 * b0[s]
            if nA:
                idx_all[c, :, ca:ca + nA // 16] = _wrap16(ia)
            if nB:
                idx_all[c, :, cb:cb + nB // 16] = _wrap16(ib)
            # dstl/ew columns: A-block (s,j) -> a0[s]+j; B-block -> ta+b0[s]+j
            if slot_ba[s]:
                dl = np.zeros(nA, f32)
                dl[:len(dstlA)] = dstlA
                ws = np.zeros(nA, f32)
                ws[:len(ewA)] = ewA
                k = a0[s]
                dstl_all[c, :, k:k + slot_ba[s]] = dl.reshape(-1, P).T
                ew_all[c, :, k:k + slot_ba[s]] = ws.reshape(-1, P).T
            if slot_bb[s]:
                dl = np.zeros(nB, f32)
                dl[:len(dstlB)] = dstlB
                ws = np.zeros(nB, f32)
                ws[:len(ewB)] = ewB
                k = ta + b0[s]
                dstl_all[c, :, k:k + slot_bb[s]] = dl.reshape(-1, P).T
                ew_all[c, :, k:k + slot_bb[s]] = ws.reshape(-1, P).T
            embT_all[c, :, s * P:(s + 1) * P] = embT_pad[:, t * P:(t + 1) * P]

    idx_rep = np.tile(idx_all, (1, 8, 1))  # replicate across the 8 Q7 cores
    sig = (tuple(int(x) for x in slot_ba), tuple(int(x) for x in slot_bb))
    return (node_pad, idx_rep, dstl_all, ew_all, embT_all, tile_of, sig, tb,
            icols)


# ----------------------------------------------------------------------------
# Device program
# ----------------------------------------------------------------------------

_PROGRAM_CACHE = {}


class _GatherStream:
    """Lazily emits chunked (<=CHUNK blocks) dma_gathers over one table
    half's concatenated block stream.  Per chunk it also emits ONE batched
    edge-weight multiply (gm = g * ew) and ONE batched one-hot build
    (S[:, j, n] = (iota[n] == dstl[j])) so the DVE cost is amortized over
    CHUNK blocks.  block(i) returns (lhsT, rhs) APs for block i."""

    def __init__(self, nc, mybir, pool, gmpool, spool, table_ap, idx_segs,
                 blk_col0, total_blocks, dstl_sb, ew_sb, iota_rep,
                 qpick, f32):
        self.nc = nc
        self.mybir = mybir
        self.pool = pool
        self.gmpool = gmpool
        self.spool = spool
        self.table_ap = table_ap
        # idx_segs: list of (tile, chunk0, nchunks); chunk k's 8*CHUNK idx
        # columns live in its group's tile at offset (k - chunk0)*8*CHUNK.
        self.idx_segs = idx_segs
        self.blk_col0 = blk_col0   # column offset into dstl/ew for block 0
        self.total = total_blocks
        self.dstl_sb = dstl_sb
        self.ew_sb = ew_sb
        self.iota_rep = iota_rep
        self.qpick = qpick
        self.f32 = f32
        self.tiles = []            # chunk index -> (gm tile, S tile)

    def _idx_ap(self, k, cols):
        for t, c0, nch in self.idx_segs:
            if c0 <= k < c0 + nch:
                off = (k - c0) * 8 * CHUNK
                return t[:, off:off + cols]
        raise AssertionError(k)

    def _emit_chunk(self, k):
        nc = self.nc
        nblk = min(CHUNK, self.total - k * CHUNK)
        g = self.pool.tile([P, nblk, D], self.f32)
        n = P * nblk
        nc.gpsimd.dma_gather(
            g[:], self.table_ap, self._idx_ap(k, n // 16), n, n, D,
            queue_num=self.qpick(), single_packet=False)
        b0 = self.blk_col0 + CHUNK * k
        gm = self.gmpool.tile([P, nblk, D], self.f32)
        nc.vector.tensor_tensor(
            out=gm[:], in0=g[:],
            in1=self.ew_sb[:, b0:b0 + nblk].to_broadcast([P, nblk, D]),
            op=self.mybir.AluOpType.mult)
        S = self.spool.tile([P, nblk, P], self.f32)
        nc.vector.tensor_tensor(
            out=S[:],
            in0=self.iota_rep[:, :nblk * P].rearrange(
                "p (k n) -> p k n", n=P),
            in1=self.dstl_sb[:, b0:b0 + nblk].to_broadcast([P, nblk, P]),
            op=self.mybir.AluOpType.is_equal)
        self.tiles.append((gm, S))

    def block(self, i):
        k, off = divmod(i, CHUNK)
        while len(self.tiles) <= k:
            self._emit_chunk(len(self.tiles))
        gm, S = self.tiles[k]
        return gm[:, off, :], S[:, off, :]


def _build_program(sig, tb, icols):
    if sig in _PROGRAM_CACHE:
        return _PROGRAM_CACHE[sig]

    from concourse import bacc
    import concourse.mybir as mybir
    import concourse.tile as tile

    slot_ba, slot_bb = sig
    ta = sum(slot_ba)
    tbb = sum(slot_bb)
    nc = bacc.Bacc("TRN2", num_swdge_queues=NQ)
    f32 = mybir.dt.float32
    t_node = nc.dram_tensor("node", [NPAD, D], f32, kind="ExternalInput")
    t_idx = nc.dram_tensor("idx", [P, icols], mybir.dt.int16,
                           kind="ExternalInput")
    t_dstl = nc.dram_tensor("dstl", [P, tb], f32, kind="ExternalInput")
    t_ew = nc.dram_tensor("ew", [P, tb], f32, kind="ExternalInput")
    t_embT = nc.dram_tensor("embT", [D, SLOTS * P], f32, kind="ExternalInput")
    t_wt = nc.dram_tensor("wt", [D, D], f32, kind="ExternalInput")
    t_b = nc.dram_tensor("bias", [1, D], f32, kind="ExternalInput")
    t_iota = nc.dram_tensor("iota", [P, CHUNK * P], f32,
                            kind="ExternalInput")
    t_out = nc.dram_tensor("out", [SLOTS * P, D], f32, kind="ExternalOutput")

    qstate = [0]

    def qpick():
        q = qstate[0] % NQ
        qstate[0] += 1
        return q

    with tile.TileContext(nc) as tc:
        with tc.tile_pool(name="const", bufs=1) as cpool, \
             tc.tile_pool(name="ga", bufs=4) as gapool, \
             tc.tile_pool(name="gb", bufs=4) as gbpool, \
             tc.tile_pool(name="gma", bufs=3) as gmapool, \
             tc.tile_pool(name="gmb", bufs=3) as gmbpool, \
             tc.tile_pool(name="sa", bufs=3) as sapool, \
             tc.tile_pool(name="sb", bufs=3) as sbpool, \
             tc.tile_pool(name="small", bufs=3) as mpool, \
             tc.tile_pool(name="psnh", bufs=3, space="PSUM") as psnh, \
             tc.tile_pool(name="psout", bufs=2, space="PSUM") as psout:
            # idx group tiles (chunk-aligned) so the first gather only
            # waits on its own small DMA, not the whole index array
            ncha = -(-ta // CHUNK)
            nchb = -(-tbb // CHUNK)
            segs = []
            for c0t, ncht in ((0, ncha), (ncha, nchb)):
                ngrp = min(4, ncht) or 1
                for gidx in range(ngrp):
                    lo = c0t + ncht * gidx // ngrp
                    hi = c0t + ncht * (gidx + 1) // ngrp
                    if hi == lo:
                        continue
                    w = min(hi * 8 * CHUNK, icols // 1) - lo * 8 * CHUNK
                    w = min(w, icols - lo * 8 * CHUNK)
                    tgt = cpool.tile([P, w], mybir.dt.int16,
                                     tag=f"idx{lo}")
                    nc.sync.dma_start(
                        out=tgt[:],
                        in_=t_idx[:, lo * 8 * CHUNK:lo * 8 * CHUNK + w])
                    segs.append((tgt, lo, hi - lo))
            idx_segs_a = [(t, c0, n) for (t, c0, n) in segs if c0 < ncha]
            idx_segs_b = [(t, c0 - ncha, n) for (t, c0, n) in segs
                          if c0 >= ncha]
            dstl_sb = cpool.tile([P, tb], f32)
            ew_sb = cpool.tile([P, tb], f32)
            bnd2 = [tb * i // 4 for i in range(5)]
            for i in range(4):
                nc.sync.dma_start(out=dstl_sb[:, bnd2[i]:bnd2[i + 1]],
                                  in_=t_dstl[:, bnd2[i]:bnd2[i + 1]])
                nc.sync.dma_start(out=ew_sb[:, bnd2[i]:bnd2[i + 1]],
                                  in_=t_ew[:, bnd2[i]:bnd2[i + 1]])
            iota_rep = cpool.tile([P, CHUNK * P], f32)
            nc.scalar.dma_start(out=iota_rep[:], in_=t_iota[:])
            ones = cpool.tile([1, P], f32)
            nc.vector.memset(ones[:], 1.0)
            wt_sb = cpool.tile([D, D], f32)
            nc.scalar.dma_start(out=wt_sb[:], in_=t_wt[:])
            b_sb = cpool.tile([1, D], f32)
            nc.scalar.dma_start(out=b_sb[:], in_=t_b[:])
            embT_sb = cpool.tile([D, SLOTS * P], f32)
            nc.scalar.dma_start(out=embT_sb[:], in_=t_embT[:])

            sa = _GatherStream(nc, mybir, gapool, gmapool, sapool,
                               t_node[0:HALF, :], idx_segs_a, 0, ta,
                               dstl_sb, ew_sb, iota_rep, qpick, f32)
            sb = _GatherStream(nc, mybir, gbpool, gmbpool, sbpool,
                               t_node[HALF:NPAD, :], idx_segs_b, ta, tbb,
                               dstl_sb, ew_sb, iota_rep, qpick, f32)

            a_off = 0
            b_off = 0
            for s in range(SLOTS):
                ba, bb = slot_ba[s], slot_bb[s]
                nb = ba + bb
                blocks = [sa.block(a_off + j) for j in range(ba)]
                blocks += [sb.block(b_off + j) for j in range(bb)]
                a_off += ba
                b_off += bb
                xT = mpool.tile([D, P], f32, tag="xT")
                if nb:
                    nh = psnh.tile([D, P], f32, space="PSUM", tag="nh")
                    for i, (lhsT, rhs) in enumerate(blocks):
                        nc.tensor.matmul(out=nh[:], lhsT=lhsT, rhs=rhs,
                                         start=(i == 0), stop=(i == nb - 1))
                    nc.vector.tensor_add(out=xT[:], in0=nh[:],
                                         in1=embT_sb[:, s * P:(s + 1) * P])
                else:
                    nc.vector.tensor_copy(out=xT[:],
                                          in_=embT_sb[:, s * P:(s + 1) * P])
                o_ps = psout.tile([P, D], f32, space="PSUM", tag="ops")
                nc.tensor.matmul(out=o_ps[:], lhsT=xT[:], rhs=wt_sb[:],
                                 start=True, stop=False)
                nc.tensor.matmul(out=o_ps[:], lhsT=ones[:], rhs=b_sb[:],
                                 start=False, stop=True)
                o_scaled = mpool.tile([P, D], f32, tag="osc")
                nc.vector.tensor_scalar_mul(o_scaled[:], o_ps[:], 0.01)
                o_sb = mpool.tile([P, D], f32, tag="osb")
                nc.vector.tensor_tensor(out=o_sb[:], in0=o_ps[:],
                                        in1=o_scaled[:],
                                        op=mybir.AluOpType.max)
                nc.sync.dma_start(out=t_out[s * P:(s + 1) * P, :], in_=o_sb[:])

    nc.compile()
    _PROGRAM_CACHE[sig] = nc
    return nc


LAST_RESULTS = None


def kernel(entity_embed, src, dst, edge_weight, out_sqrt_degree,
           in_sqrt_degree, W, b):
    _install_fixups()
    from concourse.bass_utils import run_bass_kernel_spmd

    entity_embed = np.asarray(entity_embed, np.float32)
    src = np.asarray(src)
    dst = np.asarray(dst)
    edge_weight = np.asarray(edge_weight, np.float32)
    out_sqrt_degree = np.asarray(out_sqrt_degree, np.float32)
    in_sqrt_degree = np.asarray(in_sqrt_degree, np.float32)
    W = np.asarray(W, np.float32)
    b = np.asarray(b, np.float32)

    (node_pad, idx_rep, dstl_all, ew_all, embT_all, tile_of, sig, tb,
     icols) = _prepare(entity_embed, src, dst, edge_weight, out_sqrt_degree,
                       in_sqrt_degree)

    nc = _build_program(sig, tb, icols)

    wt = np.ascontiguousarray(W.T)          # rhs[k, j] = W[j, k]
    iota_np = np.tile(np.tile(np.arange(P, dtype=np.float32), CHUNK), (P, 1))
    in_maps = []
    for c in range(N_CORES):
        in_maps.append({
            "node": node_pad,
            "idx": np.ascontiguousarray(idx_rep[c]),
            "dstl": np.ascontiguousarray(dstl_all[c]),
            "ew": np.ascontiguousarray(ew_all[c]),
            "embT": np.ascontiguousarray(embT_all[c]),
            "wt": wt,
            "bias": b[None, :],
            "iota": iota_np,
        })

    try:
        res = run_bass_kernel_spmd(nc, in_maps,
                                   core_ids=list(range(N_CORES)))
    except Exception:
        # Transient NRT_EXEC_UNIT_UNRECOVERABLE states have been observed;
        # a reset + retry recovers them.
        import os
        import time
        os.environ["NEURON_RT_RESET_CORES"] = "1"
        time.sleep(30)
        res = run_bass_kernel_spmd(nc, in_maps,
                                   core_ids=list(range(N_CORES)))
    global LAST_RESULTS
    LAST_RESULTS = res

    out = np.empty((NPAD, D), np.float32)
    for c in range(N_CORES):
        oc = res.results[c]["out"]
        for s in range(SLOTS):
            t = tile_of[c, s]
            out[t * P:(t + 1) * P] = oc[s * P:(s + 1) * P]
    return out[:N_NODES]



# revision 2
# speedup vs baseline: 1.1197x; 1.1197x over previous
"""GNN message-passing aggregator on 8 Trainium2 NeuronCores.

Computes, for the full graph:
    node = entity_embed * out_sqrt_degree
    msg  = node[src] * edge_weight
    N_h  = segment_sum(msg, dst, N) * in_sqrt_degree
    out  = leaky_relu((entity_embed + N_h) @ W.T + b, 0.01)

Strategy (dst-partitioned edge shard, no collectives):
  * Host: fold degree factors into the gather table / edge weights, sort
    edges by dst, cut the node space into 392 aligned 128-node tiles,
    and deal tiles onto 8 cores x 49 slots so every core runs the same
    instruction stream (SPMD: one program, per-core data).
  * Device, per 128-node tile: dma_gather the source rows (256B each)
    from the replicated node table, build a one-hot selection matrix
    S[e, n] = (dstl[e] == n) in bf16, weight the gathered rows by edge
    weight into bf16 messages gm, and accumulate
        nh[n, :] += S.T @ gm     (PE, PSUM fp32)
    with S as the 128-column stationary operand so the PE's fast-weight-
    load path hides the weight loads and each block streams only 64
    columns.  The dst-sorted edge order makes each tile's edges
    contiguous, so no scatter and no cross-core reduction is needed.
    Epilogue per tile: x = embed + nh (bf16), transpose x via the PE
    (identity matmul), then out = leaky(xT.T @ W.T + b) with
    leaky = max(x, 0.01x).
  * src indices must fit int16 for dma_gather, so the 50176-row table is
    gathered as two 25088-row halves; each tile's edge list is split by
    src half (A: src < 25088, B: src >= 25088), each half padded to a
    multiple of 128 edges with (idx=0, w=0) no-op edges.
  * dma_gather descriptor generation runs on one Q7 core pair selected
    by queue_num (~7ns/idx on the pair) — the whole kernel is bound by
    it.  The per-core A/B edge streams are chunked into 16-block
    (2048-edge) gathers independent of slot boundaries and cycled over
    4 SWDGE queues so up to four generators run concurrently
    (single_packet=False lifts the 64-descriptor packet cap).  The bf16
    compute path keeps PE/DVE far below the SWDGE wall so the queues
    never stall on full buffers.
  * DVE per-op fixed costs dominate per-block elementwise work, so the
    one-hot build and the edge-weight multiply are batched per chunk as
    single tensor_tensor ops over [128, nblk*128] / [128, nblk*64] using
    0-stride broadcast APs of the per-block scalars.
"""

import json
import sys
import types

import numpy as np

P = 128
D = 64
N_NODES = 50000
N_CORES = 8
HALF = 25088            # int16-safe gather-table half (196 * 128)
NPAD = 2 * HALF         # 50176 = 392 tiles
NT = NPAD // P          # 392
SLOTS = NT // N_CORES   # 49
CHUNK = 16              # blocks per dma_gather
NQ = 4                  # SWDGE queues (Q7 core pairs)


# ----------------------------------------------------------------------------
# Environment fixups (self-contained; kernel.py must run alone).
# ----------------------------------------------------------------------------

_SPLIT_COUNT = 0


def _split_multi_waits_json(bir: bytes) -> bytes:
    """This container's walrus accepts only ONE sync wait per instruction
    ('Too many sync wait commands'), while Tile's scheduler attaches
    several.  Rewrite each instruction with N>1 waits into N-1 same-engine
    NoOps (one wait each) followed by the instruction with the last wait;
    same-engine sequencer order makes this equivalent."""
    global _SPLIT_COUNT
    d = json.loads(bir)
    changed = False
    for fn in d.get("functions", []):
        for bb in fn.get("blocks", []):
            out = []
            for ins in bb.get("instructions", []):
                si = ins.get("sync_info") or {}
                ow = si.get("on_wait") or []
                if len(ow) > 1:
                    changed = True
                    for w in ow[:-1]:
                        _SPLIT_COUNT += 1
                        out.append({
                            "opcode": "NoOp",
                            "engine": ins.get("engine", "Unassigned"),
                            "name": f"I-waitsplit-{_SPLIT_COUNT}",
                            "ins": [],
                            "outs": [],
                            "sync_info": {"on_update": [], "on_wait": [w]},
                        })
                    si["on_wait"] = [ow[-1]]
                out.append(ins)
            bb["instructions"] = out
    return json.dumps(d).encode() if changed else bir


def _install_fixups():
    import concourse.bass_utils as bass_utils
    import concourse.bass2jax as bass2jax

    if not getattr(bass_utils, "_waitsplit_installed", False):
        bass_utils._waitsplit_installed = True
        orig_compile = bass_utils.compile_bir_kernel

        def patched_compile(bir_json, tmpdir, neff_name="file.neff"):
            if isinstance(bir_json, str):
                bir_json = bir_json.encode()
            return orig_compile(_split_multi_waits_json(bir_json), tmpdir,
                                neff_name=neff_name)

        bass_utils.compile_bir_kernel = patched_compile
        bass2jax.compile_bir_kernel = patched_compile
        # No artifact bucket in this container; keep profiles local.
        bass_utils.upload_artifacts = lambda tmpdir: tmpdir

    # run_bass_kernel_spmd(trace=True) under axon needs antenv.axon_hooks,
    # which this image doesn't ship.  Synthesize it and install the ctypes
    # NTFF hook from trn_agent_boot so neuron-profile works.
    if "antenv.axon_hooks" not in sys.modules:
        m = types.ModuleType("antenv.axon_hooks")
        m._hook = None
        m.set_axon_ntff_profile_hook = lambda h: setattr(m, "_hook", h)
        m.get_axon_ntff_profile_hook = lambda: m._hook
        sys.modules["antenv.axon_hooks"] = m
        try:
            import antenv
            antenv.axon_hooks = m
        except ImportError:
            pass
        try:
            from trn_agent_boot.trn_boot import _ntff_profile_via_ctypes
            hook = _ntff_profile_via_ctypes("/opt/axon/libaxon_pjrt.so")
            if hook is not None:
                m._hook = hook
        except Exception:
            pass


# ----------------------------------------------------------------------------
# Host-side graph partitioning
# ----------------------------------------------------------------------------

def _bf16():
    from ml_dtypes import bfloat16
    return bfloat16


def _wrap16(seg: np.ndarray) -> np.ndarray:
    """dma_gather index layout: index i lives at [i % 16, i // 16]."""
    assert seg.size % 16 == 0
    return seg.reshape(-1, 16).T


def _prepare(entity_embed, src, dst, edge_weight, out_sqrt_degree,
             in_sqrt_degree):
    f32 = np.float32
    bf16 = _bf16()
    node = (entity_embed * out_sqrt_degree).astype(f32)
    node_pad = np.zeros((NPAD, D), f32)
    node_pad[:N_NODES] = node
    emb_pad = np.zeros((NPAD, D), f32)
    emb_pad[:N_NODES] = entity_embed.astype(f32)

    ew2 = (edge_weight[:, 0] * in_sqrt_degree[dst, 0]).astype(f32)

    order = np.argsort(dst, kind="stable")
    sdst = dst[order].astype(np.int64)
    ssrc = src[order].astype(np.int64)
    sew = ew2[order]

    counts = np.bincount(sdst // P, minlength=NT)
    starts = np.concatenate([[0], np.cumsum(counts)])

    # Per tile: split by gather-table half, count padded 128-edge blocks.
    tiles = []
    for t in range(NT):
        lo, hi = starts[t], starts[t + 1]
        t_src = ssrc[lo:hi]
        t_dstl = (sdst[lo:hi] - t * P).astype(f32)
        t_ew = sew[lo:hi]
        a = t_src < HALF
        tiles.append((t, t_src[a], t_dstl[a], t_ew[a],
                      t_src[~a] - HALF, t_dstl[~a], t_ew[~a]))
    ba = np.array([-(-len(x[1]) // P) for x in tiles])
    bb = np.array([-(-len(x[4]) // P) for x in tiles])

    # Deal tiles into 49 slots x 8 cores; similar (BA, BB) tiles share a
    # slot so the per-slot max padding stays small.  Slot block counts are
    # shared by all cores (one SPMD program).
    rank = sorted(range(NT), key=lambda t: (-ba[t], -bb[t]))
    slot_ba = np.zeros(SLOTS, np.int64)
    slot_bb = np.zeros(SLOTS, np.int64)
    tile_of = np.zeros((N_CORES, SLOTS), np.int64)
    for s in range(SLOTS):
        octet = rank[s * N_CORES:(s + 1) * N_CORES]
        slot_ba[s] = max(ba[t] for t in octet)
        slot_bb[s] = max(bb[t] for t in octet)
        for c, t in enumerate(octet):
            tile_of[c, s] = t

    ta = int(slot_ba.sum())           # A-stream blocks per core
    tbb = int(slot_bb.sum())          # B-stream blocks per core
    tb = ta + tbb
    # idx columns padded so each stream's chunks are CHUNK-aligned
    CH = CHUNK
    acols = 8 * CH * (-(-ta // CH))
    bcols = 8 * CH * (-(-tbb // CH))
    icols = acols + bcols

    idx_all = np.zeros((N_CORES, 16, icols), np.int16)
    dstl_all = np.zeros((N_CORES, P, tb), bf16)
    ew_all = np.zeros((N_CORES, P, tb), f32)
    emb_all = np.zeros((N_CORES, P, SLOTS * D), f32)

    a0 = np.concatenate([[0], np.cumsum(slot_ba)])   # A-stream block offsets
    b0 = np.concatenate([[0], np.cumsum(slot_bb)])   # B-stream block offsets

    for c in range(N_CORES):
        for s in range(SLOTS):
            t, srcA, dstlA, ewA, srcB, dstlB, ewB = tiles[tile_of[c, s]]
            nA, nB = P * slot_ba[s], P * slot_bb[s]
            ia = np.zeros(nA, np.int16)
            ia[:len(srcA)] = srcA
            ib = np.zeros(nB, np.int16)
            ib[:len(srcB)] = srcB
            # idx columns: A-stream first [0, acols), then B-stream.
            ca = 8 * a0[s]
            cb = acols + 8 * b0[s]
            if nA:
                idx_all[c, :, ca:ca + nA // 16] = _wrap16(ia)
            if nB:
                idx_all[c, :, cb:cb + nB // 16] = _wrap16(ib)
            # dstl/ew columns: A-block (s,j) -> a0[s]+j; B-block -> ta+b0[s]+j
            if slot_ba[s]:
                dl = np.zeros(nA, f32)
                dl[:len(dstlA)] = dstlA
                ws = np.zeros(nA, f32)
                ws[:len(ewA)] = ewA
                k = a0[s]
                dstl_all[c, :, k:k + slot_ba[s]] = dl.reshape(-1, P).T
                ew_all[c, :, k:k + slot_ba[s]] = ws.reshape(-1, P).T
            if slot_bb[s]:
                dl = np.zeros(nB, f32)
                dl[:len(dstlB)] = dstlB
                ws = np.zeros(nB, f32)
                ws[:len(ewB)] = ewB
                k = ta + b0[s]
                dstl_all[c, :, k:k + slot_bb[s]] = dl.reshape(-1, P).T
                ew_all[c, :, k:k + slot_bb[s]] = ws.reshape(-1, P).T
            # emb rows for tile t, laid [node_local(partition), slot*D+d]
            emb_all[c, :, s * D:(s + 1) * D] = emb_pad[t * P:(t + 1) * P]

    idx_rep = np.tile(idx_all, (1, 8, 1))  # replicate across the 8 Q7 cores
    sig = (tuple(int(x) for x in slot_ba), tuple(int(x) for x in slot_bb))
    return (node_pad, idx_rep, dstl_all, ew_all, emb_all, tile_of, sig, tb,
            icols)


# ----------------------------------------------------------------------------
# Device program
# ----------------------------------------------------------------------------

_PROGRAM_CACHE = {}


class _GatherStream:
    """Lazily emits chunked (<=CHUNK blocks) dma_gathers over one table
    half's concatenated block stream.  Per chunk it also emits ONE batched
    edge-weight multiply (gm = bf16(g * ew)) and ONE batched one-hot build
    (S[e, j, n] = bf16(iota[n] == dstl[e])) so the DVE cost is amortized
    over CHUNK blocks.  block(i) returns (S, gm) APs for block i."""

    def __init__(self, nc, mybir, pool, gmpool, spool, table_ap, idx_segs,
                 blk_col0, total_blocks, dstl_sb, ew_sb, iota_rep,
                 qpick, f32, bf16):
        self.nc = nc
        self.mybir = mybir
        self.pool = pool
        self.gmpool = gmpool
        self.spool = spool
        self.table_ap = table_ap
        # idx_segs: list of (tile, chunk0, nchunks); chunk k's 8*CHUNK idx
        # columns live in its group's tile at offset (k - chunk0)*8*CHUNK.
        self.idx_segs = idx_segs
        self.blk_col0 = blk_col0   # column offset into dstl/ew for block 0
        self.total = total_blocks
        self.dstl_sb = dstl_sb
        self.ew_sb = ew_sb
        self.iota_rep = iota_rep
        self.qpick = qpick
        self.f32 = f32
        self.bf16 = bf16
        self.tiles = []            # chunk index -> (S tile, gm tile)

    def _idx_ap(self, k, cols):
        for t, c0, nch in self.idx_segs:
            if c0 <= k < c0 + nch:
                off = (k - c0) * 8 * CHUNK
                return t[:, off:off + cols]
        raise AssertionError(k)

    def _emit_chunk(self, k):
        nc = self.nc
        nblk = min(CHUNK, self.total - k * CHUNK)
        g = self.pool.tile([P, nblk, D], self.f32)
        n = P * nblk
        nc.gpsimd.dma_gather(
            g[:], self.table_ap, self._idx_ap(k, n // 16), n, n, D,
            queue_num=self.qpick(), single_packet=False)
        b0 = self.blk_col0 + CHUNK * k
        gm = self.gmpool.tile([P, nblk, D], self.bf16)
        nc.vector.tensor_tensor(
            out=gm[:], in0=g[:],
            in1=self.ew_sb[:, b0:b0 + nblk].to_broadcast([P, nblk, D]),
            op=self.mybir.AluOpType.mult)
        S = self.spool.tile([P, nblk, P], self.bf16)
        nc.vector.tensor_tensor(
            out=S[:],
            in0=self.iota_rep[:, :nblk * P].rearrange(
                "p (k n) -> p k n", n=P),
            in1=self.dstl_sb[:, b0:b0 + nblk].to_broadcast([P, nblk, P]),
            op=self.mybir.AluOpType.is_equal)
        self.tiles.append((S, gm))

    def block(self, i):
        k, off = divmod(i, CHUNK)
        while len(self.tiles) <= k:
            self._emit_chunk(len(self.tiles))
        S, gm = self.tiles[k]
        return S[:, off, :], gm[:, off, :]


def _build_program(sig, tb, icols):
    if sig in _PROGRAM_CACHE:
        return _PROGRAM_CACHE[sig]

    from concourse import bacc
    import concourse.mybir as mybir
    import concourse.tile as tile

    slot_ba, slot_bb = sig
    ta = sum(slot_ba)
    tbb = sum(slot_bb)
    nc = bacc.Bacc("TRN2", num_swdge_queues=NQ)
    f32 = mybir.dt.float32
    bf16 = mybir.dt.bfloat16
    t_node = nc.dram_tensor("node", [NPAD, D], f32, kind="ExternalInput")
    t_idx = nc.dram_tensor("idx", [P, icols], mybir.dt.int16,
                           kind="ExternalInput")
    t_dstl = nc.dram_tensor("dstl", [P, tb], bf16, kind="ExternalInput")
    t_ew = nc.dram_tensor("ew", [P, tb], f32, kind="ExternalInput")
    t_emb = nc.dram_tensor("emb", [P, SLOTS * D], f32, kind="ExternalInput")
    t_wt = nc.dram_tensor("wt", [D, D], bf16, kind="ExternalInput")
    t_b = nc.dram_tensor("bias", [1, D], bf16, kind="ExternalInput")
    t_iota = nc.dram_tensor("iota", [P, CHUNK * P], bf16,
                            kind="ExternalInput")
    t_ident = nc.dram_tensor("ident", [P, P], bf16, kind="ExternalInput")
    t_out = nc.dram_tensor("out", [SLOTS * P, D], f32, kind="ExternalOutput")

    qstate = [0]

    def qpick():
        q = qstate[0] % NQ
        qstate[0] += 1
        return q

    with tile.TileContext(nc) as tc:
        with tc.tile_pool(name="const", bufs=1) as cpool, \
             tc.tile_pool(name="ga", bufs=4) as gapool, \
             tc.tile_pool(name="gb", bufs=4) as gbpool, \
             tc.tile_pool(name="gma", bufs=3) as gmapool, \
             tc.tile_pool(name="gmb", bufs=3) as gmbpool, \
             tc.tile_pool(name="sa", bufs=3) as sapool, \
             tc.tile_pool(name="sb", bufs=3) as sbpool, \
             tc.tile_pool(name="small", bufs=3) as mpool, \
             tc.tile_pool(name="psnh", bufs=3, space="PSUM") as psnh, \
             tc.tile_pool(name="psxt", bufs=2, space="PSUM") as psxt, \
             tc.tile_pool(name="psout", bufs=2, space="PSUM") as psout:
            # idx group tiles (chunk-aligned) so the first gather only
            # waits on its own small DMA, not the whole index array
            ncha = -(-ta // CHUNK)
            nchb = -(-tbb // CHUNK)
            segs = []
            for c0t, ncht in ((0, ncha), (ncha, nchb)):
                ngrp = min(4, ncht) or 1
                for gidx in range(ngrp):
                    lo = c0t + ncht * gidx // ngrp
                    hi = c0t + ncht * (gidx + 1) // ngrp
                    if hi == lo:
                        continue
                    w = min(hi * 8 * CHUNK, icols // 1) - lo * 8 * CHUNK
                    w = min(w, icols - lo * 8 * CHUNK)
                    tgt = cpool.tile([P, w], mybir.dt.int16,
                                     tag=f"idx{lo}")
                    nc.sync.dma_start(
                        out=tgt[:],
                        in_=t_idx[:, lo * 8 * CHUNK:lo * 8 * CHUNK + w])
                    segs.append((tgt, lo, hi - lo))
            idx_segs_a = [(t, c0, n) for (t, c0, n) in segs if c0 < ncha]
            idx_segs_b = [(t, c0 - ncha, n) for (t, c0, n) in segs
                          if c0 >= ncha]
            dstl_sb = cpool.tile([P, tb], bf16)
            ew_sb = cpool.tile([P, tb], f32)
            bnd2 = [tb * i // 4 for i in range(5)]
            for i in range(4):
                nc.sync.dma_start(out=dstl_sb[:, bnd2[i]:bnd2[i + 1]],
                                  in_=t_dstl[:, bnd2[i]:bnd2[i + 1]])
                nc.sync.dma_start(out=ew_sb[:, bnd2[i]:bnd2[i + 1]],
                                  in_=t_ew[:, bnd2[i]:bnd2[i + 1]])
            iota_rep = cpool.tile([P, CHUNK * P], bf16)
            nc.scalar.dma_start(out=iota_rep[:], in_=t_iota[:])
            ident_sb = cpool.tile([P, P], bf16)
            nc.scalar.dma_start(out=ident_sb[:], in_=t_ident[:])
            ones = cpool.tile([1, P], bf16)
            nc.vector.memset(ones[:], 1.0)
            wt_sb = cpool.tile([D, D], bf16)
            nc.scalar.dma_start(out=wt_sb[:], in_=t_wt[:])
            b_sb = cpool.tile([1, D], bf16)
            nc.scalar.dma_start(out=b_sb[:], in_=t_b[:])
            emb_sb = cpool.tile([P, SLOTS * D], f32)
            nc.scalar.dma_start(out=emb_sb[:], in_=t_emb[:])

            sa = _GatherStream(nc, mybir, gapool, gmapool, sapool,
                               t_node[0:HALF, :], idx_segs_a, 0, ta,
                               dstl_sb, ew_sb, iota_rep, qpick, f32, bf16)
            sb = _GatherStream(nc, mybir, gbpool, gmbpool, sbpool,
                               t_node[HALF:NPAD, :], idx_segs_b, ta, tbb,
                               dstl_sb, ew_sb, iota_rep, qpick, f32, bf16)

            a_off = 0
            b_off = 0
            for s in range(SLOTS):
                ba, bb = slot_ba[s], slot_bb[s]
                nb = ba + bb
                blocks = [sa.block(a_off + j) for j in range(ba)]
                blocks += [sb.block(b_off + j) for j in range(bb)]
                a_off += ba
                b_off += bb
                x_sb = mpool.tile([P, D], bf16, tag="x")
                if nb:
                    nh = psnh.tile([P, D], f32, space="PSUM", tag="nh")
                    for i, (lhsT, rhs) in enumerate(blocks):
                        nc.tensor.matmul(out=nh[:], lhsT=lhsT, rhs=rhs,
                                         start=(i == 0), stop=(i == nb - 1))
                    nc.vector.tensor_add(out=x_sb[:], in0=nh[:],
                                         in1=emb_sb[:, s * D:(s + 1) * D])
                else:
                    nc.vector.tensor_copy(out=x_sb[:],
                                          in_=emb_sb[:, s * D:(s + 1) * D])
                xT_ps = psxt.tile([D, P], bf16, space="PSUM", tag="xt")
                nc.tensor.matmul(out=xT_ps[:], lhsT=x_sb[:], rhs=ident_sb[:],
                                 is_transpose=True)
                xT_sb = mpool.tile([D, P], bf16, tag="xts")
                nc.vector.tensor_copy(out=xT_sb[:], in_=xT_ps[:])
                o_ps = psout.tile([P, D], f32, space="PSUM", tag="ops")
                nc.tensor.matmul(out=o_ps[:], lhsT=xT_sb[:], rhs=wt_sb[:],
                                 start=True, stop=False)
                nc.tensor.matmul(out=o_ps[:], lhsT=ones[:], rhs=b_sb[:],
                                 start=False, stop=True)
                o_scaled = mpool.tile([P, D], f32, tag="osc")
                nc.vector.tensor_scalar_mul(o_scaled[:], o_ps[:], 0.01)
                o_sb = mpool.tile([P, D], f32, tag="osb")
                nc.vector.tensor_tensor(out=o_sb[:], in0=o_ps[:],
                                        in1=o_scaled[:],
                                        op=mybir.AluOpType.max)
                nc.sync.dma_start(out=t_out[s * P:(s + 1) * P, :], in_=o_sb[:])

    nc.compile()
    _PROGRAM_CACHE[sig] = nc
    return nc


LAST_RESULTS = None


def kernel(entity_embed, src, dst, edge_weight, out_sqrt_degree,
           in_sqrt_degree, W, b):
    _install_fixups()
    from concourse.bass_utils import run_bass_kernel_spmd

    bf16 = _bf16()
    entity_embed = np.asarray(entity_embed, np.float32)
    src = np.asarray(src)
    dst = np.asarray(dst)
    edge_weight = np.asarray(edge_weight, np.float32)
    out_sqrt_degree = np.asarray(out_sqrt_degree, np.float32)
    in_sqrt_degree = np.asarray(in_sqrt_degree, np.float32)
    W = np.asarray(W, np.float32)
    b = np.asarray(b, np.float32)

    (node_pad, idx_rep, dstl_all, ew_all, emb_all, tile_of, sig, tb,
     icols) = _prepare(entity_embed, src, dst, edge_weight, out_sqrt_degree,
                       in_sqrt_degree)

    nc = _build_program(sig, tb, icols)

    wt = np.ascontiguousarray(W.T).astype(bf16)     # rhs[k, j] = W[j, k]
    iota_np = np.tile(np.tile(np.arange(P, dtype=np.float32), CHUNK),
                      (P, 1)).astype(bf16)
    ident_np = np.eye(P, dtype=np.float32).astype(bf16)
    in_maps = []
    for c in range(N_CORES):
        in_maps.append({
            "node": node_pad,
            "idx": np.ascontiguousarray(idx_rep[c]),
            "dstl": np.ascontiguousarray(dstl_all[c]),
            "ew": np.ascontiguousarray(ew_all[c]),
            "emb": np.ascontiguousarray(emb_all[c]),
            "wt": wt,
            "bias": b[None, :].astype(bf16),
            "iota": iota_np,
            "ident": ident_np,
        })

    try:
        res = run_bass_kernel_spmd(nc, in_maps,
                                   core_ids=list(range(N_CORES)))
    except Exception:
        # Transient NRT_EXEC_UNIT_UNRECOVERABLE states have been observed;
        # a reset + retry recovers them.
        import os
        import time
        os.environ["NEURON_RT_RESET_CORES"] = "1"
        time.sleep(30)
        res = run_bass_kernel_spmd(nc, in_maps,
                                   core_ids=list(range(N_CORES)))
    global LAST_RESULTS
    LAST_RESULTS = res

    out = np.empty((NPAD, D), np.float32)
    for c in range(N_CORES):
        oc = res.results[c]["out"]
        for s in range(SLOTS):
            t = tile_of[c, s]
            out[t * P:(t + 1) * P] = oc[s * P:(s + 1) * P]
    return out[:N_NODES]


# revision 7
# speedup vs baseline: 1.2595x; 1.1248x over previous
"""GNN message-passing aggregator on 8 Trainium2 NeuronCores.

Computes, for the full graph:
    node = entity_embed * out_sqrt_degree
    msg  = node[src] * edge_weight
    N_h  = segment_sum(msg, dst, N) * in_sqrt_degree
    out  = leaky_relu((entity_embed + N_h) @ W.T + b, 0.01)

Strategy (dst-partitioned edge shard, no collectives):
  * Host: fold degree factors into the gather table / edge weights, sort
    edges by dst, cut the node space into 392 aligned 128-node tiles,
    and deal tiles onto 8 cores x 49 slots so every core runs the same
    instruction stream (SPMD: one program, per-core data).
  * Device, per 128-node tile: dma_gather the source rows (256B each)
    from the replicated node table, build a one-hot selection matrix
    S[e, n] = (dstl[e] == n) in bf16, weight the gathered rows by edge
    weight into bf16 messages gm, and accumulate
        nh[n, :] += S.T @ gm     (PE, PSUM fp32)
    with S as the 128-column stationary operand so the PE's fast-weight-
    load path hides the weight loads and each block streams only 64
    columns.  The dst-sorted edge order makes each tile's edges
    contiguous, so no scatter and no cross-core reduction is needed.
    Epilogue per tile: x = embed + nh (bf16), transpose x via the PE
    (identity matmul), then out = leaky(xT.T @ W.T + b) with
    leaky = max(x, 0.01x).
  * src indices must fit int16 for dma_gather, so the 50176-row table is
    gathered as two 25088-row halves; each tile's edge list is split by
    src half (A: src < 25088, B: src >= 25088), each half padded to a
    multiple of 128 edges with (idx=0, w=0) no-op edges.
  * dma_gather descriptor generation runs on one Q7 core pair selected
    by queue_num (~7ns/idx on the pair) — the whole kernel is bound by
    it.  The per-core A/B edge streams are chunked into 16-block
    (2048-edge) gathers independent of slot boundaries and cycled over
    4 SWDGE queues so up to four generators run concurrently
    (single_packet=False lifts the 64-descriptor packet cap).  The bf16
    compute path keeps PE/DVE far below the SWDGE wall so the queues
    never stall on full buffers.
  * DVE per-op fixed costs dominate per-block elementwise work, so the
    one-hot build and the edge-weight multiply are batched per chunk as
    single tensor_tensor ops over [128, nblk*128] / [128, nblk*64] using
    0-stride broadcast APs of the per-block scalars.
"""

import json
import sys
import types

import numpy as np

P = 128
D = 64
N_NODES = 50000
N_CORES = 8
HALF = 25088            # int16-safe gather-table half (196 * 128)
NPAD = 2 * HALF         # 50176 = 392 tiles
NT = NPAD // P          # 392
SLOTS = NT // N_CORES   # 49
CHUNK = 16              # blocks per dma_gather
NQ = 4                  # SWDGE queues (Q7 core pairs)


# ----------------------------------------------------------------------------
# Environment fixups (self-contained; kernel.py must run alone).
# ----------------------------------------------------------------------------

_SPLIT_COUNT = 0


def _split_multi_waits_json(bir: bytes) -> bytes:
    """This container's walrus accepts only ONE sync wait per instruction
    ('Too many sync wait commands'), while Tile's scheduler attaches
    several.  Rewrite each instruction with N>1 waits into N-1 same-engine
    NoOps (one wait each) followed by the instruction with the last wait;
    same-engine sequencer order makes this equivalent."""
    global _SPLIT_COUNT
    d = json.loads(bir)
    changed = False
    for fn in d.get("functions", []):
        for bb in fn.get("blocks", []):
            out = []
            for ins in bb.get("instructions", []):
                si = ins.get("sync_info") or {}
                ow = si.get("on_wait") or []
                if len(ow) > 1:
                    changed = True
                    for w in ow[:-1]:
                        _SPLIT_COUNT += 1
                        out.append({
                            "opcode": "NoOp",
                            "engine": ins.get("engine", "Unassigned"),
                            "name": f"I-waitsplit-{_SPLIT_COUNT}",
                            "ins": [],
                            "outs": [],
                            "sync_info": {"on_update": [], "on_wait": [w]},
                        })
                    si["on_wait"] = [ow[-1]]
                out.append(ins)
            bb["instructions"] = out
    return json.dumps(d).encode() if changed else bir


def _install_fixups():
    import concourse.bass_utils as bass_utils
    import concourse.bass2jax as bass2jax

    if not getattr(bass_utils, "_waitsplit_installed", False):
        bass_utils._waitsplit_installed = True
        orig_compile = bass_utils.compile_bir_kernel

        def patched_compile(bir_json, tmpdir, neff_name="file.neff"):
            if isinstance(bir_json, str):
                bir_json = bir_json.encode()
            return orig_compile(_split_multi_waits_json(bir_json), tmpdir,
                                neff_name=neff_name)

        bass_utils.compile_bir_kernel = patched_compile
        bass2jax.compile_bir_kernel = patched_compile
        # No artifact bucket in this container; keep profiles local.
        bass_utils.upload_artifacts = lambda tmpdir: tmpdir

    # run_bass_kernel_spmd(trace=True) under axon needs antenv.axon_hooks,
    # which this image doesn't ship.  Synthesize it and install the ctypes
    # NTFF hook from trn_agent_boot so neuron-profile works.
    if "antenv.axon_hooks" not in sys.modules:
        m = types.ModuleType("antenv.axon_hooks")
        m._hook = None
        m.set_axon_ntff_profile_hook = lambda h: setattr(m, "_hook", h)
        m.get_axon_ntff_profile_hook = lambda: m._hook
        sys.modules["antenv.axon_hooks"] = m
        try:
            import antenv
            antenv.axon_hooks = m
        except ImportError:
            pass
        try:
            from trn_agent_boot.trn_boot import _ntff_profile_via_ctypes
            hook = _ntff_profile_via_ctypes("/opt/axon/libaxon_pjrt.so")
            if hook is not None:
                m._hook = hook
        except Exception:
            pass


# ----------------------------------------------------------------------------
# Host-side graph partitioning
# ----------------------------------------------------------------------------

def _bf16():
    from ml_dtypes import bfloat16
    return bfloat16


def _wrap16(seg: np.ndarray) -> np.ndarray:
    """dma_gather index layout: index i lives at [i % 16, i // 16]."""
    assert seg.size % 16 == 0
    return seg.reshape(-1, 16).T


def _prepare(entity_embed, src, dst, edge_weight, out_sqrt_degree,
             in_sqrt_degree):
    f32 = np.float32
    bf16 = _bf16()
    node = (entity_embed * out_sqrt_degree).astype(f32)
    node_pad = np.zeros((NPAD, D), f32)
    node_pad[:N_NODES] = node
    emb_pad = np.zeros((NPAD, D), f32)
    emb_pad[:N_NODES] = entity_embed.astype(f32)

    ew2 = (edge_weight[:, 0] * in_sqrt_degree[dst, 0]).astype(f32)

    order = np.argsort(dst, kind="stable")
    sdst = dst[order].astype(np.int64)
    ssrc = src[order].astype(np.int64)
    sew = ew2[order]

    counts = np.bincount(sdst // P, minlength=NT)
    starts = np.concatenate([[0], np.cumsum(counts)])

    # Per tile: split by gather-table half, count padded 128-edge blocks.
    tiles = []
    for t in range(NT):
        lo, hi = starts[t], starts[t + 1]
        t_src = ssrc[lo:hi]
        t_dstl = (sdst[lo:hi] - t * P).astype(f32)
        t_ew = sew[lo:hi]
        a = t_src < HALF
        tiles.append((t, t_src[a], t_dstl[a], t_ew[a],
                      t_src[~a] - HALF, t_dstl[~a], t_ew[~a]))
    ba = np.array([-(-len(x[1]) // P) for x in tiles])
    bb = np.array([-(-len(x[4]) // P) for x in tiles])

    # Deal tiles into 49 slots x 8 cores; similar (BA, BB) tiles share a
    # slot so the per-slot max padding stays small.  Slot block counts are
    # shared by all cores (one SPMD program).
    rank = sorted(range(NT), key=lambda t: (-ba[t], -bb[t]))
    slot_ba = np.zeros(SLOTS, np.int64)
    slot_bb = np.zeros(SLOTS, np.int64)
    tile_of = np.zeros((N_CORES, SLOTS), np.int64)
    for s in range(SLOTS):
        octet = rank[s * N_CORES:(s + 1) * N_CORES]
        slot_ba[s] = max(ba[t] for t in octet)
        slot_bb[s] = max(bb[t] for t in octet)
        for c, t in enumerate(octet):
            tile_of[c, s] = t

    ta = int(slot_ba.sum())           # A-stream blocks per core
    tbb = int(slot_bb.sum())          # B-stream blocks per core
    tb = ta + tbb
    # idx columns padded so each stream's chunks are CHUNK-aligned
    CH = CHUNK
    acols = 8 * CH * (-(-ta // CH))
    bcols = 8 * CH * (-(-tbb // CH))
    icols = acols + bcols

    idx_all = np.zeros((N_CORES, 16, icols), np.int16)
    dstl_all = np.zeros((N_CORES, P, tb), f32)
    ew_all = np.zeros((N_CORES, P, tb), f32)
    emb_all = np.zeros((N_CORES, P, SLOTS * D), f32)

    a0 = np.concatenate([[0], np.cumsum(slot_ba)])   # A-stream block offsets
    b0 = np.concatenate([[0], np.cumsum(slot_bb)])   # B-stream block offsets

    for c in range(N_CORES):
        for s in range(SLOTS):
            t, srcA, dstlA, ewA, srcB, dstlB, ewB = tiles[tile_of[c, s]]
            nA, nB = P * slot_ba[s], P * slot_bb[s]
            ia = np.zeros(nA, np.int16)
            ia[:len(srcA)] = srcA
            ib = np.zeros(nB, np.int16)
            ib[:len(srcB)] = srcB
            # idx columns: A-stream first [0, acols), then B-stream.
            ca = 8 * a0[s]
            cb = acols + 8 * b0[s]
            if nA:
                idx_all[c, :, ca:ca + nA // 16] = _wrap16(ia)
            if nB:
                idx_all[c, :, cb:cb + nB // 16] = _wrap16(ib)
            # dstl/ew columns: A-block (s,j) -> a0[s]+j; B-block -> ta+b0[s]+j
            if slot_ba[s]:
                dl = np.zeros(nA, f32)
                dl[:len(dstlA)] = dstlA
                ws = np.zeros(nA, f32)
                ws[:len(ewA)] = ewA
                k = a0[s]
                dstl_all[c, :, k:k + slot_ba[s]] = dl.reshape(-1, P).T
                ew_all[c, :, k:k + slot_ba[s]] = ws.reshape(-1, P).T
            if slot_bb[s]:
                dl = np.zeros(nB, f32)
                dl[:len(dstlB)] = dstlB
                ws = np.zeros(nB, f32)
                ws[:len(ewB)] = ewB
                k = ta + b0[s]
                dstl_all[c, :, k:k + slot_bb[s]] = dl.reshape(-1, P).T
                ew_all[c, :, k:k + slot_bb[s]] = ws.reshape(-1, P).T
            # emb rows for tile t, laid [node_local(partition), slot*D+d]
            emb_all[c, :, s * D:(s + 1) * D] = emb_pad[t * P:(t + 1) * P]

    idx_rep = np.tile(idx_all, (1, 8, 1))  # replicate across the 8 Q7 cores
    sig = (tuple(int(x) for x in slot_ba), tuple(int(x) for x in slot_bb))
    return (node_pad, idx_rep, dstl_all, ew_all, emb_all, tile_of, sig, tb,
            icols)


# ----------------------------------------------------------------------------
# Device program
# ----------------------------------------------------------------------------

_PROGRAM_CACHE = {}


class _GatherStream:
    """Lazily emits chunked (<=CHUNK blocks) dma_gathers over one table
    half's concatenated block stream.  Per chunk it also emits ONE batched
    edge-weight multiply (gm = bf16(g * ew)) and ONE batched one-hot build
    (S[e, j, n] = bf16(iota[n] == dstl[e])) so the DVE cost is amortized
    over CHUNK blocks.  block(i) returns (S, gm) APs for block i."""

    def __init__(self, nc, mybir, pool, gmpool, spool, table_ap, idx_segs,
                 blk_col0, total_blocks, dstl_sb, ew_sb, iota_rep,
                 qpick, f32, bf16):
        self.nc = nc
        self.mybir = mybir
        self.pool = pool
        self.gmpool = gmpool
        self.spool = spool
        self.table_ap = table_ap
        # idx_segs: list of (tile, chunk0, nchunks); chunk k's 8*CHUNK idx
        # columns live in its group's tile at offset (k - chunk0)*8*CHUNK.
        self.idx_segs = idx_segs
        self.blk_col0 = blk_col0   # column offset into dstl/ew for block 0
        self.total = total_blocks
        self.dstl_sb = dstl_sb
        self.ew_sb = ew_sb
        self.iota_rep = iota_rep
        self.qpick = qpick
        self.f32 = f32
        self.bf16 = bf16
        self.tiles = []            # chunk index -> (S tile, gm tile)

    def _idx_ap(self, k, cols):
        for t, c0, nch in self.idx_segs:
            if c0 <= k < c0 + nch:
                off = (k - c0) * 8 * CHUNK
                return t[:, off:off + cols]
        raise AssertionError(k)

    def _emit_chunk(self, k):
        nc = self.nc
        nblk = min(CHUNK, self.total - k * CHUNK)
        g = self.pool.tile([P, nblk, D], self.f32)
        n = P * nblk
        nc.gpsimd.dma_gather(
            g[:], self.table_ap, self._idx_ap(k, n // 16), n, n, D,
            queue_num=self.qpick(), single_packet=False)
        b0 = self.blk_col0 + CHUNK * k
        gm = self.gmpool.tile([P, nblk, D], self.bf16)
        nc.vector.tensor_tensor(
            out=gm[:], in0=g[:],
            in1=self.ew_sb[:, b0:b0 + nblk].to_broadcast([P, nblk, D]),
            op=self.mybir.AluOpType.mult)
        S = self.spool.tile([P, nblk, P], self.bf16)
        nc.vector.tensor_tensor(
            out=S[:],
            in0=self.iota_rep[:, :nblk * P].rearrange(
                "p (k n) -> p k n", n=P),
            in1=self.dstl_sb[:, b0:b0 + nblk].to_broadcast([P, nblk, P]),
            op=self.mybir.AluOpType.is_equal)
        self.tiles.append((S, gm))

    def block(self, i):
        k, off = divmod(i, CHUNK)
        while len(self.tiles) <= k:
            self._emit_chunk(len(self.tiles))
        S, gm = self.tiles[k]
        return S[:, off, :], gm[:, off, :]


def _build_program(sig, tb, icols):
    if sig in _PROGRAM_CACHE:
        return _PROGRAM_CACHE[sig]

    from concourse import bacc
    import concourse.mybir as mybir
    import concourse.tile as tile

    slot_ba, slot_bb = sig
    ta = sum(slot_ba)
    tbb = sum(slot_bb)
    nc = bacc.Bacc("TRN2", num_swdge_queues=NQ)
    f32 = mybir.dt.float32
    bf16 = mybir.dt.bfloat16
    t_node = nc.dram_tensor("node", [NPAD, D], f32, kind="ExternalInput")
    t_idx = nc.dram_tensor("idx", [P, icols], mybir.dt.int16,
                           kind="ExternalInput")
    t_dstl = nc.dram_tensor("dstl", [P, tb], f32, kind="ExternalInput")
    t_ew = nc.dram_tensor("ew", [P, tb], f32, kind="ExternalInput")
    t_emb = nc.dram_tensor("emb", [P, SLOTS * D], f32, kind="ExternalInput")
    t_wt = nc.dram_tensor("wt", [D, D], bf16, kind="ExternalInput")
    t_b = nc.dram_tensor("bias", [1, D], bf16, kind="ExternalInput")
    t_iota = nc.dram_tensor("iota", [P, CHUNK * P], f32,
                            kind="ExternalInput")
    t_ident = nc.dram_tensor("ident", [P, P], bf16, kind="ExternalInput")
    t_out = nc.dram_tensor("out", [SLOTS * P, D], f32, kind="ExternalOutput")

    qstate = [0]

    def qpick():
        q = qstate[0] % NQ
        qstate[0] += 1
        return q

    with tile.TileContext(nc) as tc:
        with tc.tile_pool(name="const", bufs=1) as cpool, \
             tc.tile_pool(name="ga", bufs=6) as gapool, \
             tc.tile_pool(name="gb", bufs=6) as gbpool, \
             tc.tile_pool(name="gma", bufs=3) as gmapool, \
             tc.tile_pool(name="gmb", bufs=3) as gmbpool, \
             tc.tile_pool(name="sa", bufs=3) as sapool, \
             tc.tile_pool(name="sb", bufs=3) as sbpool, \
             tc.tile_pool(name="small", bufs=3) as mpool, \
             tc.tile_pool(name="pscst", bufs=1, space="PSUM") as pscst, \
             tc.tile_pool(name="psnh", bufs=2, space="PSUM") as psnh, \
             tc.tile_pool(name="psxt", bufs=1, space="PSUM") as psxt, \
             tc.tile_pool(name="psout", bufs=1, space="PSUM") as psout:
            # idx group tiles (chunk-aligned) so the first gather only
            # waits on its own small DMA, not the whole index array
            ncha = -(-ta // CHUNK)
            nchb = -(-tbb // CHUNK)
            segs = []
            for c0t, ncht in ((0, ncha), (ncha, nchb)):
                ngrp = min(4, ncht) or 1
                for gidx in range(ngrp):
                    lo = c0t + ncht * gidx // ngrp
                    hi = c0t + ncht * (gidx + 1) // ngrp
                    if hi == lo:
                        continue
                    w = min(hi * 8 * CHUNK, icols // 1) - lo * 8 * CHUNK
                    w = min(w, icols - lo * 8 * CHUNK)
                    tgt = cpool.tile([P, w], mybir.dt.int16,
                                     tag=f"idx{lo}")
                    nc.sync.dma_start(
                        out=tgt[:],
                        in_=t_idx[:, lo * 8 * CHUNK:lo * 8 * CHUNK + w])
                    segs.append((tgt, lo, hi - lo))
            idx_segs_a = [(t, c0, n) for (t, c0, n) in segs if c0 < ncha]
            idx_segs_b = [(t, c0 - ncha, n) for (t, c0, n) in segs
                          if c0 >= ncha]
            dstl_st = cpool.tile([P, tb], f32)
            ew_st = cpool.tile([P, tb], f32)
            bnd2 = [tb * i // 4 for i in range(5)]
            for i in range(4):
                nc.sync.dma_start(out=dstl_st[:, bnd2[i]:bnd2[i + 1]],
                                  in_=t_dstl[:, bnd2[i]:bnd2[i + 1]])
                nc.sync.dma_start(out=ew_st[:, bnd2[i]:bnd2[i + 1]],
                                  in_=t_ew[:, bnd2[i]:bnd2[i + 1]])
            # dstl/ew live in PSUM (bank-padded): DVE reads them through its
            # PSUM path, so the batched tensor_tensor ops below never touch
            # the DVE/GpSimd shared SBUF port pair — a 2-port DVE op would
            # lock the SWDGE descriptor generators out of SBUF and stall the
            # gathers.
            dstl_sb = pscst.tile([P, tb], f32, space="PSUM",
                                 padded_shape=[P, 1024])
            nc.vector.tensor_copy(out=dstl_sb[:], in_=dstl_st[:])
            ew_sb = pscst.tile([P, tb], f32, space="PSUM",
                               padded_shape=[P, 1024])
            nc.vector.tensor_copy(out=ew_sb[:], in_=ew_st[:])
            iota_rep = cpool.tile([P, CHUNK * P], f32)
            nc.scalar.dma_start(out=iota_rep[:], in_=t_iota[:])
            ident_sb = cpool.tile([P, P], bf16)
            nc.scalar.dma_start(out=ident_sb[:], in_=t_ident[:])
            ones = cpool.tile([1, P], bf16)
            nc.vector.memset(ones[:], 1.0)
            wt_sb = cpool.tile([D, D], bf16)
            nc.scalar.dma_start(out=wt_sb[:], in_=t_wt[:])
            b_sb = cpool.tile([1, D], bf16)
            nc.scalar.dma_start(out=b_sb[:], in_=t_b[:])
            emb_sb = cpool.tile([P, SLOTS * D], f32)
            nc.scalar.dma_start(out=emb_sb[:], in_=t_emb[:])

            sa = _GatherStream(nc, mybir, gapool, gmapool, sapool,
                               t_node[0:HALF, :], idx_segs_a, 0, ta,
                               dstl_sb, ew_sb, iota_rep, qpick, f32, bf16)
            sb = _GatherStream(nc, mybir, gbpool, gmbpool, sbpool,
                               t_node[HALF:NPAD, :], idx_segs_b, ta, tbb,
                               dstl_sb, ew_sb, iota_rep, qpick, f32, bf16)

            a_off = 0
            b_off = 0
            for s in range(SLOTS):
                ba, bb = slot_ba[s], slot_bb[s]
                nb = ba + bb
                blocks = [sa.block(a_off + j) for j in range(ba)]
                blocks += [sb.block(b_off + j) for j in range(bb)]
                a_off += ba
                b_off += bb
                x_sb = mpool.tile([P, D], bf16, tag="x")
                if nb:
                    nh = psnh.tile([P, D], f32, space="PSUM", tag="nh",
                                   padded_shape=[P, 512])
                    for i, (lhsT, rhs) in enumerate(blocks):
                        nc.tensor.matmul(out=nh[:], lhsT=lhsT, rhs=rhs,
                                         start=(i == 0), stop=(i == nb - 1))
                    nc.vector.tensor_add(out=x_sb[:], in0=nh[:],
                                         in1=emb_sb[:, s * D:(s + 1) * D])
                else:
                    nc.vector.tensor_copy(out=x_sb[:],
                                          in_=emb_sb[:, s * D:(s + 1) * D])
                xT_ps = psxt.tile([D, P], bf16, space="PSUM", tag="xt",
                                  padded_shape=[D, 1024])
                nc.tensor.matmul(out=xT_ps[:], lhsT=x_sb[:], rhs=ident_sb[:],
                                 is_transpose=True)
                xT_sb = mpool.tile([D, P], bf16, tag="xts")
                nc.vector.tensor_copy(out=xT_sb[:], in_=xT_ps[:])
                o_ps = psout.tile([P, D], f32, space="PSUM", tag="ops",
                                  padded_shape=[P, 512])
                nc.tensor.matmul(out=o_ps[:], lhsT=xT_sb[:], rhs=wt_sb[:],
                                 start=True, stop=False)
                nc.tensor.matmul(out=o_ps[:], lhsT=ones[:], rhs=b_sb[:],
                                 start=False, stop=True)
                o_scaled = mpool.tile([P, D], f32, tag="osc")
                nc.vector.tensor_scalar_mul(o_scaled[:], o_ps[:], 0.01)
                o_sb = mpool.tile([P, D], f32, tag="osb")
                nc.vector.tensor_tensor(out=o_sb[:], in0=o_ps[:],
                                        in1=o_scaled[:],
                                        op=mybir.AluOpType.max)
                nc.sync.dma_start(out=t_out[s * P:(s + 1) * P, :], in_=o_sb[:])

    nc.compile()
    _PROGRAM_CACHE[sig] = nc
    return nc


LAST_RESULTS = None


def kernel(entity_embed, src, dst, edge_weight, out_sqrt_degree,
           in_sqrt_degree, W, b):
    _install_fixups()
    from concourse.bass_utils import run_bass_kernel_spmd

    bf16 = _bf16()
    entity_embed = np.asarray(entity_embed, np.float32)
    src = np.asarray(src)
    dst = np.asarray(dst)
    edge_weight = np.asarray(edge_weight, np.float32)
    out_sqrt_degree = np.asarray(out_sqrt_degree, np.float32)
    in_sqrt_degree = np.asarray(in_sqrt_degree, np.float32)
    W = np.asarray(W, np.float32)
    b = np.asarray(b, np.float32)

    (node_pad, idx_rep, dstl_all, ew_all, emb_all, tile_of, sig, tb,
     icols) = _prepare(entity_embed, src, dst, edge_weight, out_sqrt_degree,
                       in_sqrt_degree)

    nc = _build_program(sig, tb, icols)

    wt = np.ascontiguousarray(W.T).astype(bf16)     # rhs[k, j] = W[j, k]
    iota_np = np.tile(np.tile(np.arange(P, dtype=np.float32), CHUNK),
                      (P, 1))
    ident_np = np.eye(P, dtype=np.float32).astype(bf16)
    in_maps = []
    for c in range(N_CORES):
        in_maps.append({
            "node": node_pad,
            "idx": np.ascontiguousarray(idx_rep[c]),
            "dstl": np.ascontiguousarray(dstl_all[c]),
            "ew": np.ascontiguousarray(ew_all[c]),
            "emb": np.ascontiguousarray(emb_all[c]),
            "wt": wt,
            "bias": b[None, :].astype(bf16),
            "iota": iota_np,
            "ident": ident_np,
        })

    try:
        res = run_bass_kernel_spmd(nc, in_maps,
                                   core_ids=list(range(N_CORES)))
    except Exception:
        # Transient NRT_EXEC_UNIT_UNRECOVERABLE states have been observed;
        # a reset + retry recovers them.
        import os
        import time
        os.environ["NEURON_RT_RESET_CORES"] = "1"
        time.sleep(30)
        res = run_bass_kernel_spmd(nc, in_maps,
                                   core_ids=list(range(N_CORES)))
    global LAST_RESULTS
    LAST_RESULTS = res

    out = np.empty((NPAD, D), np.float32)
    for c in range(N_CORES):
        oc = res.results[c]["out"]
        for s in range(SLOTS):
            t = tile_of[c, s]
            out[t * P:(t + 1) * P] = oc[s * P:(s + 1) * P]
    return out[:N_NODES]


# revision 12
# speedup vs baseline: 1.3246x; 1.0517x over previous
"""GNN message-passing aggregator on 8 Trainium2 NeuronCores.

Computes, for the full graph:
    node = entity_embed * out_sqrt_degree
    msg  = node[src] * edge_weight
    N_h  = segment_sum(msg, dst, N) * in_sqrt_degree
    out  = leaky_relu((entity_embed + N_h) @ W.T + b, 0.01)

Strategy (dst-partitioned edge shard, no collectives).  The kernel is
bound by SWDGE dma_gather descriptor generation on the Q7 cores
(~6-7ns per index per queue, 4 queues), so the host-side layout is
built to minimize descriptor count and keep every other engine under
that wall:

  * Node re-tiling: nodes are assigned to 392 tiles of 128 so each
    tile's in-edge count is EXACTLY 2048 (381 tiles) or 1792 (11) —
    snake-deal by degree + swap repair.  Zero block padding, and every
    core runs an identical SPMD program (tiles dealt 8 per slot).
  * Pair table: two bf16 node rows fit one 256-byte gather element, so
    one descriptor can feed TWO edges of the same tile.  Per core a
    pair table of 31232 entries (bounded by int16 indexing, <=1.31x the
    node table) covers 640 of each big tile's 2048 edges as 5 blocks of
    128 pairs; the remaining 768 edges gather singly from a zero-padded
    bf16 node table (256B rows) split in two 25088-row halves (A/B) for
    int16 indexing.  Descriptors per core: 31232 + 37632 = 68864 for
    100k edges (-35%).
  * Device, per tile: S[e, n] = bf16(dstl[e] == n) one-hot built on DVE,
    messages gm = bf16(g * ew), and nh[n, :] += S.T @ gm on the PE with
    S as the 128-column stationary operand (fast-weight-load path);
    pair blocks run two 64-column matmuls (one per half).  dstl/ew
    broadcasts live in PSUM so the DVE ops never take the DVE/GpSimd
    shared SBUF port — a 2-port DVE op would lock the SWDGE generators
    out of SBUF and stall the gathers.
  * Epilogue per tile: x = embed + nh (bf16), transpose x via the PE
    (identity matmul), out = leaky(xT.T @ W.T + b), leaky = max(x, .01x).
  * Gathers are chunked (16 single-blocks / 8 pair-blocks per op) and
    cycled over 4 SWDGE queues (single_packet=False lifts the
    64-descriptor packet cap).
"""

import json
import sys
import types

import numpy as np

P = 128
D = 64
N_NODES = 50000
N_CORES = 8
HALF = 25088            # int16-safe singles-table half (196 * 128)
NPAD = 2 * HALF         # 50176 = 392 tiles
NT = NPAD // P          # 392
SLOTS = NT // N_CORES   # 49
CHUNK_S = 16            # single-blocks per dma_gather
CHUNK_P = 8             # pair-blocks per dma_gather
NQ = 4                  # SWDGE queues (Q7 core pairs)

# per-slot profile: (pair blocks, single-A blocks, single-B blocks)
PROF = [(4, 3, 3)] + [(5, 3, 3)] * (SLOTS - 1)
TBP = sum(p for p, _, _ in PROF)     # 244 pair blocks per core
TBA = sum(a for _, a, _ in PROF)     # 147 A single blocks
TBB = sum(b for _, _, b in PROF)     # 147 B single blocks
TBS = TBA + TBB                      # 294
NPAIR_ENT = TBP * P                  # 31232 pair-table entries (< 2^15)


# ----------------------------------------------------------------------------
# Environment fixups (self-contained; kernel.py must run alone).
# ----------------------------------------------------------------------------

_SPLIT_COUNT = 0


def _split_multi_waits_json(bir: bytes) -> bytes:
    """This container's walrus accepts only ONE sync wait per instruction
    ('Too many sync wait commands'), while Tile's scheduler attaches
    several.  Rewrite each instruction with N>1 waits into N-1 same-engine
    NoOps (one wait each) followed by the instruction with the last wait;
    same-engine sequencer order makes this equivalent."""
    global _SPLIT_COUNT
    d = json.loads(bir)
    changed = False
    for fn in d.get("functions", []):
        for bb in fn.get("blocks", []):
            out = []
            for ins in bb.get("instructions", []):
                si = ins.get("sync_info") or {}
                ow = si.get("on_wait") or []
                if len(ow) > 1:
                    changed = True
                    for w in ow[:-1]:
                        _SPLIT_COUNT += 1
                        out.append({
                            "opcode": "NoOp",
                            "engine": ins.get("engine", "Unassigned"),
                            "name": f"I-waitsplit-{_SPLIT_COUNT}",
                            "ins": [],
                            "outs": [],
                            "sync_info": {"on_update": [], "on_wait": [w]},
                        })
                    si["on_wait"] = [ow[-1]]
                out.append(ins)
            bb["instructions"] = out
    return json.dumps(d).encode() if changed else bir


def _install_fixups():
    import concourse.bass_utils as bass_utils
    import concourse.bass2jax as bass2jax

    if not getattr(bass_utils, "_waitsplit_installed", False):
        bass_utils._waitsplit_installed = True
        orig_compile = bass_utils.compile_bir_kernel

        def patched_compile(bir_json, tmpdir, neff_name="file.neff"):
            if isinstance(bir_json, str):
                bir_json = bir_json.encode()
            return orig_compile(_split_multi_waits_json(bir_json), tmpdir,
                                neff_name=neff_name)

        bass_utils.compile_bir_kernel = patched_compile
        bass2jax.compile_bir_kernel = patched_compile
        # No artifact bucket in this container; keep profiles local.
        bass_utils.upload_artifacts = lambda tmpdir: tmpdir

    # run_bass_kernel_spmd(trace=True) under axon needs antenv.axon_hooks,
    # which this image doesn't ship.  Synthesize it and install the ctypes
    # NTFF hook from trn_agent_boot so neuron-profile works.
    if "antenv.axon_hooks" not in sys.modules:
        m = types.ModuleType("antenv.axon_hooks")
        m._hook = None
        m.set_axon_ntff_profile_hook = lambda h: setattr(m, "_hook", h)
        m.get_axon_ntff_profile_hook = lambda: m._hook
        sys.modules["antenv.axon_hooks"] = m
        try:
            import antenv
            antenv.axon_hooks = m
        except ImportError:
            pass
        try:
            from trn_agent_boot.trn_boot import _ntff_profile_via_ctypes
            hook = _ntff_profile_via_ctypes("/opt/axon/libaxon_pjrt.so")
            if hook is not None:
                m._hook = hook
        except Exception:
            pass


# ----------------------------------------------------------------------------
# Host-side graph partitioning
# ----------------------------------------------------------------------------

def _bf16():
    from ml_dtypes import bfloat16
    return bfloat16


def _wrap16(seg: np.ndarray) -> np.ndarray:
    """dma_gather index layout: index i lives at [i % 16, i // 16]."""
    assert seg.size % 16 == 0
    return seg.reshape(-1, 16).T


def _rebin(dst):
    """Assign nodes to 392 tiles of 128 nodes with per-tile in-edge sums
    of exactly 2048 (first 381 tiles) or 1792 (last 11): snake-deal the
    degree-sorted nodes, then repair residues with degree-delta swaps."""
    from collections import defaultdict
    deg = np.bincount(dst, minlength=NPAD).astype(np.int64)
    targets = np.array([2048] * 381 + [1792] * 11, np.int64)
    assert targets.sum() == deg.sum()
    order = np.argsort(-deg, kind="stable")
    bins = [[] for _ in range(NT)]
    for r in range(P):
        row = order[r * NT:(r + 1) * NT]
        seq = range(NT) if r % 2 == 0 else range(NT - 1, -1, -1)
        for k, t in enumerate(seq):
            bins[t].append(int(row[k]))
    sums = np.array([deg[np.array(b)].sum() for b in bins], np.int64)
    order_bins = np.argsort(sums)
    targets_of = np.full(NT, 2048, np.int64)
    targets_of[order_bins[:11]] = 1792
    diff = sums - targets_of
    assert diff.sum() == 0

    maps = []
    for t in range(NT):
        m = defaultdict(list)
        for n in bins[t]:
            m[deg[n]].append(n)
        maps.append(m)
    it = 0
    while diff.any():
        it += 1
        assert it < 200000, "rebin repair did not converge"
        i = int(np.argmax(diff))
        j = int(np.argmin(diff))
        want = int(min(diff[i], -diff[j]))
        done = False
        for delta in range(want, 0, -1):
            for da in sorted(maps[i].keys(), reverse=True):
                db = da - delta
                if db >= 0 and maps[j].get(db):
                    a = maps[i][da].pop()
                    if not maps[i][da]:
                        del maps[i][da]
                    b = maps[j][db].pop()
                    if not maps[j][db]:
                        del maps[j][db]
                    bins[i].remove(a)
                    bins[j].remove(b)
                    bins[i].append(b)
                    bins[j].append(a)
                    maps[i][db].append(b)
                    maps[j][da].append(a)
                    diff[i] -= delta
                    diff[j] += delta
                    done = True
                    break
            if done:
                break
        assert done, (i, j, diff[i], diff[j])
    tiles = [sorted(b) for b in bins]
    big = [t for t in range(NT) if targets_of[t] == 2048]
    small = [t for t in range(NT) if targets_of[t] == 1792]
    tiles = np.array([tiles[t] for t in big + small])
    for t in range(NT):
        assert deg[tiles[t]].sum() == targets[t]
    return tiles


def _prepare(entity_embed, src, dst, edge_weight, out_sqrt_degree,
             in_sqrt_degree):
    f32 = np.float32
    bf16 = _bf16()
    node = (entity_embed * out_sqrt_degree).astype(f32)
    node_bf = np.zeros((NPAD, D), bf16)
    node_bf[:N_NODES] = node.astype(bf16)
    emb_pad = np.zeros((NPAD, D), f32)
    emb_pad[:N_NODES] = entity_embed.astype(f32)
    ew2 = (edge_weight[:, 0] * in_sqrt_degree[dst, 0]).astype(f32)

    tiles = _rebin(dst)          # [392, 128] node ids; big tiles first
    tile_of_node = np.zeros(NPAD, np.int64)
    pos_of_node = np.zeros(NPAD, np.int64)
    for t in range(NT):
        tile_of_node[tiles[t]] = t
        pos_of_node[tiles[t]] = np.arange(P)

    # Deal: slot 0 <- 8 small tiles (381..388); slot 1 <- 3 small
    # (389..391) + 5 big; slots 2..48 <- remaining big.  Small tiles in
    # slot 1 pad up to the big profile with null pairs.
    deal = np.zeros((N_CORES, SLOTS), np.int64)
    deal[:, 0] = np.arange(381, 389)
    deal[:3, 1] = np.arange(389, 392)
    deal[3:, 1] = np.arange(0, 5)
    for s in range(2, SLOTS):
        deal[:, s] = np.arange(5 + (s - 2) * 8, 5 + (s - 1) * 8)
    assert sorted(deal.ravel().tolist()) == list(range(NT))

    # group edges by tile (edge ids per tile, any order)
    etile = tile_of_node[dst]
    eorder = np.argsort(etile, kind="stable")
    ecounts = np.bincount(etile, minlength=NT)
    estarts = np.concatenate([[0], np.cumsum(ecounts)])

    # idx column layouts (chunk-aligned)
    ncp = -(-TBP // CHUNK_P)             # pair chunks
    nca = -(-TBA // CHUNK_S)             # A single chunks
    ncb = -(-TBB // CHUNK_S)
    pcols = 8 * CHUNK_P * ncp
    acols = 8 * CHUNK_S * nca
    bcols = 8 * CHUNK_S * ncb
    scols = acols + bcols

    pidx_all = np.zeros((N_CORES, 16, pcols), np.int16)
    sidx_all = np.zeros((N_CORES, 16, scols), np.int16)
    dstl_p = np.zeros((N_CORES, P, TBP * 2), f32)
    ew_p = np.zeros((N_CORES, P, TBP * 2), f32)
    dstl_s = np.zeros((N_CORES, P, TBS), f32)
    ew_s = np.zeros((N_CORES, P, TBS), f32)
    emb_all = np.zeros((N_CORES, P, SLOTS * D), f32)
    ptab_u = np.zeros((N_CORES, NPAIR_ENT), np.int64)
    ptab_v = np.zeros((N_CORES, NPAIR_ENT), np.int64)

    pb0 = np.concatenate([[0], np.cumsum([p for p, _, _ in PROF])])
    ab0 = np.concatenate([[0], np.cumsum([a for _, a, _ in PROF])])
    bb0 = np.concatenate([[0], np.cumsum([b for _, _, b in PROF])])

    for c in range(N_CORES):
        for s in range(SLOTS):
            t = deal[c, s]
            pb, ab, bb = PROF[s]
            e = eorder[estarts[t]:estarts[t + 1]]
            n = len(e)
            esrc = src[e]
            caps = (pb * P, ab * P, bb * P)
            isA = esrc < HALF
            eA = e[isA]
            eB = e[~isA]
            npair = min(caps[0], n // 2)
            nsing = n - 2 * npair
            assert nsing <= caps[1] + caps[2], (n, npair, caps)
            # split singles by half under caps
            sa = min(len(eA), caps[1])
            sbn = nsing - sa
            if sbn > min(len(eB), caps[2]):
                sbn = min(len(eB), caps[2])
                sa = nsing - sbn
            assert 0 <= sa <= min(len(eA), caps[1]), (sa, len(eA), caps)
            assert 0 <= sbn <= min(len(eB), caps[2])
            singA = eA[:sa]
            singB = eB[:sbn]
            paired = np.concatenate([eA[sa:], eB[sbn:]])
            assert len(paired) == 2 * npair

            # pair entries for this slot: pb*P slots, first npair real
            ent0 = pb0[s] * P
            e1 = paired[0::2]
            e2 = paired[1::2]
            ptab_u[c, ent0:ent0 + npair] = src[e1]
            ptab_v[c, ent0:ent0 + npair] = src[e2]
            # idx: entry numbers ent0..ent0+pb*P-1 (pad entries repeat 0)
            ents = np.zeros(pb * P, np.int64)
            ents[:npair] = np.arange(ent0, ent0 + npair)
            ci = 8 * pb0[s]
            pidx_all[c, :, ci:ci + pb * P // 16] = _wrap16(
                ents.astype(np.int16))
            # dstl/ew for pairs: columns 2*blk+half
            k0 = pb0[s]
            dl = np.zeros((pb * P, 2), f32)
            wv = np.zeros((pb * P, 2), f32)
            dl[:npair, 0] = pos_of_node[dst[e1]]
            dl[:npair, 1] = pos_of_node[dst[e2]]
            wv[:npair, 0] = ew2[e1]
            wv[:npair, 1] = ew2[e2]
            # [pb*P, 2] -> blocks: edge j of block k at partition j%P?
            # stream layout: block k covers pair-slots [k*P, (k+1)*P) with
            # pair-slot p on partition p
            dl3 = dl.reshape(pb, P, 2)
            wv3 = wv.reshape(pb, P, 2)
            for k in range(pb):
                dstl_p[c, :, 2 * (k0 + k):2 * (k0 + k) + 2] = dl3[k]
                ew_p[c, :, 2 * (k0 + k):2 * (k0 + k) + 2] = wv3[k]

            # singles
            for (half, es, blks, col0, cbase, tblc) in (
                    (0, singA, ab, ab0[s], 0, 0),
                    (1, singB, bb, bb0[s], acols, TBA)):
                ns = len(es)
                idxs = np.zeros(blks * P, np.int64)
                idxs[:ns] = src[es] - half * HALF
                ci = cbase + 8 * col0
                sidx_all[c, :, ci:ci + blks * P // 16] = _wrap16(
                    idxs.astype(np.int16))
                dl = np.zeros(blks * P, f32)
                wv = np.zeros(blks * P, f32)
                dl[:ns] = pos_of_node[dst[es]]
                wv[:ns] = ew2[es]
                k0c = tblc + col0
                dstl_s[c, :, k0c:k0c + blks] = dl.reshape(blks, P).T
                ew_s[c, :, k0c:k0c + blks] = wv.reshape(blks, P).T

            emb_all[c, :, s * D:(s + 1) * D] = emb_pad[tiles[t]]

    # pair table contents (bf16 rows, 256B entries)
    ptabs = []
    for c in range(N_CORES):
        pt = np.zeros((NPAIR_ENT, 2 * D), bf16)
        pt[:, :D] = node_bf[ptab_u[c]]
        pt[:, D:] = node_bf[ptab_v[c]]
        ptabs.append(pt)
    # singles table: zero-padded bf16 rows (256B)
    stab = np.zeros((NPAD, 2 * D), bf16)
    stab[:, :D] = node_bf

    pidx_rep = np.tile(pidx_all, (1, 8, 1))
    sidx_rep = np.tile(sidx_all, (1, 8, 1))
    return (stab, ptabs, pidx_rep, sidx_rep, dstl_p, ew_p, dstl_s, ew_s,
            emb_all, deal, tiles, pcols, scols, acols)


# ----------------------------------------------------------------------------
# Device program
# ----------------------------------------------------------------------------

_PROGRAM_CACHE = {}


class _Stream:
    """Lazily emits chunked dma_gathers over one concatenated block
    stream.  Per chunk also emits ONE batched edge-weight multiply and
    ONE batched one-hot build so the DVE cost is amortized over the
    chunk.  block(i) yields the (lhsT, rhs) matmul operand pairs for
    block i (two pairs for pair-blocks, one for single-blocks)."""

    def __init__(self, nc, mybir, pool, gmpool, spool, table_ap, idx_segs,
                 chunk, pairs, blk_col0, total_blocks, dstl_ps, ew_ps,
                 iota_rep, qpick, f32, bf16):
        self.nc = nc
        self.mybir = mybir
        self.pool = pool
        self.gmpool = gmpool
        self.spool = spool
        self.table_ap = table_ap
        self.idx_segs = idx_segs   # (tile, chunk0, nchunks)
        self.chunk = chunk
        self.pairs = pairs         # True: 2 edges per gathered element
        self.blk_col0 = blk_col0   # column offset into dstl/ew for block 0
        self.total = total_blocks
        self.dstl_ps = dstl_ps
        self.ew_ps = ew_ps
        self.iota_rep = iota_rep
        self.qpick = qpick
        self.f32 = f32
        self.bf16 = bf16
        self.tiles = []

    def _idx_ap(self, k, cols):
        for t, c0, nch in self.idx_segs:
            if c0 <= k < c0 + nch:
                off = (k - c0) * 8 * self.chunk
                return t[:, off:off + cols]
        raise AssertionError(k)

    def _emit_chunk(self, k):
        nc = self.nc
        P_ = P
        nblk = min(self.chunk, self.total - k * self.chunk)
        g = self.pool.tile([P_, nblk, 2 * D], self.bf16)
        n = P_ * nblk
        nc.gpsimd.dma_gather(
            g[:], self.table_ap, self._idx_ap(k, n // 16), n, n, 2 * D,
            queue_num=self.qpick(), single_packet=False)
        if self.pairs:
            b0 = self.blk_col0 + 2 * self.chunk * k
            ncol = 2 * nblk
        else:
            b0 = self.blk_col0 + self.chunk * k
            ncol = nblk
        if self.pairs:
            # scale both halves: view [P, nblk, 2, D], ew col per half
            gm = self.gmpool.tile([P_, nblk, 2 * D], self.bf16)
            nc.vector.tensor_tensor(
                out=gm[:].rearrange("p k (h d) -> p (k h) d", h=2),
                in0=g[:].rearrange("p k (h d) -> p (k h) d", h=2),
                in1=self.ew_ps[:, b0:b0 + ncol].to_broadcast(
                    [P_, ncol, D]),
                op=self.mybir.AluOpType.mult)
        else:
            gm = self.gmpool.tile([P_, nblk, D], self.bf16)
            nc.vector.tensor_tensor(
                out=gm[:],
                in0=g[:, :, 0:D],
                in1=self.ew_ps[:, b0:b0 + ncol].to_broadcast(
                    [P_, ncol, D]),
                op=self.mybir.AluOpType.mult)
        S = self.spool.tile([P_, ncol, P_], self.bf16)
        nc.vector.tensor_tensor(
            out=S[:],
            in0=self.iota_rep[:, :ncol * P_].rearrange(
                "p (k n) -> p k n", n=P_),
            in1=self.dstl_ps[:, b0:b0 + ncol].to_broadcast(
                [P_, ncol, P_]),
            op=self.mybir.AluOpType.is_equal)
        self.tiles.append((S, gm))

    def block(self, i):
        k, off = divmod(i, self.chunk)
        while len(self.tiles) <= k:
            self._emit_chunk(len(self.tiles))
        S, gm = self.tiles[k]
        if self.pairs:
            return [(S[:, 2 * off, :], gm[:, off, 0:D]),
                    (S[:, 2 * off + 1, :], gm[:, off, D:2 * D])]
        return [(S[:, off, :], gm[:, off, :])]


def _build_program(pcols, scols, acols):
    key = (pcols, scols, acols)
    if key in _PROGRAM_CACHE:
        return _PROGRAM_CACHE[key]

    from concourse import bacc
    import concourse.mybir as mybir
    import concourse.tile as tile

    nc = bacc.Bacc("TRN2", num_swdge_queues=NQ)
    f32 = mybir.dt.float32
    bf16 = mybir.dt.bfloat16
    t_stab = nc.dram_tensor("stab", [NPAD, 2 * D], bf16,
                            kind="ExternalInput")
    t_ptab = nc.dram_tensor("ptab", [NPAIR_ENT, 2 * D], bf16,
                            kind="ExternalInput")
    t_pidx = nc.dram_tensor("pidx", [P, pcols], mybir.dt.int16,
                            kind="ExternalInput")
    t_sidx = nc.dram_tensor("sidx", [P, scols], mybir.dt.int16,
                            kind="ExternalInput")
    t_dstl_p = nc.dram_tensor("dstl_p", [P, TBP * 2], f32,
                              kind="ExternalInput")
    t_ew_p = nc.dram_tensor("ew_p", [P, TBP * 2], f32,
                            kind="ExternalInput")
    t_dstl_s = nc.dram_tensor("dstl_s", [P, TBS], f32,
                              kind="ExternalInput")
    t_ew_s = nc.dram_tensor("ew_s", [P, TBS], f32, kind="ExternalInput")
    t_emb = nc.dram_tensor("emb", [P, SLOTS * D], f32,
                           kind="ExternalInput")
    t_wt = nc.dram_tensor("wt", [D, D], bf16, kind="ExternalInput")
    t_b = nc.dram_tensor("bias", [1, D], bf16, kind="ExternalInput")
    t_iota = nc.dram_tensor("iota", [P, 2 * CHUNK_P * P], f32,
                            kind="ExternalInput")
    t_ident = nc.dram_tensor("ident", [P, P], bf16, kind="ExternalInput")
    t_out = nc.dram_tensor("out", [SLOTS * P, D], f32,
                           kind="ExternalOutput")

    qstate = [0]

    def qpick():
        q = qstate[0] % NQ
        qstate[0] += 1
        return q

    ncp = -(-TBP // CHUNK_P)
    nca = -(-TBA // CHUNK_S)
    ncb = -(-TBB // CHUNK_S)

    with tile.TileContext(nc) as tc:
        with tc.tile_pool(name="const", bufs=1) as cpool, \
             tc.tile_pool(name="gp", bufs=6) as gppool, \
             tc.tile_pool(name="ga", bufs=4) as gapool, \
             tc.tile_pool(name="gb", bufs=4) as gbpool, \
             tc.tile_pool(name="gmp", bufs=3) as gmppool, \
             tc.tile_pool(name="gma", bufs=3) as gmapool, \
             tc.tile_pool(name="gmb", bufs=3) as gmbpool, \
             tc.tile_pool(name="sp", bufs=3) as sppool, \
             tc.tile_pool(name="sa", bufs=3) as sapool, \
             tc.tile_pool(name="sb", bufs=3) as sbpool, \
             tc.tile_pool(name="small", bufs=3) as mpool, \
             tc.tile_pool(name="pscst", bufs=1, space="PSUM") as pscst, \
             tc.tile_pool(name="psnh", bufs=2, space="PSUM") as psnh, \
             tc.tile_pool(name="psxt", bufs=1, space="PSUM") as psxt, \
             tc.tile_pool(name="psout", bufs=1, space="PSUM") as psout:
            # idx group tiles (chunk-aligned) so the first gathers only
            # wait on their own small DMA
            def load_idx(tensor, nch, chunk, tag):
                segs = []
                ngrp = min(4, nch) or 1
                for gidx in range(ngrp):
                    lo = nch * gidx // ngrp
                    hi = nch * (gidx + 1) // ngrp
                    if hi == lo:
                        continue
                    w = (hi - lo) * 8 * chunk
                    tgt = cpool.tile([P, w], mybir.dt.int16,
                                     tag=f"{tag}{lo}")
                    nc.sync.dma_start(
                        out=tgt[:],
                        in_=tensor[:, lo * 8 * chunk:lo * 8 * chunk + w])
                    segs.append((tgt, lo, hi - lo))
                return segs

            psegs = load_idx(t_pidx, ncp, CHUNK_P, "pi")
            asegs = load_idx(t_sidx[:, 0:acols], nca, CHUNK_S, "ai")
            bsegs = load_idx(t_sidx[:, acols:scols], ncb, CHUNK_S, "bi")

            # dstl/ew staged in SBUF then copied into PSUM so DVE reads
            # them via its PSUM path (no shared-SBUF-port lock vs SWDGE)
            dstl_st = cpool.tile([P, TBP * 2 + TBS], f32)
            ew_st = cpool.tile([P, TBP * 2 + TBS], f32)
            nc.sync.dma_start(out=dstl_st[:, 0:TBP * 2], in_=t_dstl_p[:])
            nc.sync.dma_start(out=dstl_st[:, TBP * 2:], in_=t_dstl_s[:])
            nc.sync.dma_start(out=ew_st[:, 0:TBP * 2], in_=t_ew_p[:])
            nc.sync.dma_start(out=ew_st[:, TBP * 2:], in_=t_ew_s[:])
            dstl_ps = pscst.tile([P, TBP * 2 + TBS], f32, space="PSUM",
                                 padded_shape=[P, 1024])
            nc.vector.tensor_copy(out=dstl_ps[:], in_=dstl_st[:])
            ew_ps = pscst.tile([P, TBP * 2 + TBS], f32, space="PSUM",
                               padded_shape=[P, 1024])
            nc.vector.tensor_copy(out=ew_ps[:], in_=ew_st[:])

            iota_rep = cpool.tile([P, 2 * CHUNK_P * P], f32)
            nc.scalar.dma_start(out=iota_rep[:], in_=t_iota[:])
            ident_sb = cpool.tile([P, P], bf16)
            nc.scalar.dma_start(out=ident_sb[:], in_=t_ident[:])
            ones = cpool.tile([1, P], bf16)
            nc.vector.memset(ones[:], 1.0)
            wt_sb = cpool.tile([D, D], bf16)
            nc.scalar.dma_start(out=wt_sb[:], in_=t_wt[:])
            b_sb = cpool.tile([1, D], bf16)
            nc.scalar.dma_start(out=b_sb[:], in_=t_b[:])
            emb_sb = cpool.tile([P, SLOTS * D], f32)
            nc.scalar.dma_start(out=emb_sb[:], in_=t_emb[:])

            sp = _Stream(nc, mybir, gppool, gmppool, sppool, t_ptab[:, :],
                         psegs, CHUNK_P, True, 0, TBP, dstl_ps, ew_ps,
                         iota_rep, qpick, f32, bf16)
            sa = _Stream(nc, mybir, gapool, gmapool, sapool,
                         t_stab[0:HALF, :], asegs, CHUNK_S, False,
                         TBP * 2, TBA, dstl_ps, ew_ps, iota_rep, qpick,
                         f32, bf16)
            sb = _Stream(nc, mybir, gbpool, gmbpool, sbpool,
                         t_stab[HALF:NPAD, :], bsegs, CHUNK_S, False,
                         TBP * 2 + TBA, TBB, dstl_ps, ew_ps, iota_rep,
                         qpick, f32, bf16)

            p_off = a_off = b_off = 0
            for s in range(SLOTS):
                pb, ab, bb = PROF[s]
                mms = []
                for j in range(pb):
                    mms += sp.block(p_off + j)
                for j in range(ab):
                    mms += sa.block(a_off + j)
                for j in range(bb):
                    mms += sb.block(b_off + j)
                p_off += pb
                a_off += ab
                b_off += bb
                x_sb = mpool.tile([P, D], bf16, tag="x")
                nh = psnh.tile([P, D], f32, space="PSUM", tag="nh",
                               padded_shape=[P, 512])
                for i, (lhsT, rhs) in enumerate(mms):
                    nc.tensor.matmul(out=nh[:], lhsT=lhsT, rhs=rhs,
                                     start=(i == 0),
                                     stop=(i == len(mms) - 1))
                nc.vector.tensor_add(out=x_sb[:], in0=nh[:],
                                     in1=emb_sb[:, s * D:(s + 1) * D])
                xT_ps = psxt.tile([D, P], bf16, space="PSUM", tag="xt",
                                  padded_shape=[D, 1024])
                nc.tensor.matmul(out=xT_ps[:], lhsT=x_sb[:],
                                 rhs=ident_sb[:], is_transpose=True)
                xT_sb = mpool.tile([D, P], bf16, tag="xts")
                nc.vector.tensor_copy(out=xT_sb[:], in_=xT_ps[:])
                o_ps = psout.tile([P, D], f32, space="PSUM", tag="ops",
                                  padded_shape=[P, 512])
                nc.tensor.matmul(out=o_ps[:], lhsT=xT_sb[:], rhs=wt_sb[:],
                                 start=True, stop=False)
                nc.tensor.matmul(out=o_ps[:], lhsT=ones[:], rhs=b_sb[:],
                                 start=False, stop=True)
                o_scaled = mpool.tile([P, D], f32, tag="osc")
                nc.vector.tensor_scalar_mul(o_scaled[:], o_ps[:], 0.01)
                o_sb = mpool.tile([P, D], f32, tag="osb")
                nc.vector.tensor_tensor(out=o_sb[:], in0=o_ps[:],
                                        in1=o_scaled[:],
                                        op=mybir.AluOpType.max)
                nc.sync.dma_start(out=t_out[s * P:(s + 1) * P, :],
                                  in_=o_sb[:])

    nc.compile()
    _PROGRAM_CACHE[key] = nc
    return nc


LAST_RESULTS = None


def kernel(entity_embed, src, dst, edge_weight, out_sqrt_degree,
           in_sqrt_degree, W, b):
    _install_fixups()
    from concourse.bass_utils import run_bass_kernel_spmd

    bf16 = _bf16()
    entity_embed = np.asarray(entity_embed, np.float32)
    src = np.asarray(src).astype(np.int64)
    dst = np.asarray(dst).astype(np.int64)
    edge_weight = np.asarray(edge_weight, np.float32)
    out_sqrt_degree = np.asarray(out_sqrt_degree, np.float32)
    in_sqrt_degree = np.asarray(in_sqrt_degree, np.float32)
    W = np.asarray(W, np.float32)
    b = np.asarray(b, np.float32)

    (stab, ptabs, pidx_rep, sidx_rep, dstl_p, ew_p, dstl_s, ew_s, emb_all,
     deal, tiles, pcols, scols, acols) = _prepare(
        entity_embed, src, dst, edge_weight, out_sqrt_degree,
        in_sqrt_degree)

    nc = _build_program(pcols, scols, acols)

    wt = np.ascontiguousarray(W.T).astype(bf16)     # rhs[k, j] = W[j, k]
    iota_np = np.tile(np.tile(np.arange(P, dtype=np.float32),
                              2 * CHUNK_P), (P, 1))
    ident_np = np.eye(P, dtype=np.float32).astype(bf16)
    in_maps = []
    for c in range(N_CORES):
        in_maps.append({
            "stab": stab,
            "ptab": ptabs[c],
            "pidx": np.ascontiguousarray(pidx_rep[c]),
            "sidx": np.ascontiguousarray(sidx_rep[c]),
            "dstl_p": np.ascontiguousarray(dstl_p[c]),
            "ew_p": np.ascontiguousarray(ew_p[c]),
            "dstl_s": np.ascontiguousarray(dstl_s[c]),
            "ew_s": np.ascontiguousarray(ew_s[c]),
            "emb": np.ascontiguousarray(emb_all[c]),
            "wt": wt,
            "bias": b[None, :].astype(bf16),
            "iota": iota_np,
            "ident": ident_np,
        })

    try:
        res = run_bass_kernel_spmd(nc, in_maps,
                                   core_ids=list(range(N_CORES)))
    except Exception:
        # Transient NRT_EXEC_UNIT_UNRECOVERABLE states have been observed;
        # a reset + retry recovers them.
        import os
        import time
        os.environ["NEURON_RT_RESET_CORES"] = "1"
        time.sleep(30)
        res = run_bass_kernel_spmd(nc, in_maps,
                                   core_ids=list(range(N_CORES)))
    global LAST_RESULTS
    LAST_RESULTS = res

    out = np.empty((NPAD, D), np.float32)
    for c in range(N_CORES):
        oc = res.results[c]["out"]
        for s in range(SLOTS):
            out[tiles[deal[c, s]]] = oc[s * P:(s + 1) * P]
    return out[:N_NODES]


# revision 14
# speedup vs baseline: 1.4766x; 1.1148x over previous
"""GNN message-passing aggregator on 8 Trainium2 NeuronCores.

Computes, for the full graph:
    node = entity_embed * out_sqrt_degree
    msg  = node[src] * edge_weight
    N_h  = segment_sum(msg, dst, N) * in_sqrt_degree
    out  = leaky_relu((entity_embed + N_h) @ W.T + b, 0.01)

Strategy (dst-partitioned edge shard, no collectives).  The kernel is
bound by SWDGE dma_gather descriptor generation on the Q7 cores
(~6-7ns per index per queue, 4 queues), so the host-side layout is
built to minimize descriptor count and keep every other engine under
that wall:

  * Node re-tiling: nodes are assigned to 392 tiles of 128 so each
    tile's in-edge count is EXACTLY 2048 (381 tiles) or 1792 (11) —
    snake-deal by degree + swap repair.  Zero block padding, and every
    core runs an identical SPMD program (tiles dealt 8 per slot).
  * Pair table: two bf16 node rows fit one 256-byte gather element, so
    one descriptor can feed TWO edges of the same tile.  Per core a
    pair table of 31232 entries (bounded by int16 indexing, <=1.31x the
    node table) covers 640 of each big tile's 2048 edges as 5 blocks of
    128 pairs; the remaining 768 edges gather singly from a zero-padded
    bf16 node table (256B rows) split in two 25088-row halves (A/B) for
    int16 indexing.  Descriptors per core: 31232 + 37632 = 68864 for
    100k edges (-35%).
  * Device, per tile: S[e, n] = bf16(dstl[e] == n) one-hot built on DVE,
    messages gm = bf16(g * ew), and nh[n, :] += S.T @ gm on the PE with
    S as the 128-column stationary operand (fast-weight-load path);
    pair blocks run two 64-column matmuls (one per half).  dstl/ew
    broadcasts live in PSUM so the DVE ops never take the DVE/GpSimd
    shared SBUF port — a 2-port DVE op would lock the SWDGE generators
    out of SBUF and stall the gathers.
  * Epilogue per tile: x = embed + nh (bf16), transpose x via the PE
    (identity matmul), out = leaky(xT.T @ W.T + b), leaky = max(x, .01x).
  * Gathers are chunked (16 single-blocks / 8 pair-blocks per op) and
    cycled over 4 SWDGE queues (single_packet=False lifts the
    64-descriptor packet cap).
"""

import json
import sys
import types

import numpy as np

P = 128
D = 64
N_NODES = 50000
N_CORES = 8
HALF = 25088            # int16-safe singles-table half (196 * 128)
NPAD = 2 * HALF         # 50176 = 392 tiles
NT = NPAD // P          # 392
SLOTS = NT // N_CORES   # 49
CHUNK_S = 16            # single-blocks per dma_gather
CHUNK_P = 8             # pair-blocks per dma_gather
NQ = 4                  # SWDGE queues (Q7 core pairs)

# per-slot profile: (pair blocks, single-A blocks, single-B blocks)
PROF = [(4, 3, 3)] + [(5, 3, 3)] * (SLOTS - 1)
TBP = sum(p for p, _, _ in PROF)     # 244 pair blocks per core
TBA = sum(a for _, a, _ in PROF)     # 147 A single blocks
TBB = sum(b for _, _, b in PROF)     # 147 B single blocks
TBS = TBA + TBB                      # 294
NPAIR_ENT = TBP * P                  # 31232 pair-table entries (< 2^15)


# ----------------------------------------------------------------------------
# Environment fixups (self-contained; kernel.py must run alone).
# ----------------------------------------------------------------------------

_SPLIT_COUNT = 0


def _split_multi_waits_json(bir: bytes) -> bytes:
    """This container's walrus accepts only ONE sync wait per instruction
    ('Too many sync wait commands'), while Tile's scheduler attaches
    several.  Rewrite each instruction with N>1 waits into N-1 same-engine
    NoOps (one wait each) followed by the instruction with the last wait;
    same-engine sequencer order makes this equivalent."""
    global _SPLIT_COUNT
    d = json.loads(bir)
    changed = False
    for fn in d.get("functions", []):
        for bb in fn.get("blocks", []):
            out = []
            for ins in bb.get("instructions", []):
                si = ins.get("sync_info") or {}
                ow = si.get("on_wait") or []
                if len(ow) > 1:
                    changed = True
                    for w in ow[:-1]:
                        _SPLIT_COUNT += 1
                        out.append({
                            "opcode": "NoOp",
                            "engine": ins.get("engine", "Unassigned"),
                            "name": f"I-waitsplit-{_SPLIT_COUNT}",
                            "ins": [],
                            "outs": [],
                            "sync_info": {"on_update": [], "on_wait": [w]},
                        })
                    si["on_wait"] = [ow[-1]]
                out.append(ins)
            bb["instructions"] = out
    return json.dumps(d).encode() if changed else bir


def _install_fixups():
    import concourse.bass_utils as bass_utils
    import concourse.bass2jax as bass2jax

    if not getattr(bass_utils, "_waitsplit_installed", False):
        bass_utils._waitsplit_installed = True
        orig_compile = bass_utils.compile_bir_kernel

        def patched_compile(bir_json, tmpdir, neff_name="file.neff"):
            if isinstance(bir_json, str):
                bir_json = bir_json.encode()
            return orig_compile(_split_multi_waits_json(bir_json), tmpdir,
                                neff_name=neff_name)

        bass_utils.compile_bir_kernel = patched_compile
        bass2jax.compile_bir_kernel = patched_compile
        # No artifact bucket in this container; keep profiles local.
        bass_utils.upload_artifacts = lambda tmpdir: tmpdir

    # run_bass_kernel_spmd(trace=True) under axon needs antenv.axon_hooks,
    # which this image doesn't ship.  Synthesize it and install the ctypes
    # NTFF hook from trn_agent_boot so neuron-profile works.
    if "antenv.axon_hooks" not in sys.modules:
        m = types.ModuleType("antenv.axon_hooks")
        m._hook = None
        m.set_axon_ntff_profile_hook = lambda h: setattr(m, "_hook", h)
        m.get_axon_ntff_profile_hook = lambda: m._hook
        sys.modules["antenv.axon_hooks"] = m
        try:
            import antenv
            antenv.axon_hooks = m
        except ImportError:
            pass
        try:
            from trn_agent_boot.trn_boot import _ntff_profile_via_ctypes
            hook = _ntff_profile_via_ctypes("/opt/axon/libaxon_pjrt.so")
            if hook is not None:
                m._hook = hook
        except Exception:
            pass


# ----------------------------------------------------------------------------
# Host-side graph partitioning
# ----------------------------------------------------------------------------

def _bf16():
    from ml_dtypes import bfloat16
    return bfloat16


def _fp8():
    from ml_dtypes import float8_e4m3
    return float8_e4m3


def _wrap16(seg: np.ndarray) -> np.ndarray:
    """dma_gather index layout: index i lives at [i % 16, i // 16]."""
    assert seg.size % 16 == 0
    return seg.reshape(-1, 16).T


def _rebin(dst):
    """Assign nodes to 392 tiles of 128 nodes with per-tile in-edge sums
    of exactly 2048 (first 381 tiles) or 1792 (last 11): snake-deal the
    degree-sorted nodes, then repair residues with degree-delta swaps."""
    from collections import defaultdict
    deg = np.bincount(dst, minlength=NPAD).astype(np.int64)
    targets = np.array([2048] * 381 + [1792] * 11, np.int64)
    assert targets.sum() == deg.sum()
    order = np.argsort(-deg, kind="stable")
    bins = [[] for _ in range(NT)]
    for r in range(P):
        row = order[r * NT:(r + 1) * NT]
        seq = range(NT) if r % 2 == 0 else range(NT - 1, -1, -1)
        for k, t in enumerate(seq):
            bins[t].append(int(row[k]))
    sums = np.array([deg[np.array(b)].sum() for b in bins], np.int64)
    order_bins = np.argsort(sums)
    targets_of = np.full(NT, 2048, np.int64)
    targets_of[order_bins[:11]] = 1792
    diff = sums - targets_of
    assert diff.sum() == 0

    maps = []
    for t in range(NT):
        m = defaultdict(list)
        for n in bins[t]:
            m[deg[n]].append(n)
        maps.append(m)
    it = 0
    while diff.any():
        it += 1
        assert it < 200000, "rebin repair did not converge"
        i = int(np.argmax(diff))
        j = int(np.argmin(diff))
        want = int(min(diff[i], -diff[j]))
        done = False
        for delta in range(want, 0, -1):
            for da in sorted(maps[i].keys(), reverse=True):
                db = da - delta
                if db >= 0 and maps[j].get(db):
                    a = maps[i][da].pop()
                    if not maps[i][da]:
                        del maps[i][da]
                    b = maps[j][db].pop()
                    if not maps[j][db]:
                        del maps[j][db]
                    bins[i].remove(a)
                    bins[j].remove(b)
                    bins[i].append(b)
                    bins[j].append(a)
                    maps[i][db].append(b)
                    maps[j][da].append(a)
                    diff[i] -= delta
                    diff[j] += delta
                    done = True
                    break
            if done:
                break
        assert done, (i, j, diff[i], diff[j])
    tiles = [sorted(b) for b in bins]
    big = [t for t in range(NT) if targets_of[t] == 2048]
    small = [t for t in range(NT) if targets_of[t] == 1792]
    tiles = np.array([tiles[t] for t in big + small])
    for t in range(NT):
        assert deg[tiles[t]].sum() == targets[t]
    return tiles


def _prepare(entity_embed, src, dst, edge_weight, out_sqrt_degree,
             in_sqrt_degree):
    f32 = np.float32
    bf16 = _bf16()
    node = (entity_embed * out_sqrt_degree).astype(f32)
    node_bf = np.zeros((NPAD, D), bf16)
    node_bf[:N_NODES] = node.astype(bf16)
    emb_pad = np.zeros((NPAD, D), f32)
    emb_pad[:N_NODES] = entity_embed.astype(f32)
    ew2 = (edge_weight[:, 0] * in_sqrt_degree[dst, 0]).astype(f32)

    tiles = _rebin(dst)          # [392, 128] node ids; big tiles first
    tile_of_node = np.zeros(NPAD, np.int64)
    pos_of_node = np.zeros(NPAD, np.int64)
    for t in range(NT):
        tile_of_node[tiles[t]] = t
        pos_of_node[tiles[t]] = np.arange(P)

    # Deal: slot 0 <- 8 small tiles (381..388); slot 1 <- 3 small
    # (389..391) + 5 big; slots 2..48 <- remaining big.  Small tiles in
    # slot 1 pad up to the big profile with null pairs.
    deal = np.zeros((N_CORES, SLOTS), np.int64)
    deal[:, 0] = np.arange(381, 389)
    deal[:3, 1] = np.arange(389, 392)
    deal[3:, 1] = np.arange(0, 5)
    for s in range(2, SLOTS):
        deal[:, s] = np.arange(5 + (s - 2) * 8, 5 + (s - 1) * 8)
    assert sorted(deal.ravel().tolist()) == list(range(NT))

    # group edges by tile (edge ids per tile, any order)
    etile = tile_of_node[dst]
    eorder = np.argsort(etile, kind="stable")
    ecounts = np.bincount(etile, minlength=NT)
    estarts = np.concatenate([[0], np.cumsum(ecounts)])

    # idx column layouts (chunk-aligned)
    ncp = -(-TBP // CHUNK_P)             # pair chunks
    nca = -(-TBA // CHUNK_S)             # A single chunks
    ncb = -(-TBB // CHUNK_S)
    pcols = 8 * CHUNK_P * ncp
    acols = 8 * CHUNK_S * nca
    bcols = 8 * CHUNK_S * ncb
    scols = acols + bcols

    pidx_all = np.zeros((N_CORES, 16, pcols), np.int16)
    sidx_all = np.zeros((N_CORES, 16, scols), np.int16)
    dstl_p = np.zeros((N_CORES, P, TBP * 2), np.int64)
    ew_p = np.zeros((N_CORES, P, TBP * 2), f32)
    dstl_s = np.zeros((N_CORES, P, TBS), np.int64)
    ew_s = np.zeros((N_CORES, P, TBS), f32)
    emb_all = np.zeros((N_CORES, P, SLOTS * D), f32)
    ptab_u = np.zeros((N_CORES, NPAIR_ENT), np.int64)
    ptab_v = np.zeros((N_CORES, NPAIR_ENT), np.int64)

    pb0 = np.concatenate([[0], np.cumsum([p for p, _, _ in PROF])])
    ab0 = np.concatenate([[0], np.cumsum([a for _, a, _ in PROF])])
    bb0 = np.concatenate([[0], np.cumsum([b for _, _, b in PROF])])

    for c in range(N_CORES):
        for s in range(SLOTS):
            t = deal[c, s]
            pb, ab, bb = PROF[s]
            e = eorder[estarts[t]:estarts[t + 1]]
            n = len(e)
            esrc = src[e]
            caps = (pb * P, ab * P, bb * P)
            isA = esrc < HALF
            eA = e[isA]
            eB = e[~isA]
            npair = min(caps[0], n // 2)
            nsing = n - 2 * npair
            assert nsing <= caps[1] + caps[2], (n, npair, caps)
            # split singles by half under caps
            sa = min(len(eA), caps[1])
            sbn = nsing - sa
            if sbn > min(len(eB), caps[2]):
                sbn = min(len(eB), caps[2])
                sa = nsing - sbn
            assert 0 <= sa <= min(len(eA), caps[1]), (sa, len(eA), caps)
            assert 0 <= sbn <= min(len(eB), caps[2])
            singA = eA[:sa]
            singB = eB[:sbn]
            paired = np.concatenate([eA[sa:], eB[sbn:]])
            assert len(paired) == 2 * npair

            # pair entries for this slot: pb*P slots, first npair real
            ent0 = pb0[s] * P
            e1 = paired[0::2]
            e2 = paired[1::2]
            ptab_u[c, ent0:ent0 + npair] = src[e1]
            ptab_v[c, ent0:ent0 + npair] = src[e2]
            # idx: entry numbers ent0..ent0+pb*P-1 (pad entries repeat 0)
            ents = np.zeros(pb * P, np.int64)
            ents[:npair] = np.arange(ent0, ent0 + npair)
            ci = 8 * pb0[s]
            pidx_all[c, :, ci:ci + pb * P // 16] = _wrap16(
                ents.astype(np.int16))
            # dstl/ew for pairs: columns 2*blk+half
            k0 = pb0[s]
            dl = np.zeros((pb * P, 2), np.int64)
            wv = np.zeros((pb * P, 2), f32)
            dl[:npair, 0] = pos_of_node[dst[e1]]
            dl[:npair, 1] = pos_of_node[dst[e2]]
            wv[:npair, 0] = ew2[e1]
            wv[:npair, 1] = ew2[e2]
            # [pb*P, 2] -> blocks: edge j of block k at partition j%P?
            # stream layout: block k covers pair-slots [k*P, (k+1)*P) with
            # pair-slot p on partition p
            dl3 = dl.reshape(pb, P, 2)
            wv3 = wv.reshape(pb, P, 2)
            for k in range(pb):
                dstl_p[c, :, 2 * (k0 + k):2 * (k0 + k) + 2] = dl3[k]
                ew_p[c, :, 2 * (k0 + k):2 * (k0 + k) + 2] = wv3[k]

            # singles
            for (half, es, blks, col0, cbase, tblc) in (
                    (0, singA, ab, ab0[s], 0, 0),
                    (1, singB, bb, bb0[s], acols, TBA)):
                ns = len(es)
                idxs = np.zeros(blks * P, np.int64)
                idxs[:ns] = src[es] - half * HALF
                ci = cbase + 8 * col0
                sidx_all[c, :, ci:ci + blks * P // 16] = _wrap16(
                    idxs.astype(np.int16))
                dl = np.zeros(blks * P, np.int64)
                wv = np.zeros(blks * P, f32)
                dl[:ns] = pos_of_node[dst[es]]
                wv[:ns] = ew2[es]
                k0c = tblc + col0
                dstl_s[c, :, k0c:k0c + blks] = dl.reshape(blks, P).T
                ew_s[c, :, k0c:k0c + blks] = wv.reshape(blks, P).T

            emb_all[c, :, s * D:(s + 1) * D] = emb_pad[tiles[t]]

    # precomputed one-hot S in fp8 (1.0 exact): S8[e, c*128 + dstl] = 1
    fp8 = _fp8()
    ncols = TBP * 2 + TBS
    s8_all = np.zeros((N_CORES, P, ncols * P), fp8)
    one = fp8(1.0)
    rows = np.arange(P)[:, None]
    for c in range(N_CORES):
        dall = np.concatenate([dstl_p[c], dstl_s[c]], axis=1)  # [P, ncols]
        cols = np.arange(ncols)[None, :] * P + dall
        s8_all[c][rows, cols] = one

    # pair table contents (bf16 rows, 256B entries)
    ptabs = []
    for c in range(N_CORES):
        pt = np.zeros((NPAIR_ENT, 2 * D), bf16)
        pt[:, :D] = node_bf[ptab_u[c]]
        pt[:, D:] = node_bf[ptab_v[c]]
        ptabs.append(pt)
    # singles table: zero-padded bf16 rows (256B)
    stab = np.zeros((NPAD, 2 * D), bf16)
    stab[:, :D] = node_bf

    pidx_rep = np.tile(pidx_all, (1, 8, 1))
    sidx_rep = np.tile(sidx_all, (1, 8, 1))
    return (stab, ptabs, pidx_rep, sidx_rep, s8_all, ew_p, ew_s,
            emb_all, deal, tiles, pcols, scols, acols)


# ----------------------------------------------------------------------------
# Device program
# ----------------------------------------------------------------------------

_PROGRAM_CACHE = {}


class _Stream:
    """Lazily emits chunked dma_gathers over one concatenated block
    stream.  Per chunk also emits ONE batched edge-weight multiply and
    ONE batched one-hot build so the DVE cost is amortized over the
    chunk.  block(i) yields the (lhsT, rhs) matmul operand pairs for
    block i (two pairs for pair-blocks, one for single-blocks)."""

    def __init__(self, nc, mybir, pool, gmpool, spool, table_ap, idx_segs,
                 chunk, pairs, blk_col0, total_blocks, t_s8, ew_ps,
                 qpick, hwq, f32, bf16, fp8):
        self.nc = nc
        self.mybir = mybir
        self.pool = pool
        self.gmpool = gmpool
        self.spool = spool
        self.table_ap = table_ap
        self.idx_segs = idx_segs   # (tile, chunk0, nchunks)
        self.chunk = chunk
        self.pairs = pairs         # True: 2 edges per gathered element
        self.blk_col0 = blk_col0   # column offset into S/ew for block 0
        self.total = total_blocks
        self.t_s8 = t_s8
        self.ew_ps = ew_ps
        self.qpick = qpick
        self.hwq = hwq             # cycles the two HWDGE rings (sync/scalar)
        self.f32 = f32
        self.bf16 = bf16
        self.fp8 = fp8
        self.tiles = []

    def _idx_ap(self, k, cols):
        for t, c0, nch in self.idx_segs:
            if c0 <= k < c0 + nch:
                off = (k - c0) * 8 * self.chunk
                return t[:, off:off + cols]
        raise AssertionError(k)

    def _emit_chunk(self, k):
        nc = self.nc
        P_ = P
        nblk = min(self.chunk, self.total - k * self.chunk)
        g = self.pool.tile([P_, nblk, 2 * D], self.bf16)
        n = P_ * nblk
        nc.gpsimd.dma_gather(
            g[:], self.table_ap, self._idx_ap(k, n // 16), n, n, 2 * D,
            queue_num=self.qpick(n), single_packet=False)
        if self.pairs:
            b0 = self.blk_col0 + 2 * self.chunk * k
            ncol = 2 * nblk
        else:
            b0 = self.blk_col0 + self.chunk * k
            ncol = nblk
        if self.pairs:
            # scale both halves: view [P, nblk, 2, D], ew col per half
            gm = self.gmpool.tile([P_, nblk, 2 * D], self.bf16)
            nc.vector.tensor_tensor(
                out=gm[:].rearrange("p k (h d) -> p (k h) d", h=2),
                in0=g[:].rearrange("p k (h d) -> p (k h) d", h=2),
                in1=self.ew_ps[:, b0:b0 + ncol].to_broadcast(
                    [P_, ncol, D]),
                op=self.mybir.AluOpType.mult)
        else:
            gm = self.gmpool.tile([P_, nblk, D], self.bf16)
            nc.vector.tensor_tensor(
                out=gm[:],
                in0=g[:, :, 0:D],
                in1=self.ew_ps[:, b0:b0 + ncol].to_broadcast(
                    [P_, ncol, D]),
                op=self.mybir.AluOpType.mult)
        S = self.spool.tile([P_, ncol, P_], self.fp8)
        self.hwq().dma_start(
            out=S[:], in_=self.t_s8[:, b0 * P_:(b0 + ncol) * P_])
        self.tiles.append((S, gm))

    def block(self, i):
        k, off = divmod(i, self.chunk)
        while len(self.tiles) <= k:
            self._emit_chunk(len(self.tiles))
        S, gm = self.tiles[k]
        if self.pairs:
            return [(S[:, 2 * off, :], gm[:, off, 0:D]),
                    (S[:, 2 * off + 1, :], gm[:, off, D:2 * D])]
        return [(S[:, off, :], gm[:, off, :])]


def _build_program(pcols, scols, acols):
    key = (pcols, scols, acols)
    if key in _PROGRAM_CACHE:
        return _PROGRAM_CACHE[key]

    from concourse import bacc
    import concourse.mybir as mybir
    import concourse.tile as tile

    nc = bacc.Bacc("TRN2", num_swdge_queues=NQ)
    f32 = mybir.dt.float32
    bf16 = mybir.dt.bfloat16
    fp8 = mybir.dt.float8e4
    t_stab = nc.dram_tensor("stab", [NPAD, 2 * D], bf16,
                            kind="ExternalInput")
    t_ptab = nc.dram_tensor("ptab", [NPAIR_ENT, 2 * D], bf16,
                            kind="ExternalInput")
    t_pidx = nc.dram_tensor("pidx", [P, pcols], mybir.dt.int16,
                            kind="ExternalInput")
    t_sidx = nc.dram_tensor("sidx", [P, scols], mybir.dt.int16,
                            kind="ExternalInput")
    t_s8 = nc.dram_tensor("s8", [P, (TBP * 2 + TBS) * P], fp8,
                          kind="ExternalInput")
    t_ew_p = nc.dram_tensor("ew_p", [P, TBP * 2], f32,
                            kind="ExternalInput")
    t_ew_s = nc.dram_tensor("ew_s", [P, TBS], f32, kind="ExternalInput")
    t_emb = nc.dram_tensor("emb", [P, SLOTS * D], f32,
                           kind="ExternalInput")
    t_wt = nc.dram_tensor("wt", [D, D], bf16, kind="ExternalInput")
    t_b = nc.dram_tensor("bias", [1, D], bf16, kind="ExternalInput")
    t_ident = nc.dram_tensor("ident", [P, P], bf16, kind="ExternalInput")
    t_out = nc.dram_tensor("out", [SLOTS * P, D], f32,
                           kind="ExternalOutput")

    qload = [0] * NQ

    def qpick_n(n):
        q = min(range(NQ), key=lambda i: qload[i])
        qload[q] += n
        return q

    ncp = -(-TBP // CHUNK_P)
    nca = -(-TBA // CHUNK_S)
    ncb = -(-TBB // CHUNK_S)

    with tile.TileContext(nc) as tc:
        with tc.tile_pool(name="const", bufs=1) as cpool, \
             tc.tile_pool(name="gp", bufs=6) as gppool, \
             tc.tile_pool(name="ga", bufs=4) as gapool, \
             tc.tile_pool(name="gb", bufs=4) as gbpool, \
             tc.tile_pool(name="gmp", bufs=3) as gmppool, \
             tc.tile_pool(name="gma", bufs=3) as gmapool, \
             tc.tile_pool(name="gmb", bufs=3) as gmbpool, \
             tc.tile_pool(name="sp", bufs=3) as sppool, \
             tc.tile_pool(name="sa", bufs=3) as sapool, \
             tc.tile_pool(name="sb", bufs=3) as sbpool, \
             tc.tile_pool(name="small", bufs=3) as mpool, \
             tc.tile_pool(name="pscst", bufs=1, space="PSUM") as pscst, \
             tc.tile_pool(name="psnh", bufs=2, space="PSUM") as psnh, \
             tc.tile_pool(name="psxt", bufs=1, space="PSUM") as psxt, \
             tc.tile_pool(name="psout", bufs=1, space="PSUM") as psout:
            # idx group tiles (chunk-aligned) so the first gathers only
            # wait on their own small DMA
            def load_idx(tensor, nch, chunk, tag):
                segs = []
                ngrp = min(4, nch) or 1
                for gidx in range(ngrp):
                    lo = nch * gidx // ngrp
                    hi = nch * (gidx + 1) // ngrp
                    if hi == lo:
                        continue
                    w = (hi - lo) * 8 * chunk
                    tgt = cpool.tile([P, w], mybir.dt.int16,
                                     tag=f"{tag}{lo}")
                    nc.sync.dma_start(
                        out=tgt[:],
                        in_=tensor[:, lo * 8 * chunk:lo * 8 * chunk + w])
                    segs.append((tgt, lo, hi - lo))
                return segs

            psegs = load_idx(t_pidx, ncp, CHUNK_P, "pi")
            asegs = load_idx(t_sidx[:, 0:acols], nca, CHUNK_S, "ai")
            bsegs = load_idx(t_sidx[:, acols:scols], ncb, CHUNK_S, "bi")

            # ew staged in SBUF then copied into PSUM so DVE reads it
            # via its PSUM path (no shared-SBUF-port lock vs SWDGE)
            ew_st = cpool.tile([P, TBP * 2 + TBS], f32)
            nc.sync.dma_start(out=ew_st[:, 0:TBP * 2], in_=t_ew_p[:])
            nc.sync.dma_start(out=ew_st[:, TBP * 2:], in_=t_ew_s[:])
            ew_ps = pscst.tile([P, TBP * 2 + TBS], f32, space="PSUM",
                               padded_shape=[P, 1024])
            nc.vector.tensor_copy(out=ew_ps[:], in_=ew_st[:])

            hwstate = [0]

            def hwq():
                hwstate[0] += 1
                return nc.sync if hwstate[0] % 2 else nc.scalar

            ident_sb = cpool.tile([P, P], bf16)
            nc.scalar.dma_start(out=ident_sb[:], in_=t_ident[:])
            ones = cpool.tile([1, P], bf16)
            nc.vector.memset(ones[:], 1.0)
            wt_sb = cpool.tile([D, D], bf16)
            nc.scalar.dma_start(out=wt_sb[:], in_=t_wt[:])
            b_sb = cpool.tile([1, D], bf16)
            nc.scalar.dma_start(out=b_sb[:], in_=t_b[:])
            emb_sb = cpool.tile([P, SLOTS * D], f32)
            nc.scalar.dma_start(out=emb_sb[:], in_=t_emb[:])

            sp = _Stream(nc, mybir, gppool, gmppool, sppool, t_ptab[:, :],
                         psegs, CHUNK_P, True, 0, TBP, t_s8, ew_ps,
                         qpick_n, hwq, f32, bf16, fp8)
            sa = _Stream(nc, mybir, gapool, gmapool, sapool,
                         t_stab[0:HALF, :], asegs, CHUNK_S, False,
                         TBP * 2, TBA, t_s8, ew_ps, qpick_n, hwq,
                         f32, bf16, fp8)
            sb = _Stream(nc, mybir, gbpool, gmbpool, sbpool,
                         t_stab[HALF:NPAD, :], bsegs, CHUNK_S, False,
                         TBP * 2 + TBA, TBB, t_s8, ew_ps, qpick_n, hwq,
                         f32, bf16, fp8)

            p_off = a_off = b_off = 0
            for s in range(SLOTS):
                pb, ab, bb = PROF[s]
                mms = []
                for j in range(pb):
                    mms += sp.block(p_off + j)
                for j in range(ab):
                    mms += sa.block(a_off + j)
                for j in range(bb):
                    mms += sb.block(b_off + j)
                p_off += pb
                a_off += ab
                b_off += bb
                x_sb = mpool.tile([P, D], bf16, tag="x")
                nh = psnh.tile([P, D], f32, space="PSUM", tag="nh",
                               padded_shape=[P, 512])
                for i, (lhsT, rhs) in enumerate(mms):
                    nc.tensor.matmul(out=nh[:], lhsT=lhsT, rhs=rhs,
                                     start=(i == 0),
                                     stop=(i == len(mms) - 1))
                nc.vector.tensor_add(out=x_sb[:], in0=nh[:],
                                     in1=emb_sb[:, s * D:(s + 1) * D])
                xT_ps = psxt.tile([D, P], bf16, space="PSUM", tag="xt",
                                  padded_shape=[D, 1024])
                nc.tensor.matmul(out=xT_ps[:], lhsT=x_sb[:],
                                 rhs=ident_sb[:], is_transpose=True)
                xT_sb = mpool.tile([D, P], bf16, tag="xts")
                nc.vector.tensor_copy(out=xT_sb[:], in_=xT_ps[:])
                o_ps = psout.tile([P, D], f32, space="PSUM", tag="ops",
                                  padded_shape=[P, 512])
                nc.tensor.matmul(out=o_ps[:], lhsT=xT_sb[:], rhs=wt_sb[:],
                                 start=True, stop=False)
                nc.tensor.matmul(out=o_ps[:], lhsT=ones[:], rhs=b_sb[:],
                                 start=False, stop=True)
                o_scaled = mpool.tile([P, D], f32, tag="osc")
                nc.vector.tensor_scalar_mul(o_scaled[:], o_ps[:], 0.01)
                o_sb = mpool.tile([P, D], f32, tag="osb")
                nc.vector.tensor_tensor(out=o_sb[:], in0=o_ps[:],
                                        in1=o_scaled[:],
                                        op=mybir.AluOpType.max)
                nc.sync.dma_start(out=t_out[s * P:(s + 1) * P, :],
                                  in_=o_sb[:])

    nc.compile()
    _PROGRAM_CACHE[key] = nc
    return nc


LAST_RESULTS = None


def kernel(entity_embed, src, dst, edge_weight, out_sqrt_degree,
           in_sqrt_degree, W, b):
    _install_fixups()
    from concourse.bass_utils import run_bass_kernel_spmd

    bf16 = _bf16()
    entity_embed = np.asarray(entity_embed, np.float32)
    src = np.asarray(src).astype(np.int64)
    dst = np.asarray(dst).astype(np.int64)
    edge_weight = np.asarray(edge_weight, np.float32)
    out_sqrt_degree = np.asarray(out_sqrt_degree, np.float32)
    in_sqrt_degree = np.asarray(in_sqrt_degree, np.float32)
    W = np.asarray(W, np.float32)
    b = np.asarray(b, np.float32)

    (stab, ptabs, pidx_rep, sidx_rep, s8_all, ew_p, ew_s, emb_all,
     deal, tiles, pcols, scols, acols) = _prepare(
        entity_embed, src, dst, edge_weight, out_sqrt_degree,
        in_sqrt_degree)

    nc = _build_program(pcols, scols, acols)

    wt = np.ascontiguousarray(W.T).astype(bf16)     # rhs[k, j] = W[j, k]
    ident_np = np.eye(P, dtype=np.float32).astype(bf16)
    in_maps = []
    for c in range(N_CORES):
        in_maps.append({
            "stab": stab,
            "ptab": ptabs[c],
            "pidx": np.ascontiguousarray(pidx_rep[c]),
            "sidx": np.ascontiguousarray(sidx_rep[c]),
            "s8": s8_all[c],
            "ew_p": np.ascontiguousarray(ew_p[c]),
            "ew_s": np.ascontiguousarray(ew_s[c]),
            "emb": np.ascontiguousarray(emb_all[c]),
            "wt": wt,
            "bias": b[None, :].astype(bf16),
            "ident": ident_np,
        })

    try:
        res = run_bass_kernel_spmd(nc, in_maps,
                                   core_ids=list(range(N_CORES)))
    except Exception:
        # Transient NRT_EXEC_UNIT_UNRECOVERABLE states have been observed;
        # a reset + retry recovers them.
        import os
        import time
        os.environ["NEURON_RT_RESET_CORES"] = "1"
        time.sleep(30)
        res = run_bass_kernel_spmd(nc, in_maps,
                                   core_ids=list(range(N_CORES)))
    global LAST_RESULTS
    LAST_RESULTS = res

    out = np.empty((NPAD, D), np.float32)
    for c in range(N_CORES):
        oc = res.results[c]["out"]
        for s in range(SLOTS):
            out[tiles[deal[c, s]]] = oc[s * P:(s + 1) * P]
    return out[:N_NODES]


# revision 15
# speedup vs baseline: 1.8200x; 1.2325x over previous
"""GNN message-passing aggregator on 8 Trainium2 NeuronCores.

Computes, for the full graph:
    node = entity_embed * out_sqrt_degree
    msg  = node[src] * edge_weight
    N_h  = segment_sum(msg, dst, N) * in_sqrt_degree
    out  = leaky_relu((entity_embed + N_h) @ W.T + b, 0.01)

Strategy (dst-partitioned edge shard, no collectives).  The kernel is
bound by SWDGE dma_gather descriptor generation on the Q7 cores
(~6-7ns per index per queue, 4 queues), so the host-side layout is
built to minimize descriptor count and keep every other engine under
that wall:

  * Node re-tiling: nodes are assigned to 392 tiles of 128 so each
    tile's in-edge count is EXACTLY 2048 (381 tiles) or 1792 (11) —
    snake-deal by degree + swap repair.  Zero block padding, and every
    core runs an identical SPMD program (tiles dealt 8 per slot).
  * Pair table: two bf16 node rows fit one 256-byte gather element, so
    one descriptor can feed TWO edges of the same tile.  Per core a
    pair table of 31232 entries (bounded by int16 indexing, <=1.31x the
    node table) covers 640 of each big tile's 2048 edges as 5 blocks of
    128 pairs; the remaining 768 edges gather singly from a zero-padded
    bf16 node table (256B rows) split in two 25088-row halves (A/B) for
    int16 indexing.  Descriptors per core: 31232 + 37632 = 68864 for
    100k edges (-35%).
  * Device, per tile: S[e, n] = bf16(dstl[e] == n) one-hot built on DVE,
    messages gm = bf16(g * ew), and nh[n, :] += S.T @ gm on the PE with
    S as the 128-column stationary operand (fast-weight-load path);
    pair blocks run two 64-column matmuls (one per half).  dstl/ew
    broadcasts live in PSUM so the DVE ops never take the DVE/GpSimd
    shared SBUF port — a 2-port DVE op would lock the SWDGE generators
    out of SBUF and stall the gathers.
  * Epilogue per tile: x = embed + nh (bf16), transpose x via the PE
    (identity matmul), out = leaky(xT.T @ W.T + b), leaky = max(x, .01x).
  * Gathers are chunked (16 single-blocks / 8 pair-blocks per op) and
    cycled over 4 SWDGE queues (single_packet=False lifts the
    64-descriptor packet cap).
"""

import json
import sys
import types

import numpy as np

P = 128
D = 64
N_NODES = 50000
N_CORES = 8
HALF = 25088            # int16-safe singles-table half (196 * 128)
NPAD = 2 * HALF         # 50176 = 392 tiles
NT = NPAD // P          # 392
SLOTS = NT // N_CORES   # 49
CHUNK_S = 16            # single-blocks per dma_gather
CHUNK_P = 16            # pair-blocks per dma_gather
NQ = 4                  # SWDGE queues (Q7 core pairs)

# per-slot profile: (pair blocks, single-A blocks, single-B blocks)
PROF = [(4, 3, 3)] + [(5, 3, 3)] * (SLOTS - 1)
TBP = sum(p for p, _, _ in PROF)     # 244 pair blocks per core
TBA = sum(a for _, a, _ in PROF)     # 147 A single blocks
TBB = sum(b for _, _, b in PROF)     # 147 B single blocks
TBS = TBA + TBB                      # 294
NPAIR_ENT = TBP * P                  # 31232 pair-table entries (< 2^15)


# ----------------------------------------------------------------------------
# Environment fixups (self-contained; kernel.py must run alone).
# ----------------------------------------------------------------------------

_SPLIT_COUNT = 0


def _split_multi_waits_json(bir: bytes) -> bytes:
    """This container's walrus accepts only ONE sync wait per instruction
    ('Too many sync wait commands'), while Tile's scheduler attaches
    several.  Rewrite each instruction with N>1 waits into N-1 same-engine
    NoOps (one wait each) followed by the instruction with the last wait;
    same-engine sequencer order makes this equivalent."""
    global _SPLIT_COUNT
    d = json.loads(bir)
    changed = False
    for fn in d.get("functions", []):
        for bb in fn.get("blocks", []):
            out = []
            for ins in bb.get("instructions", []):
                si = ins.get("sync_info") or {}
                ow = si.get("on_wait") or []
                if len(ow) > 1:
                    changed = True
                    for w in ow[:-1]:
                        _SPLIT_COUNT += 1
                        out.append({
                            "opcode": "NoOp",
                            "engine": ins.get("engine", "Unassigned"),
                            "name": f"I-waitsplit-{_SPLIT_COUNT}",
                            "ins": [],
                            "outs": [],
                            "sync_info": {"on_update": [], "on_wait": [w]},
                        })
                    si["on_wait"] = [ow[-1]]
                out.append(ins)
            bb["instructions"] = out
    return json.dumps(d).encode() if changed else bir


def _install_fixups():
    import concourse.bass_utils as bass_utils
    import concourse.bass2jax as bass2jax

    if not getattr(bass_utils, "_waitsplit_installed", False):
        bass_utils._waitsplit_installed = True
        orig_compile = bass_utils.compile_bir_kernel

        def patched_compile(bir_json, tmpdir, neff_name="file.neff"):
            if isinstance(bir_json, str):
                bir_json = bir_json.encode()
            return orig_compile(_split_multi_waits_json(bir_json), tmpdir,
                                neff_name=neff_name)

        bass_utils.compile_bir_kernel = patched_compile
        bass2jax.compile_bir_kernel = patched_compile
        # No artifact bucket in this container; keep profiles local.
        bass_utils.upload_artifacts = lambda tmpdir: tmpdir

    # run_bass_kernel_spmd(trace=True) under axon needs antenv.axon_hooks,
    # which this image doesn't ship.  Synthesize it and install the ctypes
    # NTFF hook from trn_agent_boot so neuron-profile works.
    if "antenv.axon_hooks" not in sys.modules:
        m = types.ModuleType("antenv.axon_hooks")
        m._hook = None
        m.set_axon_ntff_profile_hook = lambda h: setattr(m, "_hook", h)
        m.get_axon_ntff_profile_hook = lambda: m._hook
        sys.modules["antenv.axon_hooks"] = m
        try:
            import antenv
            antenv.axon_hooks = m
        except ImportError:
            pass
        try:
            from trn_agent_boot.trn_boot import _ntff_profile_via_ctypes
            hook = _ntff_profile_via_ctypes("/opt/axon/libaxon_pjrt.so")
            if hook is not None:
                m._hook = hook
        except Exception:
            pass


# ----------------------------------------------------------------------------
# Host-side graph partitioning
# ----------------------------------------------------------------------------

def _bf16():
    from ml_dtypes import bfloat16
    return bfloat16


def _fp8():
    from ml_dtypes import float8_e4m3
    return float8_e4m3


def _wrap16(seg: np.ndarray) -> np.ndarray:
    """dma_gather index layout: index i lives at [i % 16, i // 16]."""
    assert seg.size % 16 == 0
    return seg.reshape(-1, 16).T


def _rebin(dst):
    """Assign nodes to 392 tiles of 128 nodes with per-tile in-edge sums
    of exactly 2048 (first 381 tiles) or 1792 (last 11): snake-deal the
    degree-sorted nodes, then repair residues with degree-delta swaps."""
    from collections import defaultdict
    deg = np.bincount(dst, minlength=NPAD).astype(np.int64)
    targets = np.array([2048] * 381 + [1792] * 11, np.int64)
    assert targets.sum() == deg.sum()
    order = np.argsort(-deg, kind="stable")
    bins = [[] for _ in range(NT)]
    for r in range(P):
        row = order[r * NT:(r + 1) * NT]
        seq = range(NT) if r % 2 == 0 else range(NT - 1, -1, -1)
        for k, t in enumerate(seq):
            bins[t].append(int(row[k]))
    sums = np.array([deg[np.array(b)].sum() for b in bins], np.int64)
    order_bins = np.argsort(sums)
    targets_of = np.full(NT, 2048, np.int64)
    targets_of[order_bins[:11]] = 1792
    diff = sums - targets_of
    assert diff.sum() == 0

    maps = []
    for t in range(NT):
        m = defaultdict(list)
        for n in bins[t]:
            m[deg[n]].append(n)
        maps.append(m)
    it = 0
    while diff.any():
        it += 1
        assert it < 200000, "rebin repair did not converge"
        i = int(np.argmax(diff))
        j = int(np.argmin(diff))
        want = int(min(diff[i], -diff[j]))
        done = False
        for delta in range(want, 0, -1):
            for da in sorted(maps[i].keys(), reverse=True):
                db = da - delta
                if db >= 0 and maps[j].get(db):
                    a = maps[i][da].pop()
                    if not maps[i][da]:
                        del maps[i][da]
                    b = maps[j][db].pop()
                    if not maps[j][db]:
                        del maps[j][db]
                    bins[i].remove(a)
                    bins[j].remove(b)
                    bins[i].append(b)
                    bins[j].append(a)
                    maps[i][db].append(b)
                    maps[j][da].append(a)
                    diff[i] -= delta
                    diff[j] += delta
                    done = True
                    break
            if done:
                break
        assert done, (i, j, diff[i], diff[j])
    tiles = [sorted(b) for b in bins]
    big = [t for t in range(NT) if targets_of[t] == 2048]
    small = [t for t in range(NT) if targets_of[t] == 1792]
    tiles = np.array([tiles[t] for t in big + small])
    for t in range(NT):
        assert deg[tiles[t]].sum() == targets[t]
    return tiles


def _prepare(entity_embed, src, dst, edge_weight, out_sqrt_degree,
             in_sqrt_degree):
    f32 = np.float32
    bf16 = _bf16()
    node = (entity_embed * out_sqrt_degree).astype(f32)
    node_bf = np.zeros((NPAD, D), bf16)
    node_bf[:N_NODES] = node.astype(bf16)
    emb_pad = np.zeros((NPAD, D), f32)
    emb_pad[:N_NODES] = entity_embed.astype(f32)
    ew2 = (edge_weight[:, 0] * in_sqrt_degree[dst, 0]).astype(f32)

    tiles = _rebin(dst)          # [392, 128] node ids; big tiles first
    tile_of_node = np.zeros(NPAD, np.int64)
    pos_of_node = np.zeros(NPAD, np.int64)
    for t in range(NT):
        tile_of_node[tiles[t]] = t
        pos_of_node[tiles[t]] = np.arange(P)

    # Deal: slot 0 <- 8 small tiles (381..388); slot 1 <- 3 small
    # (389..391) + 5 big; slots 2..48 <- remaining big.  Small tiles in
    # slot 1 pad up to the big profile with null pairs.
    deal = np.zeros((N_CORES, SLOTS), np.int64)
    deal[:, 0] = np.arange(381, 389)
    deal[:3, 1] = np.arange(389, 392)
    deal[3:, 1] = np.arange(0, 5)
    for s in range(2, SLOTS):
        deal[:, s] = np.arange(5 + (s - 2) * 8, 5 + (s - 1) * 8)
    assert sorted(deal.ravel().tolist()) == list(range(NT))

    # group edges by tile (edge ids per tile, any order)
    etile = tile_of_node[dst]
    eorder = np.argsort(etile, kind="stable")
    ecounts = np.bincount(etile, minlength=NT)
    estarts = np.concatenate([[0], np.cumsum(ecounts)])

    # idx column layouts (chunk-aligned)
    ncp = -(-TBP // CHUNK_P)             # pair chunks
    nca = -(-TBA // CHUNK_S)             # A single chunks
    ncb = -(-TBB // CHUNK_S)
    pcols = 8 * CHUNK_P * ncp
    acols = 8 * CHUNK_S * nca
    bcols = 8 * CHUNK_S * ncb
    scols = acols + bcols

    pidx_all = np.zeros((N_CORES, 16, pcols), np.int16)
    sidx_all = np.zeros((N_CORES, 16, scols), np.int16)
    dstl_p = np.zeros((N_CORES, P, TBP * 2), np.int64)
    ew_p = np.zeros((N_CORES, P, TBP * 2), f32)
    dstl_s = np.zeros((N_CORES, P, TBS), np.int64)
    ew_s = np.zeros((N_CORES, P, TBS), f32)
    emb_all = np.zeros((N_CORES, P, SLOTS * D), f32)
    ptab_u = np.zeros((N_CORES, NPAIR_ENT), np.int64)
    ptab_v = np.zeros((N_CORES, NPAIR_ENT), np.int64)

    pb0 = np.concatenate([[0], np.cumsum([p for p, _, _ in PROF])])
    ab0 = np.concatenate([[0], np.cumsum([a for _, a, _ in PROF])])
    bb0 = np.concatenate([[0], np.cumsum([b for _, _, b in PROF])])

    for c in range(N_CORES):
        for s in range(SLOTS):
            t = deal[c, s]
            pb, ab, bb = PROF[s]
            e = eorder[estarts[t]:estarts[t + 1]]
            n = len(e)
            esrc = src[e]
            caps = (pb * P, ab * P, bb * P)
            isA = esrc < HALF
            eA = e[isA]
            eB = e[~isA]
            npair = min(caps[0], n // 2)
            nsing = n - 2 * npair
            assert nsing <= caps[1] + caps[2], (n, npair, caps)
            # split singles by half under caps
            sa = min(len(eA), caps[1])
            sbn = nsing - sa
            if sbn > min(len(eB), caps[2]):
                sbn = min(len(eB), caps[2])
                sa = nsing - sbn
            assert 0 <= sa <= min(len(eA), caps[1]), (sa, len(eA), caps)
            assert 0 <= sbn <= min(len(eB), caps[2])
            singA = eA[:sa]
            singB = eB[:sbn]
            paired = np.concatenate([eA[sa:], eB[sbn:]])
            assert len(paired) == 2 * npair

            # pair entries for this slot: pb*P slots, first npair real
            ent0 = pb0[s] * P
            e1 = paired[0::2]
            e2 = paired[1::2]
            ptab_u[c, ent0:ent0 + npair] = src[e1]
            ptab_v[c, ent0:ent0 + npair] = src[e2]
            # idx: entry numbers ent0..ent0+pb*P-1 (pad entries repeat 0)
            ents = np.zeros(pb * P, np.int64)
            ents[:npair] = np.arange(ent0, ent0 + npair)
            ci = 8 * pb0[s]
            pidx_all[c, :, ci:ci + pb * P // 16] = _wrap16(
                ents.astype(np.int16))
            # dstl/ew for pairs: columns 2*blk+half
            k0 = pb0[s]
            dl = np.zeros((pb * P, 2), np.int64)
            wv = np.zeros((pb * P, 2), f32)
            dl[:npair, 0] = pos_of_node[dst[e1]]
            dl[:npair, 1] = pos_of_node[dst[e2]]
            wv[:npair, 0] = ew2[e1]
            wv[:npair, 1] = ew2[e2]
            # [pb*P, 2] -> blocks: edge j of block k at partition j%P?
            # stream layout: block k covers pair-slots [k*P, (k+1)*P) with
            # pair-slot p on partition p
            dl3 = dl.reshape(pb, P, 2)
            wv3 = wv.reshape(pb, P, 2)
            for k in range(pb):
                dstl_p[c, :, 2 * (k0 + k):2 * (k0 + k) + 2] = dl3[k]
                ew_p[c, :, 2 * (k0 + k):2 * (k0 + k) + 2] = wv3[k]

            # singles
            for (half, es, blks, col0, cbase, tblc) in (
                    (0, singA, ab, ab0[s], 0, 0),
                    (1, singB, bb, bb0[s], acols, TBA)):
                ns = len(es)
                idxs = np.zeros(blks * P, np.int64)
                idxs[:ns] = src[es] - half * HALF
                ci = cbase + 8 * col0
                sidx_all[c, :, ci:ci + blks * P // 16] = _wrap16(
                    idxs.astype(np.int16))
                dl = np.zeros(blks * P, np.int64)
                wv = np.zeros(blks * P, f32)
                dl[:ns] = pos_of_node[dst[es]]
                wv[:ns] = ew2[es]
                k0c = tblc + col0
                dstl_s[c, :, k0c:k0c + blks] = dl.reshape(blks, P).T
                ew_s[c, :, k0c:k0c + blks] = wv.reshape(blks, P).T

            emb_all[c, :, s * D:(s + 1) * D] = emb_pad[tiles[t]]

    # precomputed one-hot S in fp8 (1.0 exact): S8[e, c*128 + dstl] = 1
    fp8 = _fp8()
    ncols = TBP * 2 + TBS
    s8_all = np.zeros((N_CORES, P, ncols * P), fp8)
    one = fp8(1.0)
    rows = np.arange(P)[:, None]
    for c in range(N_CORES):
        dall = np.concatenate([dstl_p[c], dstl_s[c]], axis=1)  # [P, ncols]
        cols = np.arange(ncols)[None, :] * P + dall
        s8_all[c][rows, cols] = one

    # pair table contents (bf16 rows, 256B entries)
    ptabs = []
    for c in range(N_CORES):
        pt = np.zeros((NPAIR_ENT, 2 * D), bf16)
        pt[:, :D] = node_bf[ptab_u[c]]
        pt[:, D:] = node_bf[ptab_v[c]]
        ptabs.append(pt)
    # singles table: zero-padded bf16 rows (256B)
    stab = np.zeros((NPAD, 2 * D), bf16)
    stab[:, :D] = node_bf

    pidx_rep = np.tile(pidx_all, (1, 8, 1))
    sidx_rep = np.tile(sidx_all, (1, 8, 1))
    return (stab, ptabs, pidx_rep, sidx_rep, s8_all, ew_p, ew_s,
            emb_all, deal, tiles, pcols, scols, acols)


# ----------------------------------------------------------------------------
# Device program
# ----------------------------------------------------------------------------

_PROGRAM_CACHE = {}


class _Stream:
    """Lazily emits chunked dma_gathers over one concatenated block
    stream.  Per chunk also emits ONE batched edge-weight multiply and
    ONE batched one-hot build so the DVE cost is amortized over the
    chunk.  block(i) yields the (lhsT, rhs) matmul operand pairs for
    block i (two pairs for pair-blocks, one for single-blocks)."""

    def __init__(self, nc, mybir, pool, gmpool, spool, table_ap, idx_segs,
                 chunk, pairs, blk_col0, total_blocks, t_s8, ew_ps,
                 qpick, hwq, f32, bf16, fp8):
        self.nc = nc
        self.mybir = mybir
        self.pool = pool
        self.gmpool = gmpool
        self.spool = spool
        self.table_ap = table_ap
        self.idx_segs = idx_segs   # (tile, chunk0, nchunks)
        self.chunk = chunk
        self.pairs = pairs         # True: 2 edges per gathered element
        self.blk_col0 = blk_col0   # column offset into S/ew for block 0
        self.total = total_blocks
        self.t_s8 = t_s8
        self.ew_ps = ew_ps
        self.qpick = qpick
        self.hwq = hwq             # cycles the two HWDGE rings (sync/scalar)
        self.f32 = f32
        self.bf16 = bf16
        self.fp8 = fp8
        self.tiles = []

    def _idx_ap(self, k, cols):
        for t, c0, nch in self.idx_segs:
            if c0 <= k < c0 + nch:
                off = (k - c0) * 8 * self.chunk
                return t[:, off:off + cols]
        raise AssertionError(k)

    def _emit_chunk(self, k):
        nc = self.nc
        P_ = P
        nblk = min(self.chunk, self.total - k * self.chunk)
        g = self.pool.tile([P_, nblk, 2 * D], self.bf16)
        n = P_ * nblk
        nc.gpsimd.dma_gather(
            g[:], self.table_ap, self._idx_ap(k, n // 16), n, n, 2 * D,
            queue_num=self.qpick(n), single_packet=False)
        if self.pairs:
            b0 = self.blk_col0 + 2 * self.chunk * k
            ncol = 2 * nblk
        else:
            b0 = self.blk_col0 + self.chunk * k
            ncol = nblk
        if self.pairs:
            # scale both halves: view [P, nblk, 2, D], ew col per half
            gm = self.gmpool.tile([P_, nblk, 2 * D], self.bf16)
            nc.vector.tensor_tensor(
                out=gm[:].rearrange("p k (h d) -> p (k h) d", h=2),
                in0=g[:].rearrange("p k (h d) -> p (k h) d", h=2),
                in1=self.ew_ps[:, b0:b0 + ncol].to_broadcast(
                    [P_, ncol, D]),
                op=self.mybir.AluOpType.mult)
        else:
            gm = self.gmpool.tile([P_, nblk, D], self.bf16)
            nc.vector.tensor_tensor(
                out=gm[:],
                in0=g[:, :, 0:D],
                in1=self.ew_ps[:, b0:b0 + ncol].to_broadcast(
                    [P_, ncol, D]),
                op=self.mybir.AluOpType.mult)
        S = self.spool.tile([P_, ncol, P_], self.fp8)
        self.hwq().dma_start(
            out=S[:], in_=self.t_s8[:, b0 * P_:(b0 + ncol) * P_])
        self.tiles.append((S, gm))

    def block(self, i):
        k, off = divmod(i, self.chunk)
        while len(self.tiles) <= k:
            self._emit_chunk(len(self.tiles))
        S, gm = self.tiles[k]
        if self.pairs:
            return [(S[:, 2 * off, :], gm[:, off, 0:D]),
                    (S[:, 2 * off + 1, :], gm[:, off, D:2 * D])]
        return [(S[:, off, :], gm[:, off, :])]


def _build_program(pcols, scols, acols):
    key = (pcols, scols, acols)
    if key in _PROGRAM_CACHE:
        return _PROGRAM_CACHE[key]

    from concourse import bacc
    import concourse.mybir as mybir
    import concourse.tile as tile

    nc = bacc.Bacc("TRN2", num_swdge_queues=NQ)
    f32 = mybir.dt.float32
    bf16 = mybir.dt.bfloat16
    fp8 = mybir.dt.float8e4
    t_stab = nc.dram_tensor("stab", [NPAD, 2 * D], bf16,
                            kind="ExternalInput")
    t_ptab = nc.dram_tensor("ptab", [NPAIR_ENT, 2 * D], bf16,
                            kind="ExternalInput")
    t_pidx = nc.dram_tensor("pidx", [P, pcols], mybir.dt.int16,
                            kind="ExternalInput")
    t_sidx = nc.dram_tensor("sidx", [P, scols], mybir.dt.int16,
                            kind="ExternalInput")
    t_s8 = nc.dram_tensor("s8", [P, (TBP * 2 + TBS) * P], fp8,
                          kind="ExternalInput")
    t_ew_p = nc.dram_tensor("ew_p", [P, TBP * 2], f32,
                            kind="ExternalInput")
    t_ew_s = nc.dram_tensor("ew_s", [P, TBS], f32, kind="ExternalInput")
    t_emb = nc.dram_tensor("emb", [P, SLOTS * D], f32,
                           kind="ExternalInput")
    t_wt = nc.dram_tensor("wt", [D, D], bf16, kind="ExternalInput")
    t_b = nc.dram_tensor("bias", [1, D], bf16, kind="ExternalInput")
    t_ident = nc.dram_tensor("ident", [P, P], bf16, kind="ExternalInput")
    t_out = nc.dram_tensor("out", [SLOTS * P, D], f32,
                           kind="ExternalOutput")

    qload = [0] * NQ

    def qpick_n(n):
        q = min(range(NQ), key=lambda i: qload[i])
        qload[q] += n
        return q

    ncp = -(-TBP // CHUNK_P)
    nca = -(-TBA // CHUNK_S)
    ncb = -(-TBB // CHUNK_S)

    with tile.TileContext(nc) as tc:
        with tc.tile_pool(name="const", bufs=1) as cpool, \
             tc.tile_pool(name="gp", bufs=8) as gppool, \
             tc.tile_pool(name="ga", bufs=6) as gapool, \
             tc.tile_pool(name="gb", bufs=6) as gbpool, \
             tc.tile_pool(name="gmp", bufs=4) as gmppool, \
             tc.tile_pool(name="gma", bufs=4) as gmapool, \
             tc.tile_pool(name="gmb", bufs=4) as gmbpool, \
             tc.tile_pool(name="sp", bufs=4) as sppool, \
             tc.tile_pool(name="sa", bufs=4) as sapool, \
             tc.tile_pool(name="sb", bufs=4) as sbpool, \
             tc.tile_pool(name="small", bufs=3) as mpool, \
             tc.tile_pool(name="pscst", bufs=1, space="PSUM") as pscst, \
             tc.tile_pool(name="psnh", bufs=3, space="PSUM") as psnh, \
             tc.tile_pool(name="psxt", bufs=1, space="PSUM") as psxt, \
             tc.tile_pool(name="psout", bufs=1, space="PSUM") as psout:
            # idx group tiles (chunk-aligned) so the first gathers only
            # wait on their own small DMA
            def load_idx(tensor, nch, chunk, tag):
                segs = []
                ngrp = min(4, nch) or 1
                for gidx in range(ngrp):
                    lo = nch * gidx // ngrp
                    hi = nch * (gidx + 1) // ngrp
                    if hi == lo:
                        continue
                    w = (hi - lo) * 8 * chunk
                    tgt = cpool.tile([P, w], mybir.dt.int16,
                                     tag=f"{tag}{lo}")
                    nc.sync.dma_start(
                        out=tgt[:],
                        in_=tensor[:, lo * 8 * chunk:lo * 8 * chunk + w])
                    segs.append((tgt, lo, hi - lo))
                return segs

            psegs = load_idx(t_pidx, ncp, CHUNK_P, "pi")
            asegs = load_idx(t_sidx[:, 0:acols], nca, CHUNK_S, "ai")
            bsegs = load_idx(t_sidx[:, acols:scols], ncb, CHUNK_S, "bi")

            # ew staged in SBUF then copied into PSUM so DVE reads it
            # via its PSUM path (no shared-SBUF-port lock vs SWDGE)
            ew_st = cpool.tile([P, TBP * 2 + TBS], f32)
            nc.sync.dma_start(out=ew_st[:, 0:TBP * 2], in_=t_ew_p[:])
            nc.sync.dma_start(out=ew_st[:, TBP * 2:], in_=t_ew_s[:])
            ew_ps = pscst.tile([P, TBP * 2 + TBS], f32, space="PSUM",
                               padded_shape=[P, 1024])
            nc.vector.tensor_copy(out=ew_ps[:], in_=ew_st[:])

            hwstate = [0]

            def hwq():
                hwstate[0] += 1
                return nc.sync if hwstate[0] % 2 else nc.scalar

            sp = _Stream(nc, mybir, gppool, gmppool, sppool, t_ptab[:, :],
                         psegs, CHUNK_P, True, 0, TBP, t_s8, ew_ps,
                         qpick_n, hwq, f32, bf16, fp8)
            sa = _Stream(nc, mybir, gapool, gmapool, sapool,
                         t_stab[0:HALF, :], asegs, CHUNK_S, False,
                         TBP * 2, TBA, t_s8, ew_ps, qpick_n, hwq,
                         f32, bf16, fp8)
            sb = _Stream(nc, mybir, gbpool, gmbpool, sbpool,
                         t_stab[HALF:NPAD, :], bsegs, CHUNK_S, False,
                         TBP * 2 + TBA, TBB, t_s8, ew_ps, qpick_n, hwq,
                         f32, bf16, fp8)
            # prime the pipeline before queueing the big constant loads so
            # the first gathers / S loads are not stuck behind them
            sp.block(0)
            sa.block(0)
            sb.block(0)

            ident_sb = cpool.tile([P, P], bf16)
            nc.scalar.dma_start(out=ident_sb[:], in_=t_ident[:])
            ones = cpool.tile([1, P], bf16)
            nc.vector.memset(ones[:], 1.0)
            wt_sb = cpool.tile([D, D], bf16)
            nc.scalar.dma_start(out=wt_sb[:], in_=t_wt[:])
            b_sb = cpool.tile([1, D], bf16)
            nc.scalar.dma_start(out=b_sb[:], in_=t_b[:])
            emb_sb = cpool.tile([P, SLOTS * D], f32)
            for i in range(4):
                lo = SLOTS * D * i // 4
                hi = SLOTS * D * (i + 1) // 4
                hwq().dma_start(out=emb_sb[:, lo:hi], in_=t_emb[:, lo:hi])

            p_off = a_off = b_off = 0
            for s in range(SLOTS):
                pb, ab, bb = PROF[s]
                mms = []
                for j in range(pb):
                    mms += sp.block(p_off + j)
                for j in range(ab):
                    mms += sa.block(a_off + j)
                for j in range(bb):
                    mms += sb.block(b_off + j)
                p_off += pb
                a_off += ab
                b_off += bb
                x_sb = mpool.tile([P, D], bf16, tag="x")
                nh = psnh.tile([P, D], f32, space="PSUM", tag="nh",
                               padded_shape=[P, 512])
                for i, (lhsT, rhs) in enumerate(mms):
                    nc.tensor.matmul(out=nh[:], lhsT=lhsT, rhs=rhs,
                                     start=(i == 0),
                                     stop=(i == len(mms) - 1))
                nc.vector.tensor_add(out=x_sb[:], in0=nh[:],
                                     in1=emb_sb[:, s * D:(s + 1) * D])
                xT_ps = psxt.tile([D, P], bf16, space="PSUM", tag="xt",
                                  padded_shape=[D, 1024])
                nc.tensor.matmul(out=xT_ps[:], lhsT=x_sb[:],
                                 rhs=ident_sb[:], is_transpose=True)
                xT_sb = mpool.tile([D, P], bf16, tag="xts")
                nc.vector.tensor_copy(out=xT_sb[:], in_=xT_ps[:])
                o_ps = psout.tile([P, D], f32, space="PSUM", tag="ops",
                                  padded_shape=[P, 512])
                nc.tensor.matmul(out=o_ps[:], lhsT=xT_sb[:], rhs=wt_sb[:],
                                 start=True, stop=False)
                nc.tensor.matmul(out=o_ps[:], lhsT=ones[:], rhs=b_sb[:],
                                 start=False, stop=True)
                o_sb = mpool.tile([P, D], f32, tag="osb")
                nc.scalar.activation(
                    out=o_sb[:], in_=o_ps[:],
                    func=mybir.ActivationFunctionType.Lrelu, alpha=0.01)
                nc.sync.dma_start(out=t_out[s * P:(s + 1) * P, :],
                                  in_=o_sb[:])

    nc.compile()
    _PROGRAM_CACHE[key] = nc
    return nc


LAST_RESULTS = None


def kernel(entity_embed, src, dst, edge_weight, out_sqrt_degree,
           in_sqrt_degree, W, b):
    _install_fixups()
    from concourse.bass_utils import run_bass_kernel_spmd

    bf16 = _bf16()
    entity_embed = np.asarray(entity_embed, np.float32)
    src = np.asarray(src).astype(np.int64)
    dst = np.asarray(dst).astype(np.int64)
    edge_weight = np.asarray(edge_weight, np.float32)
    out_sqrt_degree = np.asarray(out_sqrt_degree, np.float32)
    in_sqrt_degree = np.asarray(in_sqrt_degree, np.float32)
    W = np.asarray(W, np.float32)
    b = np.asarray(b, np.float32)

    (stab, ptabs, pidx_rep, sidx_rep, s8_all, ew_p, ew_s, emb_all,
     deal, tiles, pcols, scols, acols) = _prepare(
        entity_embed, src, dst, edge_weight, out_sqrt_degree,
        in_sqrt_degree)

    nc = _build_program(pcols, scols, acols)

    wt = np.ascontiguousarray(W.T).astype(bf16)     # rhs[k, j] = W[j, k]
    ident_np = np.eye(P, dtype=np.float32).astype(bf16)
    in_maps = []
    for c in range(N_CORES):
        in_maps.append({
            "stab": stab,
            "ptab": ptabs[c],
            "pidx": np.ascontiguousarray(pidx_rep[c]),
            "sidx": np.ascontiguousarray(sidx_rep[c]),
            "s8": s8_all[c],
            "ew_p": np.ascontiguousarray(ew_p[c]),
            "ew_s": np.ascontiguousarray(ew_s[c]),
            "emb": np.ascontiguousarray(emb_all[c]),
            "wt": wt,
            "bias": b[None, :].astype(bf16),
            "ident": ident_np,
        })

    try:
        res = run_bass_kernel_spmd(nc, in_maps,
                                   core_ids=list(range(N_CORES)))
    except Exception:
        # Transient NRT_EXEC_UNIT_UNRECOVERABLE states have been observed;
        # a reset + retry recovers them.
        import os
        import time
        os.environ["NEURON_RT_RESET_CORES"] = "1"
        time.sleep(30)
        res = run_bass_kernel_spmd(nc, in_maps,
                                   core_ids=list(range(N_CORES)))
    global LAST_RESULTS
    LAST_RESULTS = res

    out = np.empty((NPAD, D), np.float32)
    for c in range(N_CORES):
        oc = res.results[c]["out"]
        for s in range(SLOTS):
            out[tiles[deal[c, s]]] = oc[s * P:(s + 1) * P]
    return out[:N_NODES]


# revision 16
# speedup vs baseline: 2.0065x; 1.1025x over previous
"""GNN message-passing aggregator on 8 Trainium2 NeuronCores.

Computes, for the full graph:
    node = entity_embed * out_sqrt_degree
    msg  = node[src] * edge_weight
    N_h  = segment_sum(msg, dst, N) * in_sqrt_degree
    out  = leaky_relu((entity_embed + N_h) @ W.T + b, 0.01)

Strategy (dst-partitioned edge shard, no collectives).  The kernel is
bound by SWDGE dma_gather descriptor generation on the Q7 cores (~6ns
per descriptor per queue, 4 queues), so the host-side layout packs TWO
edges into every 256-byte gather element (two bf16 node rows) and keeps
every other engine under that wall:

  * Node re-tiling: nodes are assigned to 392 tiles of 128 so each
    tile's in-edge count is EXACTLY 2048 (381 tiles) or 1792 (11) —
    snake-deal by degree + swap repair.  Zero block padding, and every
    core runs an identical SPMD program (tiles dealt 8 per slot).
  * Paired node table (rho stream): per core, the 50176-row node table
    is laid out as 25088 two-row entries, where the pairing is a
    per-core matching that puts two nodes in one entry iff both have an
    edge into the same tile — so one descriptor feeds two edges.  A
    round-robin greedy matching guarantees 384 such pairs per tile.
  * Pair table (ptab stream): the remaining 1280 edges per tile are
    paired arbitrarily via an explicit 31232-entry two-row table
    (bounded by int16 indexing, <=1.25x the node table).
  * Every edge is covered by a two-edge descriptor: 384 + 640 = 1024
    descriptors per 2048-edge tile, 50048 per core for 100k edges.
  * Device, per tile: the one-hot S[e, n] (fp8, exact) is precomputed on
    the host from the index structure and streamed in by HWDGE DMA;
    messages gm = bf16(g * ew) on the DVE (the ew broadcast lives in
    PSUM so the op never takes the DVE/GpSimd shared SBUF port, which
    would lock the SWDGE generators out of SBUF); and
    nh[n, :] += S.T @ gm on the PE with S as the 128-column stationary
    operand (fast-weight-load path), two 64-column matmuls per block
    (one per element half).
  * Epilogue per tile: x = embed + nh (bf16), transpose x via the PE
    (identity matmul), out = Lrelu(xT.T @ W.T + b) on the ACT engine.
  * Gathers are chunked 16 blocks (2048 descriptors) per op and spread
    over 4 SWDGE queues by least-loaded assignment
    (single_packet=False lifts the 64-descriptor packet cap).
"""

import json
import sys
import types

import numpy as np

P = 128
D = 64
N_NODES = 50000
N_CORES = 8
HALF = 25088
NPAD = 2 * HALF         # 50176 = 392 tiles
NT = NPAD // P          # 392
SLOTS = NT // N_CORES   # 49
CHUNK = 16              # blocks per dma_gather
NQ = 4                  # SWDGE queues (Q7 core pairs)

RHO_B = 3                                  # rho (paired-table) blocks/slot
PTAB_B = [4] + [5] * (SLOTS - 1)           # ptab blocks per slot
TBR = RHO_B * SLOTS                        # 147 rho blocks per core
TBPT = sum(PTAB_B)                         # 244 ptab blocks per core
NPAIR_ENT = TBPT * P                       # 31232 ptab entries (< 2^15)
NCOLS = 2 * (TBPT + TBR)                   # S/ew half-columns (782)


# ----------------------------------------------------------------------------
# Environment fixups (self-contained; kernel.py must run alone).
# ----------------------------------------------------------------------------

_SPLIT_COUNT = 0


def _split_multi_waits_json(bir: bytes) -> bytes:
    """This container's walrus accepts only ONE sync wait per instruction
    ('Too many sync wait commands'), while Tile's scheduler attaches
    several.  Rewrite each instruction with N>1 waits into N-1 same-engine
    NoOps (one wait each) followed by the instruction with the last wait;
    same-engine sequencer order makes this equivalent."""
    global _SPLIT_COUNT
    d = json.loads(bir)
    changed = False
    for fn in d.get("functions", []):
        for bb in fn.get("blocks", []):
            out = []
            for ins in bb.get("instructions", []):
                si = ins.get("sync_info") or {}
                ow = si.get("on_wait") or []
                if len(ow) > 1:
                    changed = True
                    for w in ow[:-1]:
                        _SPLIT_COUNT += 1
                        out.append({
                            "opcode": "NoOp",
                            "engine": ins.get("engine", "Unassigned"),
                            "name": f"I-waitsplit-{_SPLIT_COUNT}",
                            "ins": [],
                            "outs": [],
                            "sync_info": {"on_update": [], "on_wait": [w]},
                        })
                    si["on_wait"] = [ow[-1]]
                out.append(ins)
            bb["instructions"] = out
    return json.dumps(d).encode() if changed else bir


def _install_fixups():
    import concourse.bass_utils as bass_utils
    import concourse.bass2jax as bass2jax

    if not getattr(bass_utils, "_waitsplit_installed", False):
        bass_utils._waitsplit_installed = True
        orig_compile = bass_utils.compile_bir_kernel

        def patched_compile(bir_json, tmpdir, neff_name="file.neff"):
            if isinstance(bir_json, str):
                bir_json = bir_json.encode()
            return orig_compile(_split_multi_waits_json(bir_json), tmpdir,
                                neff_name=neff_name)

        bass_utils.compile_bir_kernel = patched_compile
        bass2jax.compile_bir_kernel = patched_compile
        # No artifact bucket in this container; keep profiles local.
        bass_utils.upload_artifacts = lambda tmpdir: tmpdir

    # run_bass_kernel_spmd(trace=True) under axon needs antenv.axon_hooks,
    # which this image doesn't ship.  Synthesize it and install the ctypes
    # NTFF hook from trn_agent_boot so neuron-profile works.
    if "antenv.axon_hooks" not in sys.modules:
        m = types.ModuleType("antenv.axon_hooks")
        m._hook = None
        m.set_axon_ntff_profile_hook = lambda h: setattr(m, "_hook", h)
        m.get_axon_ntff_profile_hook = lambda: m._hook
        sys.modules["antenv.axon_hooks"] = m
        try:
            import antenv
            antenv.axon_hooks = m
        except ImportError:
            pass
        try:
            from trn_agent_boot.trn_boot import _ntff_profile_via_ctypes
            hook = _ntff_profile_via_ctypes("/opt/axon/libaxon_pjrt.so")
            if hook is not None:
                m._hook = hook
        except Exception:
            pass


# ----------------------------------------------------------------------------
# Host-side graph partitioning
# ----------------------------------------------------------------------------

def _bf16():
    from ml_dtypes import bfloat16
    return bfloat16


def _fp8():
    from ml_dtypes import float8_e4m3
    return float8_e4m3


def _wrap16(seg: np.ndarray) -> np.ndarray:
    """dma_gather index layout: index i lives at [i % 16, i // 16]."""
    assert seg.size % 16 == 0
    return seg.reshape(-1, 16).T


def _rebin(dst):
    """Assign nodes to 392 tiles of 128 nodes with per-tile in-edge sums
    of exactly 2048 (first 381 tiles) or 1792 (last 11): snake-deal the
    degree-sorted nodes, then repair residues with degree-delta swaps."""
    from collections import defaultdict
    deg = np.bincount(dst, minlength=NPAD).astype(np.int64)
    targets = np.array([2048] * 381 + [1792] * 11, np.int64)
    assert targets.sum() == deg.sum()
    order = np.argsort(-deg, kind="stable")
    bins = [[] for _ in range(NT)]
    for r in range(P):
        row = order[r * NT:(r + 1) * NT]
        seq = range(NT) if r % 2 == 0 else range(NT - 1, -1, -1)
        for k, t in enumerate(seq):
            bins[t].append(int(row[k]))
    sums = np.array([deg[np.array(b)].sum() for b in bins], np.int64)
    order_bins = np.argsort(sums)
    targets_of = np.full(NT, 2048, np.int64)
    targets_of[order_bins[:11]] = 1792
    diff = sums - targets_of
    assert diff.sum() == 0

    maps = []
    for t in range(NT):
        m = defaultdict(list)
        for n in bins[t]:
            m[deg[n]].append(n)
        maps.append(m)
    it = 0
    while diff.any():
        it += 1
        assert it < 200000, "rebin repair did not converge"
        i = int(np.argmax(diff))
        j = int(np.argmin(diff))
        want = int(min(diff[i], -diff[j]))
        done = False
        for delta in range(want, 0, -1):
            for da in sorted(maps[i].keys(), reverse=True):
                db = da - delta
                if db >= 0 and maps[j].get(db):
                    a = maps[i][da].pop()
                    if not maps[i][da]:
                        del maps[i][da]
                    b = maps[j][db].pop()
                    if not maps[j][db]:
                        del maps[j][db]
                    bins[i].remove(a)
                    bins[j].remove(b)
                    bins[i].append(b)
                    bins[j].append(a)
                    maps[i][db].append(b)
                    maps[j][da].append(a)
                    diff[i] -= delta
                    diff[j] += delta
                    done = True
                    break
            if done:
                break
        assert done, (i, j, diff[i], diff[j])
    tiles = [sorted(b) for b in bins]
    big = [t for t in range(NT) if targets_of[t] == 2048]
    small = [t for t in range(NT) if targets_of[t] == 1792]
    tiles = np.array([tiles[t] for t in big + small])
    for t in range(NT):
        assert deg[tiles[t]].sum() == targets[t]
    return tiles


def _match(core_slots, src, etile):
    """Per core: pick 384 node pairs per tile (both nodes have an edge
    into the tile; each node used once per core) by round-robin greedy,
    then extend to a full pairing of all NPAD nodes (the per-core stab
    layout).  Returns (pairs_per_slot, entries, ent_of, half_of)."""
    NEED = 384
    pools = []
    for t in core_slots:
        es = np.where(etile == t)[0]
        pools.append(list(np.unique(src[es])))
    used = np.zeros(NPAD, bool)
    pairs = [[] for _ in range(SLOTS)]
    ptr = [0] * SLOTS
    progress = True
    while progress and min(len(p) for p in pairs) < NEED:
        progress = False
        for k in range(SLOTS):
            if len(pairs[k]) >= NEED:
                continue
            grabbed = 0
            first = None
            while grabbed < 128 and ptr[k] < len(pools[k]) \
                    and len(pairs[k]) < NEED:
                n = int(pools[k][ptr[k]])
                ptr[k] += 1
                if used[n]:
                    continue
                used[n] = True
                if first is None:
                    first = n
                else:
                    pairs[k].append((first, n))
                    first = None
                    grabbed += 2
                    progress = True
            if first is not None:
                used[first] = False
                ptr[k] -= 1
    assert min(len(p) for p in pairs) >= NEED, [len(p) for p in pairs]
    entries = []
    for k in range(SLOTS):
        pairs[k] = pairs[k][:NEED]
        entries.extend(pairs[k])
    left = np.where(~used)[0]
    assert (len(entries) * 2 + len(left)) == NPAD
    for i in range(0, len(left), 2):
        entries.append((int(left[i]), int(left[i + 1])))
    assert len(entries) == HALF
    ent_of = np.zeros(NPAD, np.int64)
    half_of = np.zeros(NPAD, np.int64)
    for i, (a, b) in enumerate(entries):
        ent_of[a] = i
        half_of[a] = 0
        ent_of[b] = i
        half_of[b] = 1
    return pairs, np.array(entries, np.int64), ent_of, half_of


def _prepare(entity_embed, src, dst, edge_weight, out_sqrt_degree,
             in_sqrt_degree):
    f32 = np.float32
    bf16 = _bf16()
    fp8 = _fp8()
    node = (entity_embed * out_sqrt_degree).astype(f32)
    node_bf = np.zeros((NPAD, D), bf16)
    node_bf[:N_NODES] = node.astype(bf16)
    emb_pad = np.zeros((NPAD, D), f32)
    emb_pad[:N_NODES] = entity_embed.astype(f32)
    ew2 = (edge_weight[:, 0] * in_sqrt_degree[dst, 0]).astype(f32)

    tiles = _rebin(dst)          # [392, 128] node ids; big tiles first
    tile_of_node = np.zeros(NPAD, np.int64)
    pos_of_node = np.zeros(NPAD, np.int64)
    for t in range(NT):
        tile_of_node[tiles[t]] = t
        pos_of_node[tiles[t]] = np.arange(P)

    # Deal: slot 0 <- small tiles 381..388 (one per core); stray smalls
    # 389..391 ride in big slots on cores 0..2 (padded); bigs fill in.
    deal = np.zeros((N_CORES, SLOTS), np.int64)
    deal[:, 0] = np.arange(381, 389)
    nxt = [1] * N_CORES
    for i, t in enumerate(range(389, 392)):
        deal[i, 1] = t
        nxt[i] = 2
    bi = 0
    for c in range(N_CORES):
        while nxt[c] < SLOTS:
            deal[c, nxt[c]] = bi
            bi += 1
            nxt[c] += 1
    assert bi == 381
    assert sorted(deal.ravel().tolist()) == list(range(NT))

    etile = tile_of_node[dst]

    ncr = -(-TBR // CHUNK)              # rho chunks
    ncp = -(-TBPT // CHUNK)             # ptab chunks
    rcols = 8 * CHUNK * ncr
    pcols = 8 * CHUNK * ncp

    pidx_all = np.zeros((N_CORES, 16, pcols), np.int16)
    ridx_all = np.zeros((N_CORES, 16, rcols), np.int16)
    ew_all = np.zeros((N_CORES, P, NCOLS), f32)
    dstl_all = np.zeros((N_CORES, P, NCOLS), np.int64)
    live = np.zeros((N_CORES, P, NCOLS), bool)
    emb_all = np.zeros((N_CORES, P, SLOTS * D), f32)
    ptab_u = np.zeros((N_CORES, NPAIR_ENT), np.int64)
    ptab_v = np.zeros((N_CORES, NPAIR_ENT), np.int64)
    stab_entries = []

    pb0 = np.concatenate([[0], np.cumsum(PTAB_B)])

    for c in range(N_CORES):
        pairs, entries, ent_of, half_of = _match(deal[c], src, etile)
        stab_entries.append(entries)
        for s in range(SLOTS):
            t = deal[c, s]
            es = np.where(etile == t)[0]
            order = np.argsort(src[es], kind="stable")
            es = es[order]
            srcs = src[es]
            starts = {}
            counts = {}
            for i, u in enumerate(srcs):
                u = int(u)
                if u not in starts:
                    starts[u] = i
                    counts[u] = 0
                counts[u] += 1
            taken = {u: 0 for u in starts}

            def pop(u):
                i = starts[u] + taken[u]
                taken[u] += 1
                return es[i]

            # rho descriptors: 384 matched pairs, halves in entry order
            rk0 = 2 * TBPT + 2 * RHO_B * s      # ew/dstl column base
            ents = np.zeros(RHO_B * P, np.int64)
            for j, (a, b) in enumerate(pairs[s]):
                ea = pop(a)
                eb = pop(b)
                ents[j] = ent_of[a]
                blk, p_ = divmod(j, P)
                col = rk0 + 2 * blk
                ha, hb = half_of[a], half_of[b]
                dstl_all[c, p_, col + ha] = pos_of_node[dst[ea]]
                ew_all[c, p_, col + ha] = ew2[ea]
                live[c, p_, col + ha] = True
                dstl_all[c, p_, col + hb] = pos_of_node[dst[eb]]
                ew_all[c, p_, col + hb] = ew2[eb]
                live[c, p_, col + hb] = True
            ci = 8 * RHO_B * s
            ridx_all[c, :, ci:ci + RHO_B * P // 16] = _wrap16(
                ents.astype(np.int16))

            # remaining instances -> ptab pairs
            rest = []
            for u in starts:
                while taken[u] < counts[u]:
                    rest.append(pop(u))
            rest = np.array(rest, np.int64)
            npp = len(rest) // 2
            assert len(rest) % 2 == 0 and npp <= PTAB_B[s] * P, \
                (len(rest), PTAB_B[s])
            e1 = rest[0::2]
            e2 = rest[1::2]
            ent0 = pb0[s] * P
            ptab_u[c, ent0:ent0 + npp] = src[e1]
            ptab_v[c, ent0:ent0 + npp] = src[e2]
            ents = np.zeros(PTAB_B[s] * P, np.int64)
            ents[:npp] = np.arange(ent0, ent0 + npp)
            ci = 8 * pb0[s]
            pidx_all[c, :, ci:ci + PTAB_B[s] * P // 16] = _wrap16(
                ents.astype(np.int16))
            pk0 = 2 * pb0[s]
            for j in range(npp):
                blk, p_ = divmod(j, P)
                col = pk0 + 2 * blk
                dstl_all[c, p_, col] = pos_of_node[dst[e1[j]]]
                ew_all[c, p_, col] = ew2[e1[j]]
                live[c, p_, col] = True
                dstl_all[c, p_, col + 1] = pos_of_node[dst[e2[j]]]
                ew_all[c, p_, col + 1] = ew2[e2[j]]
                live[c, p_, col + 1] = True

            emb_all[c, :, s * D:(s + 1) * D] = emb_pad[tiles[t]]

    # S8 one-hot in fp8 (exact): S8[p, col*128 + dstl] = 1 where live
    s8_all = np.zeros((N_CORES, P, NCOLS * P), fp8)
    one = fp8(1.0)
    for c in range(N_CORES):
        pp, cc = np.where(live[c])
        s8_all[c][pp, cc * P + dstl_all[c][pp, cc]] = one

    # tables (bf16 rows, 256B two-row entries)
    ptabs = []
    stabs = []
    for c in range(N_CORES):
        pt = np.zeros((NPAIR_ENT, 2 * D), bf16)
        pt[:, :D] = node_bf[ptab_u[c]]
        pt[:, D:] = node_bf[ptab_v[c]]
        ptabs.append(pt)
        st = np.zeros((HALF, 2 * D), bf16)
        st[:, :D] = node_bf[stab_entries[c][:, 0]]
        st[:, D:] = node_bf[stab_entries[c][:, 1]]
        stabs.append(st)

    pidx_rep = np.tile(pidx_all, (1, 8, 1))
    ridx_rep = np.tile(ridx_all, (1, 8, 1))
    return (stabs, ptabs, pidx_rep, ridx_rep, s8_all, ew_all, emb_all,
            deal, tiles, pcols, rcols)


# ----------------------------------------------------------------------------
# Device program
# ----------------------------------------------------------------------------

_PROGRAM_CACHE = {}


class _Stream:
    """Lazily emits chunked dma_gathers over one concatenated block
    stream of two-edge 256B elements.  Per chunk also emits ONE batched
    edge-weight multiply (DVE) and ONE S-tile load (HWDGE).  block(i)
    yields the two (lhsT, rhs) matmul operand pairs for block i."""

    def __init__(self, nc, mybir, pool, gmpool, spool, table_ap, idx_segs,
                 blk_col0, total_blocks, t_s8, ew_ps, qpick, hwq,
                 bf16, fp8):
        self.nc = nc
        self.mybir = mybir
        self.pool = pool
        self.gmpool = gmpool
        self.spool = spool
        self.table_ap = table_ap
        self.idx_segs = idx_segs   # (tile, chunk0, nchunks)
        self.blk_col0 = blk_col0   # half-column offset for block 0
        self.total = total_blocks
        self.t_s8 = t_s8
        self.ew_ps = ew_ps
        self.qpick = qpick
        self.hwq = hwq
        self.bf16 = bf16
        self.fp8 = fp8
        self.tiles = []

    def _idx_ap(self, k, cols):
        for t, c0, nch in self.idx_segs:
            if c0 <= k < c0 + nch:
                off = (k - c0) * 8 * CHUNK
                return t[:, off:off + cols]
        raise AssertionError(k)

    def _emit_chunk(self, k):
        nc = self.nc
        nblk = min(CHUNK, self.total - k * CHUNK)
        g = self.pool.tile([P, nblk, 2 * D], self.bf16)
        n = P * nblk
        nc.gpsimd.dma_gather(
            g[:], self.table_ap, self._idx_ap(k, n // 16), n, n, 2 * D,
            queue_num=self.qpick(n), single_packet=False)
        b0 = self.blk_col0 + 2 * CHUNK * k
        ncol = 2 * nblk
        gm = self.gmpool.tile([P, nblk, 2 * D], self.bf16)
        nc.vector.tensor_tensor(
            out=gm[:].rearrange("p k (h d) -> p (k h) d", h=2),
            in0=g[:].rearrange("p k (h d) -> p (k h) d", h=2),
            in1=self.ew_ps[:, b0:b0 + ncol].to_broadcast([P, ncol, D]),
            op=self.mybir.AluOpType.mult)
        S = self.spool.tile([P, ncol, P], self.fp8)
        self.hwq().dma_start(
            out=S[:], in_=self.t_s8[:, b0 * P:(b0 + ncol) * P])
        self.tiles.append((S, gm))

    def block(self, i):
        k, off = divmod(i, CHUNK)
        while len(self.tiles) <= k:
            self._emit_chunk(len(self.tiles))
        S, gm = self.tiles[k]
        return [(S[:, 2 * off, :], gm[:, off, 0:D]),
                (S[:, 2 * off + 1, :], gm[:, off, D:2 * D])]


def _build_program(pcols, rcols):
    key = (pcols, rcols)
    if key in _PROGRAM_CACHE:
        return _PROGRAM_CACHE[key]

    from concourse import bacc
    import concourse.mybir as mybir
    import concourse.tile as tile

    nc = bacc.Bacc("TRN2", num_swdge_queues=NQ)
    f32 = mybir.dt.float32
    bf16 = mybir.dt.bfloat16
    fp8 = mybir.dt.float8e4
    t_stab = nc.dram_tensor("stab", [HALF, 2 * D], bf16,
                            kind="ExternalInput")
    t_ptab = nc.dram_tensor("ptab", [NPAIR_ENT, 2 * D], bf16,
                            kind="ExternalInput")
    t_pidx = nc.dram_tensor("pidx", [P, pcols], mybir.dt.int16,
                            kind="ExternalInput")
    t_ridx = nc.dram_tensor("ridx", [P, rcols], mybir.dt.int16,
                            kind="ExternalInput")
    t_s8 = nc.dram_tensor("s8", [P, NCOLS * P], fp8, kind="ExternalInput")
    t_ew = nc.dram_tensor("ew", [P, NCOLS], f32, kind="ExternalInput")
    t_emb = nc.dram_tensor("emb", [P, SLOTS * D], f32,
                           kind="ExternalInput")
    t_wt = nc.dram_tensor("wt", [D, D], bf16, kind="ExternalInput")
    t_b = nc.dram_tensor("bias", [1, D], bf16, kind="ExternalInput")
    t_ident = nc.dram_tensor("ident", [P, P], bf16, kind="ExternalInput")
    t_out = nc.dram_tensor("out", [SLOTS * P, D], f32,
                           kind="ExternalOutput")

    qload = [0] * NQ

    def qpick_n(n):
        q = min(range(NQ), key=lambda i: qload[i])
        qload[q] += n
        return q

    ncr = -(-TBR // CHUNK)
    ncp = -(-TBPT // CHUNK)

    with tile.TileContext(nc) as tc:
        with tc.tile_pool(name="const", bufs=1) as cpool, \
             tc.tile_pool(name="gp", bufs=8) as gppool, \
             tc.tile_pool(name="gr", bufs=6) as grpool, \
             tc.tile_pool(name="gmp", bufs=4) as gmppool, \
             tc.tile_pool(name="gmr", bufs=4) as gmrpool, \
             tc.tile_pool(name="sp", bufs=4) as sppool, \
             tc.tile_pool(name="sr", bufs=4) as srpool, \
             tc.tile_pool(name="small", bufs=3) as mpool, \
             tc.tile_pool(name="pscst", bufs=1, space="PSUM") as pscst, \
             tc.tile_pool(name="psnh", bufs=3, space="PSUM") as psnh, \
             tc.tile_pool(name="psxt", bufs=1, space="PSUM") as psxt, \
             tc.tile_pool(name="psout", bufs=1, space="PSUM") as psout:
            def load_idx(tensor, nch, tag):
                segs = []
                ngrp = min(4, nch) or 1
                for gidx in range(ngrp):
                    lo = nch * gidx // ngrp
                    hi = nch * (gidx + 1) // ngrp
                    if hi == lo:
                        continue
                    w = (hi - lo) * 8 * CHUNK
                    tgt = cpool.tile([P, w], mybir.dt.int16,
                                     tag=f"{tag}{lo}")
                    nc.sync.dma_start(
                        out=tgt[:],
                        in_=tensor[:, lo * 8 * CHUNK:lo * 8 * CHUNK + w])
                    segs.append((tgt, lo, hi - lo))
                return segs

            psegs = load_idx(t_pidx, ncp, "pi")
            rsegs = load_idx(t_ridx, ncr, "ri")

            # ew staged in SBUF then copied into PSUM so DVE reads it
            # via its PSUM path (no shared-SBUF-port lock vs SWDGE)
            ew_st = cpool.tile([P, NCOLS], f32)
            nc.sync.dma_start(out=ew_st[:], in_=t_ew[:])
            ew_ps = pscst.tile([P, NCOLS], f32, space="PSUM",
                               padded_shape=[P, 1024])
            nc.vector.tensor_copy(out=ew_ps[:], in_=ew_st[:])

            hwstate = [0]

            def hwq():
                hwstate[0] += 1
                return nc.sync if hwstate[0] % 2 else nc.scalar

            sp = _Stream(nc, mybir, gppool, gmppool, sppool, t_ptab[:, :],
                         psegs, 0, TBPT, t_s8, ew_ps, qpick_n, hwq,
                         bf16, fp8)
            sr = _Stream(nc, mybir, grpool, gmrpool, srpool, t_stab[:, :],
                         rsegs, 2 * TBPT, TBR, t_s8, ew_ps, qpick_n, hwq,
                         bf16, fp8)
            # prime the pipeline before queueing the big constant loads
            sp.block(0)
            sr.block(0)

            ident_sb = cpool.tile([P, P], bf16)
            nc.scalar.dma_start(out=ident_sb[:], in_=t_ident[:])
            ones = cpool.tile([1, P], bf16)
            nc.vector.memset(ones[:], 1.0)
            wt_sb = cpool.tile([D, D], bf16)
            nc.scalar.dma_start(out=wt_sb[:], in_=t_wt[:])
            b_sb = cpool.tile([1, D], bf16)
            nc.scalar.dma_start(out=b_sb[:], in_=t_b[:])
            emb_sb = cpool.tile([P, SLOTS * D], f32)
            for i in range(4):
                lo = SLOTS * D * i // 4
                hi = SLOTS * D * (i + 1) // 4
                hwq().dma_start(out=emb_sb[:, lo:hi], in_=t_emb[:, lo:hi])

            p_off = r_off = 0
            for s in range(SLOTS):
                mms = []
                for j in range(PTAB_B[s]):
                    mms += sp.block(p_off + j)
                for j in range(RHO_B):
                    mms += sr.block(r_off + j)
                p_off += PTAB_B[s]
                r_off += RHO_B
                x_sb = mpool.tile([P, D], bf16, tag="x")
                nh = psnh.tile([P, D], f32, space="PSUM", tag="nh",
                               padded_shape=[P, 512])
                for i, (lhsT, rhs) in enumerate(mms):
                    nc.tensor.matmul(out=nh[:], lhsT=lhsT, rhs=rhs,
                                     start=(i == 0),
                                     stop=(i == len(mms) - 1))
                nc.vector.tensor_add(out=x_sb[:], in0=nh[:],
                                     in1=emb_sb[:, s * D:(s + 1) * D])
                xT_ps = psxt.tile([D, P], bf16, space="PSUM", tag="xt",
                                  padded_shape=[D, 1024])
                nc.tensor.matmul(out=xT_ps[:], lhsT=x_sb[:],
                                 rhs=ident_sb[:], is_transpose=True)
                xT_sb = mpool.tile([D, P], bf16, tag="xts")
                nc.vector.tensor_copy(out=xT_sb[:], in_=xT_ps[:])
                o_ps = psout.tile([P, D], f32, space="PSUM", tag="ops",
                                  padded_shape=[P, 512])
                nc.tensor.matmul(out=o_ps[:], lhsT=xT_sb[:], rhs=wt_sb[:],
                                 start=True, stop=False)
                nc.tensor.matmul(out=o_ps[:], lhsT=ones[:], rhs=b_sb[:],
                                 start=False, stop=True)
                o_sb = mpool.tile([P, D], f32, tag="osb")
                nc.scalar.activation(
                    out=o_sb[:], in_=o_ps[:],
                    func=mybir.ActivationFunctionType.Lrelu, alpha=0.01)
                nc.sync.dma_start(out=t_out[s * P:(s + 1) * P, :],
                                  in_=o_sb[:])

    nc.compile()
    _PROGRAM_CACHE[key] = nc
    return nc


LAST_RESULTS = None


def kernel(entity_embed, src, dst, edge_weight, out_sqrt_degree,
           in_sqrt_degree, W, b):
    _install_fixups()
    from concourse.bass_utils import run_bass_kernel_spmd

    bf16 = _bf16()
    entity_embed = np.asarray(entity_embed, np.float32)
    src = np.asarray(src).astype(np.int64)
    dst = np.asarray(dst).astype(np.int64)
    edge_weight = np.asarray(edge_weight, np.float32)
    out_sqrt_degree = np.asarray(out_sqrt_degree, np.float32)
    in_sqrt_degree = np.asarray(in_sqrt_degree, np.float32)
    W = np.asarray(W, np.float32)
    b = np.asarray(b, np.float32)

    (stabs, ptabs, pidx_rep, ridx_rep, s8_all, ew_all, emb_all, deal,
     tiles, pcols, rcols) = _prepare(
        entity_embed, src, dst, edge_weight, out_sqrt_degree,
        in_sqrt_degree)

    nc = _build_program(pcols, rcols)

    wt = np.ascontiguousarray(W.T).astype(bf16)     # rhs[k, j] = W[j, k]
    ident_np = np.eye(P, dtype=np.float32).astype(bf16)
    in_maps = []
    for c in range(N_CORES):
        in_maps.append({
            "stab": stabs[c],
            "ptab": ptabs[c],
            "pidx": np.ascontiguousarray(pidx_rep[c]),
            "ridx": np.ascontiguousarray(ridx_rep[c]),
            "s8": s8_all[c],
            "ew": np.ascontiguousarray(ew_all[c]),
            "emb": np.ascontiguousarray(emb_all[c]),
            "wt": wt,
            "bias": b[None, :].astype(bf16),
            "ident": ident_np,
        })

    try:
        res = run_bass_kernel_spmd(nc, in_maps,
                                   core_ids=list(range(N_CORES)))
    except Exception:
        # Transient NRT_EXEC_UNIT_UNRECOVERABLE states have been observed;
        # a reset + retry recovers them.
        import os
        import time
        os.environ["NEURON_RT_RESET_CORES"] = "1"
        time.sleep(30)
        res = run_bass_kernel_spmd(nc, in_maps,
                                   core_ids=list(range(N_CORES)))
    global LAST_RESULTS
    LAST_RESULTS = res

    out = np.empty((NPAD, D), np.float32)
    for c in range(N_CORES):
        oc = res.results[c]["out"]
        for s in range(SLOTS):
            out[tiles[deal[c, s]]] = oc[s * P:(s + 1) * P]
    return out[:N_NODES]
